# revision 39
# baseline (speedup 1.0000x reference)
"""Trainium2 Bass kernel for Llama-like attention (16 heads, tanh softcap, RoPE).

Sharding: tensor-parallel over heads, fully collective-free. Each of the 8
cores computes 2 heads end-to-end and a *partial* output projection
(o_local @ wo_rows_local)^T; the host sums the 8 partial outputs. With no
on-device collective, each core's NEFF span is pure local compute — no
cross-core rendezvous.

Per-core pipeline (engine-balanced against the ~165us PE floor):
  - q^T/k^T computed directly in transposed layout ([head_dim, s]) via
    matmul(w_slice^T, x^T): no PE transposes. Weight columns of wq/wk are
    pre-permuted on the host to de-interleave even/odd rotary pairs (the
    permutation cancels inside q.k).
  - RoPE in transposed layout straight out of PSUM: rot = A*C + swap(A)*S'
    with C = [cosT; cosT], S' = [-sinT; sinT]. The partition-half swap is
    two half-height Vector multiplies reading PSUM at a partition offset
    (Pool cannot touch PSUM); the all-SBUF add runs on Pool. ACT stays
    free for the softmax chain.
  - attention with scores transposed ([kj, qi]) so softmaxed probabilities
    feed the PV matmul directly as the moving operand. tanh softcap bounds
    scores, so softmax needs no row-max pass: p = exp(50*tanh(.)),
    l = ones-row matmul, o = p@v / l. Head 0's ACT-bound window is filled
    with head 1's q/k projection and the tail v chunks; head 1's windows
    are filled with the output-projection pieces for the q-tile that just
    finished.
  - output projection pieces acc[oc(128), st(512)] += wo_h[:, oc]^T @ oT_h
    accumulated over the 2 local heads, copied to SBUF bf16 (ACT/DVE
    alternating) and DMA'd per piece. Host sums partials and transposes.
"""

import os
import sys

for _p in ("/root/.axon_site/_ro/trn_rl_repo", "/opt/trn_rl_repo"):
    if os.path.isdir(_p) and _p not in sys.path:
        sys.path.append(_p)

import numpy as np
import ml_dtypes
from contextlib import ExitStack

import concourse.bass as bass
import concourse.bacc as bacc
import concourse.mybir as mybir
import concourse.tile as tile
from concourse.bass_utils import run_bass_kernel_spmd

BF16 = mybir.dt.bfloat16
F32 = mybir.dt.float32
NPBF16 = ml_dtypes.bfloat16

N_CORES = 8
S = 2048          # sequence length
DM = 2048         # model dim
H = 16            # heads
HD = 128          # head dim
HPC = H // N_CORES  # heads per core = 2
CW = HPC * HD     # per-core projection width = 256
P = 128
HW = HD // 2      # 64
QT = 512          # query tile (free dim of attention matmuls)
NQT = S // QT     # 4 query tiles per head
NSC = S // P      # 16 sequence chunks
NKC = DM // P     # 16 contraction chunks
NST = S // QT     # 4 s-tiles
SOFTCAP = 50.0
C1 = 1.0 / (SOFTCAP * np.sqrt(HD))

Tanh = mybir.ActivationFunctionType.Tanh
Exp = mybir.ActivationFunctionType.Exp


def build_nc(reps=1, single=False):
    nc = bacc.Bacc("TRN2", target_bir_lowering=False, num_devices=N_CORES)

    xT_d = nc.dram_tensor("xT", [DM, S], BF16, kind="ExternalInput")
    w_d = nc.dram_tensor("w_all", [DM, 3 * CW], BF16, kind="ExternalInput")
    wo_d = nc.dram_tensor("wo_c", [CW, DM], BF16, kind="ExternalInput")
    cos_d = nc.dram_tensor("cosT2", [P, S], BF16, kind="ExternalInput")
    sin_d = nc.dram_tensor("sinT2", [P, S], BF16, kind="ExternalInput")
    mask_d = nc.dram_tensor("mask", [P, 4 * QT], BF16, kind="ExternalInput")
    out_d = nc.dram_tensor("outT", [DM, S], BF16, kind="ExternalOutput")

    with tile.TileContext(nc) as tc:
        for _rep in range(reps):
            _emit_body(nc, tc, xT_d, w_d, wo_d, cos_d, sin_d, mask_d, out_d)
    nc.compile()
    return nc


def _emit_body(nc, tc, xT_d, w_d, wo_d, cos_d, sin_d, mask_d, out_d):
    with ExitStack() as ctx:
        # ---------- persistent SBUF ----------
        persist = ctx.enter_context(tc.tile_pool(name="persist", bufs=1))
        qT = [persist.tile([P, S], BF16, name=f"qT{h}") for h in range(HPC)]
        kT = [persist.tile([P, S], BF16, name=f"kT{h}") for h in range(HPC)]
        v_sb = [persist.tile([P, S], BF16, name=f"v{h}") for h in range(HPC)]
        oT = [persist.tile([P, S], BF16, name=f"oT{h}") for h in range(HPC)]
        mask_sb = persist.tile([P, 4 * QT], BF16, name="mask")
        ones_bf = persist.tile([P, 1], BF16, name="ones")
        cos_sb = persist.tile([P, S], BF16, name="cosT2")
        sin_sb = persist.tile([P, S], BF16, name="sinT2")
        wo_sb = [persist.tile([P, DM], BF16, name=f"wo{h}") for h in range(HPC)]
        xp = ctx.enter_context(tc.tile_pool(name="xT", bufs=1))
        wp = ctx.enter_context(tc.tile_pool(name="w", bufs=1))
        rp = ctx.enter_context(tc.tile_pool(name="rope", bufs=4))

        # DMA priority order (HWDGE + the transfer engines serialize, so
        # issue order IS arrival order): rope tables, then the pre-phase
        # wavefront (w cols [q0|k0|v] + x first halves, k-interleaved),
        # then x second halves, mask, w cols [q1|k1], wo. w_all columns
        # are host-reordered to [q0, k0, v, q1, k1] to enable the split.
        # Batched loads: HWDGE issue bandwidth (~0.63us per DMA) is the
        # startup bottleneck, so w/x load as 4-k-group DMAs via 3-D tiles
        # and partition-inner DRAM views; x additionally splits into
        # column halves so the first s-tiles unblock early.
        wt_all = wp.tile([P, NKC, 3 * CW], BF16, name="wt")
        xt_all = xp.tile([P, NKC, S], BF16, name="xt")
        w_r = w_d.rearrange("(k p) c -> p k c", p=P)
        x_r = xT_d.rearrange("(k p) c -> p k c", p=P)
        HS = S // 2
        for g in range(0, NKC, 4):
            nc.sync.dma_start(
                out=wt_all[:, g:g + 4, :], in_=w_r[:, g:g + 4, :])
            nc.scalar.dma_start(
                out=xt_all[:, g:g + 4, 0:HS], in_=x_r[:, g:g + 4, 0:HS])
            if g == 0:
                # rope tables: needed ~10us in, after the first k-group
                nc.scalar.dma_start(out=cos_sb[:], in_=cos_d[:])
                nc.scalar.dma_start(out=sin_sb[:], in_=sin_d[:])
        for g in range(0, NKC, 4):
            (nc.sync if g % 8 == 0 else nc.scalar).dma_start(
                out=xt_all[:, g:g + 4, HS:S], in_=x_r[:, g:g + 4, HS:S])
        nc.sync.dma_start(out=mask_sb[:], in_=mask_d[:])
        nc.vector.memset(ones_bf[:], 1.0)
        for h in range(HPC):
            nc.sync.dma_start(out=wo_sb[h][:], in_=wo_d[h * P:(h + 1) * P, :])

        # w_all column offsets after host reorder [q0, k0, v, q1, k1]
        W_OFF = {0: 0, 2: P, 1: 2 * P + CW, 3: 3 * P + CW}
        V_OFF = 2 * P

        def qk_chunks(pool, c, st):
            """q/k feature chunk c (0: q-h0, 1: q-h1, 2: k-h0, 3: k-h1),
            s-tile st, transposed layout + fused rope, as 4 PE micro-steps."""
            dst = (qT, kT)[c // HPC][c % HPC]
            wo_ = W_OFF[c]
            state = {}

            def mm(k0):
                def f():
                    if k0 == 0:
                        state["ps"] = pool.tile([P, QT], F32, name="f")
                    ps = state["ps"]
                    for k in range(k0, k0 + 4):
                        nc.tensor.matmul(
                            ps[:],
                            wt_all[:, k, wo_:wo_ + P],
                            xt_all[:, k, st * QT:(st + 1) * QT],
                            start=(k == 0), stop=(k == NKC - 1),
                        )
                    if k0 == NKC - 4:
                        ps = state["ps"]
                        cs = slice(st * QT, (st + 1) * QT)
                        # Pool cannot read PSUM: both rope multiplies run
                        # on DVE; the all-SBUF add goes to Pool.
                        t1 = rp.tile([P, QT], BF16, name="t1")
                        nc.vector.tensor_mul(t1[:], ps[:], cos_sb[:, cs])
                        t2 = rp.tile([P, QT], BF16, name="t2")
                        nc.vector.tensor_mul(
                            t2[0:HW, :], ps[HW:HD, :], sin_sb[0:HW, cs])
                        nc.vector.tensor_mul(
                            t2[HW:HD, :], ps[0:HW, :], sin_sb[HW:HD, cs])
                        nc.gpsimd.tensor_add(dst[:, cs], t1[:], t2[:])
                return f
            return [mm(k0) for k0 in range(0, NKC, 4)]

        def v_chunks(pool, sc):
            """v s-chunk sc in natural layout, as 2 PE micro-steps."""
            state = {}

            def mm(k0):
                def f():
                    if k0 == 0:
                        state["ps"] = pool.tile([P, QT], F32, name="f")
                    ps = state["ps"]
                    for k in range(k0, k0 + 8):
                        nc.tensor.matmul(
                            ps[:, 0:CW],
                            xt_all[:, k, sc * P:(sc + 1) * P],
                            wt_all[:, k, V_OFF:V_OFF + CW],
                            start=(k == 0), stop=(k == NKC - 1),
                        )
                    if k0 == NKC - 8:
                        for h in range(HPC):
                            nc.vector.tensor_copy(
                                v_sb[h][:, sc * P:(sc + 1) * P],
                                ps[:, h * HD:(h + 1) * HD],
                            )
                return f
            return [mm(0), mm(8)]

        o_r = out_d.rearrange("(o p) s -> p o s", p=P)

        def c_chunks(st, outp, c_ps, alt=False):
            """output-projection pieces for s-tile st, 1 PE micro-step each;
            results stage into 4-oc-wide tiles DMA'd as one transfer."""
            state = {}

            def piece(oc):
                def f():
                    acc = c_ps.tile([P, QT], F32, name="f")
                    for h in range(HPC):
                        nc.tensor.matmul(
                            acc[:],
                            wo_sb[h][:, oc * P:(oc + 1) * P],
                            oT[h][:, st * QT:(st + 1) * QT],
                            start=(h == 0), stop=(h == HPC - 1),
                        )
                    if oc % 4 == 0:
                        state["osb"] = outp.tile([P, 4, QT], BF16, name="osb")
                    osb = state["osb"]
                    # ACT carries the B1 tanh/exp chain: only 1 in 4 copies
                    # goes there — except in the drain round (alt), where
                    # ACT is free and copies alternate 50/50
                    if (oc % 2 == 0) if alt else (oc % 4 == 0):
                        nc.scalar.copy(osb[:, oc % 4, :], acc[:])
                    else:
                        nc.vector.tensor_copy(osb[:, oc % 4, :], acc[:])
                    if oc % 4 == 3:
                        nc.sync.dma_start(
                            out=o_r[:, oc - 3:oc + 1,
                                    st * QT:(st + 1) * QT],
                            in_=osb[:])
                return f
            return [piece(oc) for oc in range(NKC)]

        class Feeder:
            """Doles out independent PE micro-steps to hide ACT latency."""
            def __init__(self):
                self.chunks = []

            def add(self, chunks):
                self.chunks.extend(chunks)

            def step(self, n):
                for _ in range(n):
                    if self.chunks:
                        self.chunks.pop(0)()

            def drain(self):
                self.step(len(self.chunks))

        def emit_attn(h, t, pools, feeder, per_pair):
            s_ps, o_ps, l_ps, thp, pp, np_ = pools
            o_acc = o_ps.tile([P, QT], F32, name="o_acc")
            l_acc = l_ps.tile([1, QT], F32, name="l_acc")
            npair = 2 * t + 2
            q_ap = qT[h][:, t * QT:(t + 1) * QT]

            def emit_pv(pT, p, last):
                for i in range(2):
                    kc = 2 * p + i
                    nc.tensor.matmul(
                        o_acc[:],
                        v_sb[h][:, kc * P:(kc + 1) * P],
                        pT[:, i * QT:(i + 1) * QT],
                        start=(kc == 0), stop=(last and i == 1),
                    )
                    nc.tensor.matmul(
                        l_acc[:], ones_bf[:, 0:1],
                        pT[:, i * QT:(i + 1) * QT],
                        start=(kc == 0), stop=(last and i == 1),
                    )

            prev = None
            for p in range(npair):
                sp = s_ps.tile([P, 2 * QT], F32, name="sp")
                for i in range(2):
                    kc = 2 * p + i
                    nc.tensor.matmul(
                        sp[:, i * QT:(i + 1) * QT],
                        kT[h][:, kc * P:(kc + 1) * P], q_ap,
                        start=True, stop=True,
                    )
                feeder.step(per_pair)
                th = thp.tile([P, 2 * QT], F32, name="th")
                nc.scalar.activation(th[:], sp[:], Tanh, scale=C1)
                pT = pp.tile([P, 2 * QT], BF16, name="pTt")
                nc.scalar.activation(pT[:], th[:], Exp, scale=SOFTCAP)
                # masked pairs are the last two: p==2t (u=0,1), p==2t+1 (u=2,3)
                u0 = 2 * (p - 2 * t)
                if u0 >= 0:
                    nc.vector.tensor_mul(
                        pT[:], pT[:], mask_sb[:, u0 * QT:(u0 + 2) * QT])
                if prev is not None:
                    emit_pv(prev[0], prev[1], last=False)
                prev = (pT, p)
            emit_pv(prev[0], prev[1], last=True)
            recip = np_.tile([1, QT], F32, name="recip")
            nc.vector.reciprocal(recip[:], l_acc[:])
            bcast = np_.tile([P, QT], F32, name="bcast")
            nc.gpsimd.partition_broadcast(bcast[:], recip[:])
            nc.vector.tensor_mul(
                oT[h][:, t * QT:(t + 1) * QT], o_acc[:], bcast[:])

        # ---------- phase A (pre-attention part) ----------
        # head 0's q/k + the first 4 v chunks. Tiles needing only the x
        # first halves come first, k-interleaved within 3-tile windows so
        # the PE tracks the DMA wavefront instead of stalling on one tile.
        def interleave(units):
            out = []
            for step in range(max(len(u) for u in units)):
                for u in units:
                    if step < len(u):
                        out.append(u[step])
            return out

        # The A phase is DMA-bound (~35us of input wavefront), so all v
        # chunks ride along in its PE bubbles, ordered by which x quarter
        # they need.
        with ExitStack() as ctxA:
            qkA = ctxA.enter_context(
                tc.tile_pool(name="qkA", bufs=4, space="PSUM"))
            pre = Feeder()
            pre.add(interleave([qk_chunks(qkA, 0, 0), qk_chunks(qkA, 2, 0)]))
            pre.add(interleave([qk_chunks(qkA, 0, 1), qk_chunks(qkA, 2, 1)]))
            for sc in range(0, 8):
                pre.add(v_chunks(qkA, sc))
            pre.add(interleave([qk_chunks(qkA, 0, 2), qk_chunks(qkA, 2, 2)]))
            for sc in range(8, 12):
                pre.add(v_chunks(qkA, sc))
            pre.add(interleave([qk_chunks(qkA, 0, 3), qk_chunks(qkA, 2, 3)]))
            for sc in range(12, 16):
                pre.add(v_chunks(qkA, sc))
            pre.drain()

        # ---------- phase B0: head-0 attention + A-fill ----------
        # shared fill/output-projection PSUM pool (one tag, 2 banks);
        # created below the B pools so those can close before the drain
        fps = ctx.enter_context(tc.tile_pool(name="fps", bufs=2, space="PSUM"))
        outp = ctx.enter_context(tc.tile_pool(name="out", bufs=4))
        fill = Feeder()
        with ExitStack() as ctxB:
            s_ps = ctxB.enter_context(
                tc.tile_pool(name="s_ps", bufs=2, space="PSUM"))
            o_ps = ctxB.enter_context(
                tc.tile_pool(name="o_ps", bufs=1, space="PSUM"))
            l_ps = ctxB.enter_context(
                tc.tile_pool(name="l_ps", bufs=1, space="PSUM"))
            thp = ctxB.enter_context(tc.tile_pool(name="tanh", bufs=3))
            pp = ctxB.enter_context(tc.tile_pool(name="pT", bufs=3))
            np_ = ctxB.enter_context(tc.tile_pool(name="norm", bufs=2))
            bpools = (s_ps, o_ps, l_ps, thp, pp, np_)

            for st in (0, 1):
                fill.add(qk_chunks(fps, 1, st))
                fill.add(qk_chunks(fps, 3, st))
            for t in range(NQT):
                emit_attn(0, t, bpools, fill, per_pair=1)
            # q1/k1 st2/st3 are first needed by B1 t2/t3: defer them into
            # the otherwise-unfilled B1 t0/t1 windows.
            for st in (2, 3):
                fill.add(qk_chunks(fps, 1, st))
                fill.add(qk_chunks(fps, 3, st))

            # ---------- phase B1 + C: attention + output projection ----
            for t in range(NQT):
                emit_attn(1, t, bpools, fill, per_pair=4)
                if t < NQT - 1:
                    fill.add(c_chunks(t, outp, fps))
        # drain round: B pools are closed, give the last output-projection
        # round a wide PSUM pool so its pieces pipeline
        cD = ctx.enter_context(tc.tile_pool(name="cD", bufs=5, space="PSUM"))
        fill.add(c_chunks(NQT - 1, outp, cD, alt=True))
        fill.drain()


_NC_CACHE = None


def _get_nc():
    global _NC_CACHE
    if _NC_CACHE is None:
        _NC_CACHE = build_nc()
    return _NC_CACHE


def _rope_perm():
    """per-head column permutation de-interleaving rotary pairs"""
    perm = np.zeros(DM, np.int64)
    for h in range(H):
        base = h * HD
        perm[base:base + HD // 2] = base + np.arange(0, HD, 2)
        perm[base + HD // 2:base + HD] = base + np.arange(1, HD, 2)
    return perm


def make_in_maps(x, wq, wk, wv, wo, freqs_cos, freqs_sin):
    x = np.asarray(x, np.float32).reshape(S, DM)
    wq = np.asarray(wq, np.float32)
    wk = np.asarray(wk, np.float32)
    wv = np.asarray(wv, np.float32)
    wo = np.asarray(wo, np.float32)
    xT = np.ascontiguousarray(x.T).astype(NPBF16)
    perm = _rope_perm()
    wq_p = wq[:, perm]
    wk_p = wk[:, perm]
    # transposed rope tables: C = [cosT; cosT], S' = [-sinT; sinT]
    cosT = np.asarray(freqs_cos, np.float32).T  # [64, S]
    sinT = np.asarray(freqs_sin, np.float32).T
    cosT2 = np.concatenate([cosT, cosT], axis=0).astype(NPBF16)
    sinT2 = np.concatenate([-sinT, sinT], axis=0).astype(NPBF16)
    # mask[i, u*QT + j] = 1 if i <= j - 128*u else 0  (keep kj <= qi)
    i_idx = np.arange(P)[:, None]
    j_idx = np.arange(QT)[None, :]
    mask = np.concatenate(
        [(i_idx <= j_idx - P * u) for u in range(4)], axis=1
    ).astype(NPBF16)
    in_maps = []
    for c in range(N_CORES):
        cs = slice(c * CW, (c + 1) * CW)
        h0 = slice(c * CW, c * CW + HD)
        h1 = slice(c * CW + HD, (c + 1) * CW)
        # device column order: [q-h0, k-h0, v, q-h1, k-h1]
        w_all = np.concatenate(
            [wq_p[:, h0], wk_p[:, h0], wv[:, cs],
             wq_p[:, h1], wk_p[:, h1]], axis=1).astype(NPBF16)
        wo_c = np.ascontiguousarray(wo[cs, :]).astype(NPBF16)
        in_maps.append({
            "xT": xT,
            "w_all": np.ascontiguousarray(w_all),
            "wo_c": wo_c,
            "cosT2": cosT2,
            "sinT2": sinT2,
            "mask": mask,
        })
    return in_maps


def assemble_output(results):
    acc = results[0]["outT"].astype(np.float32)
    for r in results[1:]:
        acc += np.asarray(r["outT"]).astype(np.float32)
    return np.ascontiguousarray(acc.T).reshape(1, S, DM).astype(np.float32)


def kernel(x, wq, wk, wv, wo, freqs_cos, freqs_sin):
    nc = _get_nc()
    in_maps = make_in_maps(x, wq, wk, wv, wo, freqs_cos, freqs_sin)
    res = run_bass_kernel_spmd(nc, in_maps, core_ids=list(range(N_CORES)))
    return assemble_output(res.results)


if __name__ == "__main__":
    rng = np.random.default_rng(0)
    ins = {
        "x": rng.standard_normal((1, S, DM), np.float32),
        "wq": rng.standard_normal((DM, DM), np.float32) / np.sqrt(DM),
        "wk": rng.standard_normal((DM, DM), np.float32) / np.sqrt(DM),
        "wv": rng.standard_normal((DM, DM), np.float32) / np.sqrt(DM),
        "wo": rng.standard_normal((DM, DM), np.float32) / np.sqrt(DM),
        "freqs_cos": rng.standard_normal((S, HD // 2), np.float32),
        "freqs_sin": rng.standard_normal((S, HD // 2), np.float32),
    }
    out = kernel(**ins)
    print("out", out.shape, out.dtype, np.abs(out).mean())


# revision 41
# speedup vs baseline: 1.0270x; 1.0270x over previous
"""Trainium2 Bass kernel for Llama-like attention (16 heads, tanh softcap, RoPE).

Sharding: tensor-parallel over heads, fully collective-free. Each of the 8
cores computes 2 heads end-to-end and a *partial* output projection
(o_local @ wo_rows_local)^T; the host sums the 8 partial outputs. With no
on-device collective, each core's NEFF span is pure local compute — no
cross-core rendezvous.

Per-core pipeline (engine-balanced against the ~165us PE floor):
  - q^T/k^T computed directly in transposed layout ([head_dim, s]) via
    matmul(w_slice^T, x^T): no PE transposes. Weight columns of wq/wk are
    pre-permuted on the host to de-interleave even/odd rotary pairs (the
    permutation cancels inside q.k).
  - RoPE in transposed layout straight out of PSUM: rot = A*C + swap(A)*S'
    with C = [cosT; cosT], S' = [-sinT; sinT]. The partition-half swap is
    two half-height Vector multiplies reading PSUM at a partition offset
    (Pool cannot touch PSUM); the all-SBUF add runs on Pool. ACT stays
    free for the softmax chain.
  - attention with scores transposed ([kj, qi]) so softmaxed probabilities
    feed the PV matmul directly as the moving operand. tanh softcap bounds
    scores, so softmax needs no row-max pass: p = exp(50*tanh(.)),
    l = ones-row matmul, o = p@v / l. Head 0's ACT-bound window is filled
    with head 1's q/k projection and the tail v chunks; head 1's windows
    are filled with the output-projection pieces for the q-tile that just
    finished.
  - output projection pieces acc[oc(128), st(512)] += wo_h[:, oc]^T @ oT_h
    accumulated over the 2 local heads, copied to SBUF bf16 (ACT/DVE
    alternating) and DMA'd per piece. Host sums partials and transposes.
"""

import os
import sys

for _p in ("/root/.axon_site/_ro/trn_rl_repo", "/opt/trn_rl_repo"):
    if os.path.isdir(_p) and _p not in sys.path:
        sys.path.append(_p)

import numpy as np
import ml_dtypes
from contextlib import ExitStack

import concourse.bass as bass
import concourse.bacc as bacc
import concourse.mybir as mybir
import concourse.tile as tile
from concourse.bass_utils import run_bass_kernel_spmd

BF16 = mybir.dt.bfloat16
F32 = mybir.dt.float32
NPBF16 = ml_dtypes.bfloat16

N_CORES = 8
S = 2048          # sequence length
DM = 2048         # model dim
H = 16            # heads
HD = 128          # head dim
HPC = H // N_CORES  # heads per core = 2
CW = HPC * HD     # per-core projection width = 256
P = 128
HW = HD // 2      # 64
QT = 512          # query tile (free dim of attention matmuls)
NQT = S // QT     # 4 query tiles per head
NSC = S // P      # 16 sequence chunks
NKC = DM // P     # 16 contraction chunks
NST = S // QT     # 4 s-tiles
SOFTCAP = 50.0
C1 = 1.0 / (SOFTCAP * np.sqrt(HD))

Tanh = mybir.ActivationFunctionType.Tanh
Exp = mybir.ActivationFunctionType.Exp


def build_nc(reps=1, single=False):
    nc = bacc.Bacc("TRN2", target_bir_lowering=False, num_devices=N_CORES)

    xT_d = nc.dram_tensor("xT", [DM, S], BF16, kind="ExternalInput")
    w_d = nc.dram_tensor("w_all", [DM, 3 * CW], BF16, kind="ExternalInput")
    wo_d = nc.dram_tensor("wo_c", [CW, DM], BF16, kind="ExternalInput")
    cos_d = nc.dram_tensor("cosT2", [P, S], BF16, kind="ExternalInput")
    sin_d = nc.dram_tensor("sinT2", [P, S], BF16, kind="ExternalInput")
    mask_d = nc.dram_tensor("mask", [P, 4 * QT], BF16, kind="ExternalInput")
    out_d = nc.dram_tensor("outT", [DM, S], BF16, kind="ExternalOutput")

    with tile.TileContext(nc) as tc:
        for _rep in range(reps):
            _emit_body(nc, tc, xT_d, w_d, wo_d, cos_d, sin_d, mask_d, out_d)
    nc.compile()
    return nc


def _emit_body(nc, tc, xT_d, w_d, wo_d, cos_d, sin_d, mask_d, out_d):
    with ExitStack() as ctx:
        # ---------- persistent SBUF ----------
        persist = ctx.enter_context(tc.tile_pool(name="persist", bufs=1))
        qT = [persist.tile([P, S], BF16, name=f"qT{h}") for h in range(HPC)]
        kT = [persist.tile([P, S], BF16, name=f"kT{h}") for h in range(HPC)]
        v_sb = [persist.tile([P, S], BF16, name=f"v{h}") for h in range(HPC)]
        oT = [persist.tile([P, S], BF16, name=f"oT{h}") for h in range(HPC)]
        mask_sb = persist.tile([P, 4 * QT], BF16, name="mask")
        ones_bf = persist.tile([P, 1], BF16, name="ones")
        cos_sb = persist.tile([P, S], BF16, name="cosT2")
        sin_sb = persist.tile([P, S], BF16, name="sinT2")
        wo_sb = [persist.tile([P, DM], BF16, name=f"wo{h}") for h in range(HPC)]
        xp = ctx.enter_context(tc.tile_pool(name="xT", bufs=1))
        wp = ctx.enter_context(tc.tile_pool(name="w", bufs=1))
        rp = ctx.enter_context(tc.tile_pool(name="rope", bufs=4))

        # DMA priority order (HWDGE + the transfer engines serialize, so
        # issue order IS arrival order): rope tables, then the pre-phase
        # wavefront (w cols [q0|k0|v] + x first halves, k-interleaved),
        # then x second halves, mask, w cols [q1|k1], wo. w_all columns
        # are host-reordered to [q0, k0, v, q1, k1] to enable the split.
        # Batched loads: HWDGE issue bandwidth (~0.63us per DMA) is the
        # startup bottleneck, so w/x load as 4-k-group DMAs via 3-D tiles
        # and partition-inner DRAM views; x additionally splits into
        # column halves so the first s-tiles unblock early.
        wt_all = wp.tile([P, NKC, 3 * CW], BF16, name="wt")
        xt_all = xp.tile([P, NKC, S], BF16, name="xt")
        w_r = w_d.rearrange("(k p) c -> p k c", p=P)
        x_r = xT_d.rearrange("(k p) c -> p k c", p=P)
        HS = S // 2
        WA = 2 * P + CW  # wavefront w cols: q0 | k0 | v
        # first k-group split in two so the very first matmuls start ~3us
        nc.sync.dma_start(out=wt_all[:, 0:2, 0:WA], in_=w_r[:, 0:2, 0:WA])
        nc.scalar.dma_start(out=xt_all[:, 0:2, 0:HS], in_=x_r[:, 0:2, 0:HS])
        nc.sync.dma_start(out=wt_all[:, 2:4, 0:WA], in_=w_r[:, 2:4, 0:WA])
        nc.scalar.dma_start(out=xt_all[:, 2:4, 0:HS], in_=x_r[:, 2:4, 0:HS])
        # rope tables: needed ~10us in, after the first k-group
        nc.scalar.dma_start(out=cos_sb[:], in_=cos_d[:])
        nc.scalar.dma_start(out=sin_sb[:], in_=sin_d[:])
        for g in range(4, NKC, 4):
            nc.sync.dma_start(
                out=wt_all[:, g:g + 4, 0:WA], in_=w_r[:, g:g + 4, 0:WA])
            nc.scalar.dma_start(
                out=xt_all[:, g:g + 4, 0:HS], in_=x_r[:, g:g + 4, 0:HS])
        for g in range(0, NKC, 4):
            (nc.sync if g % 8 == 0 else nc.scalar).dma_start(
                out=xt_all[:, g:g + 4, HS:S], in_=x_r[:, g:g + 4, HS:S])
        for g in range(0, NKC, 8):  # q1|k1 w cols, first needed mid-B0
            nc.sync.dma_start(
                out=wt_all[:, g:g + 8, WA:3 * CW],
                in_=w_r[:, g:g + 8, WA:3 * CW])
        nc.sync.dma_start(out=mask_sb[:], in_=mask_d[:])
        nc.vector.memset(ones_bf[:], 1.0)
        for h in range(HPC):
            nc.sync.dma_start(out=wo_sb[h][:], in_=wo_d[h * P:(h + 1) * P, :])

        # w_all column offsets after host reorder [q0, k0, v, q1, k1]
        W_OFF = {0: 0, 2: P, 1: 2 * P + CW, 3: 3 * P + CW}
        V_OFF = 2 * P

        def qk_chunks(pool, c, st):
            """q/k feature chunk c (0: q-h0, 1: q-h1, 2: k-h0, 3: k-h1),
            s-tile st, transposed layout + fused rope, as 4 PE micro-steps."""
            dst = (qT, kT)[c // HPC][c % HPC]
            wo_ = W_OFF[c]
            state = {}

            def mm(k0):
                def f():
                    if k0 == 0:
                        state["ps"] = pool.tile([P, QT], F32, name="f")
                    ps = state["ps"]
                    for k in range(k0, k0 + 4):
                        nc.tensor.matmul(
                            ps[:],
                            wt_all[:, k, wo_:wo_ + P],
                            xt_all[:, k, st * QT:(st + 1) * QT],
                            start=(k == 0), stop=(k == NKC - 1),
                        )
                    if k0 == NKC - 4:
                        ps = state["ps"]
                        cs = slice(st * QT, (st + 1) * QT)
                        # Pool cannot read PSUM: both rope multiplies run
                        # on DVE; the all-SBUF add goes to Pool.
                        t1 = rp.tile([P, QT], BF16, name="t1")
                        nc.vector.tensor_mul(t1[:], ps[:], cos_sb[:, cs])
                        t2 = rp.tile([P, QT], BF16, name="t2")
                        nc.vector.tensor_mul(
                            t2[0:HW, :], ps[HW:HD, :], sin_sb[0:HW, cs])
                        nc.vector.tensor_mul(
                            t2[HW:HD, :], ps[0:HW, :], sin_sb[HW:HD, cs])
                        nc.gpsimd.tensor_add(dst[:, cs], t1[:], t2[:])
                return f
            return [mm(k0) for k0 in range(0, NKC, 4)]

        def v_chunks(pool, sc):
            """v s-chunk sc in natural layout, as 2 PE micro-steps."""
            state = {}

            def mm(k0):
                def f():
                    if k0 == 0:
                        state["ps"] = pool.tile([P, QT], F32, name="f")
                    ps = state["ps"]
                    for k in range(k0, k0 + 8):
                        nc.tensor.matmul(
                            ps[:, 0:CW],
                            xt_all[:, k, sc * P:(sc + 1) * P],
                            wt_all[:, k, V_OFF:V_OFF + CW],
                            start=(k == 0), stop=(k == NKC - 1),
                        )
                    if k0 == NKC - 8:
                        for h in range(HPC):
                            nc.vector.tensor_copy(
                                v_sb[h][:, sc * P:(sc + 1) * P],
                                ps[:, h * HD:(h + 1) * HD],
                            )
                return f
            return [mm(0), mm(8)]

        o_r = out_d.rearrange("(o p) s -> p o s", p=P)

        def c_chunks(st, outp, c_ps, alt=False):
            """output-projection pieces for s-tile st, 1 PE micro-step each;
            results stage into 4-oc-wide tiles DMA'd as one transfer."""
            state = {}

            def piece(oc):
                def f():
                    acc = c_ps.tile([P, QT], F32, name="f")
                    for h in range(HPC):
                        nc.tensor.matmul(
                            acc[:],
                            wo_sb[h][:, oc * P:(oc + 1) * P],
                            oT[h][:, st * QT:(st + 1) * QT],
                            start=(h == 0), stop=(h == HPC - 1),
                        )
                    if oc % 4 == 0:
                        state["osb"] = outp.tile([P, 4, QT], BF16, name="osb")
                    osb = state["osb"]
                    # ACT carries the B1 tanh/exp chain: only 1 in 4 copies
                    # goes there — except in the drain round (alt), where
                    # ACT is free and copies alternate 50/50
                    if (oc % 2 == 0) if alt else (oc % 4 == 0):
                        nc.scalar.copy(osb[:, oc % 4, :], acc[:])
                    else:
                        nc.vector.tensor_copy(osb[:, oc % 4, :], acc[:])
                    if oc % 4 == 3:
                        nc.sync.dma_start(
                            out=o_r[:, oc - 3:oc + 1,
                                    st * QT:(st + 1) * QT],
                            in_=osb[:])
                return f
            return [piece(oc) for oc in range(NKC)]

        class Feeder:
            """Doles out independent PE micro-steps to hide ACT latency."""
            def __init__(self):
                self.chunks = []

            def add(self, chunks):
                self.chunks.extend(chunks)

            def step(self, n):
                for _ in range(n):
                    if self.chunks:
                        self.chunks.pop(0)()

            def drain(self):
                self.step(len(self.chunks))

        def emit_attn(h, t, pools, feeder, per_pair):
            s_ps, o_ps, l_ps, thp, pp, np_ = pools
            o_acc = o_ps.tile([P, QT], F32, name="o_acc")
            l_acc = l_ps.tile([1, QT], F32, name="l_acc")
            npair = 2 * t + 2
            q_ap = qT[h][:, t * QT:(t + 1) * QT]

            def emit_pv(pT, p, last):
                for i in range(2):
                    kc = 2 * p + i
                    nc.tensor.matmul(
                        o_acc[:],
                        v_sb[h][:, kc * P:(kc + 1) * P],
                        pT[:, i * QT:(i + 1) * QT],
                        start=(kc == 0), stop=(last and i == 1),
                    )
                    nc.tensor.matmul(
                        l_acc[:], ones_bf[:, 0:1],
                        pT[:, i * QT:(i + 1) * QT],
                        start=(kc == 0), stop=(last and i == 1),
                    )

            prev = None
            for p in range(npair):
                sp = s_ps.tile([P, 2 * QT], F32, name="sp")
                for i in range(2):
                    kc = 2 * p + i
                    nc.tensor.matmul(
                        sp[:, i * QT:(i + 1) * QT],
                        kT[h][:, kc * P:(kc + 1) * P], q_ap,
                        start=True, stop=True,
                    )
                feeder.step(per_pair)
                th = thp.tile([P, 2 * QT], F32, name="th")
                nc.scalar.activation(th[:], sp[:], Tanh, scale=C1)
                pT = pp.tile([P, 2 * QT], BF16, name="pTt")
                nc.scalar.activation(pT[:], th[:], Exp, scale=SOFTCAP)
                # masked pairs are the last two: p==2t (u=0,1), p==2t+1 (u=2,3)
                u0 = 2 * (p - 2 * t)
                if u0 >= 0:
                    nc.vector.tensor_mul(
                        pT[:], pT[:], mask_sb[:, u0 * QT:(u0 + 2) * QT])
                if prev is not None:
                    emit_pv(prev[0], prev[1], last=False)
                prev = (pT, p)
            emit_pv(prev[0], prev[1], last=True)
            recip = np_.tile([1, QT], F32, name="recip")
            nc.vector.reciprocal(recip[:], l_acc[:])
            bcast = np_.tile([P, QT], F32, name="bcast")
            nc.gpsimd.partition_broadcast(bcast[:], recip[:])
            nc.vector.tensor_mul(
                oT[h][:, t * QT:(t + 1) * QT], o_acc[:], bcast[:])

        # ---------- phase A (pre-attention part) ----------
        # head 0's q/k + the first 4 v chunks. Tiles needing only the x
        # first halves come first, k-interleaved within 3-tile windows so
        # the PE tracks the DMA wavefront instead of stalling on one tile.
        def interleave(units):
            out = []
            for step in range(max(len(u) for u in units)):
                for u in units:
                    if step < len(u):
                        out.append(u[step])
            return out

        # The A phase is DMA-bound (~35us of input wavefront), so all v
        # chunks ride along in its PE bubbles, ordered by which x quarter
        # they need.
        with ExitStack() as ctxA:
            qkA = ctxA.enter_context(
                tc.tile_pool(name="qkA", bufs=4, space="PSUM"))
            pre = Feeder()
            pre.add(interleave([qk_chunks(qkA, 0, 0), qk_chunks(qkA, 2, 0)]))
            pre.add(interleave([qk_chunks(qkA, 0, 1), qk_chunks(qkA, 2, 1)]))
            for sc in range(0, 8):
                pre.add(v_chunks(qkA, sc))
            pre.add(interleave([qk_chunks(qkA, 0, 2), qk_chunks(qkA, 2, 2)]))
            for sc in range(8, 12):
                pre.add(v_chunks(qkA, sc))
            pre.add(interleave([qk_chunks(qkA, 0, 3), qk_chunks(qkA, 2, 3)]))
            for sc in range(12, 16):
                pre.add(v_chunks(qkA, sc))
            pre.drain()

        # ---------- phase B0: head-0 attention + A-fill ----------
        # shared fill/output-projection PSUM pool (one tag, 2 banks);
        # created below the B pools so those can close before the drain
        fps = ctx.enter_context(tc.tile_pool(name="fps", bufs=2, space="PSUM"))
        outp = ctx.enter_context(tc.tile_pool(name="out", bufs=4))
        fill = Feeder()
        with ExitStack() as ctxB:
            s_ps = ctxB.enter_context(
                tc.tile_pool(name="s_ps", bufs=2, space="PSUM"))
            o_ps = ctxB.enter_context(
                tc.tile_pool(name="o_ps", bufs=1, space="PSUM"))
            l_ps = ctxB.enter_context(
                tc.tile_pool(name="l_ps", bufs=1, space="PSUM"))
            thp = ctxB.enter_context(tc.tile_pool(name="tanh", bufs=3))
            pp = ctxB.enter_context(tc.tile_pool(name="pT", bufs=3))
            np_ = ctxB.enter_context(tc.tile_pool(name="norm", bufs=2))
            bpools = (s_ps, o_ps, l_ps, thp, pp, np_)

            for st in (0, 1):
                fill.add(qk_chunks(fps, 1, st))
                fill.add(qk_chunks(fps, 3, st))
            for t in range(NQT):
                emit_attn(0, t, bpools, fill, per_pair=1)
            # q1/k1 st2/st3 are first needed by B1 t2/t3: defer them into
            # the otherwise-unfilled B1 t0/t1 windows.
            for st in (2, 3):
                fill.add(qk_chunks(fps, 1, st))
                fill.add(qk_chunks(fps, 3, st))

            # ---------- phase B1 + C: attention + output projection ----
            for t in range(NQT):
                emit_attn(1, t, bpools, fill, per_pair=4)
                if t < NQT - 1:
                    fill.add(c_chunks(t, outp, fps))
        # drain round: B pools are closed, give the last output-projection
        # round a wide PSUM pool so its pieces pipeline
        cD = ctx.enter_context(tc.tile_pool(name="cD", bufs=5, space="PSUM"))
        fill.add(c_chunks(NQT - 1, outp, cD, alt=True))
        fill.drain()


_NC_CACHE = None


def _get_nc():
    global _NC_CACHE
    if _NC_CACHE is None:
        _NC_CACHE = build_nc()
    return _NC_CACHE


def _rope_perm():
    """per-head column permutation de-interleaving rotary pairs"""
    perm = np.zeros(DM, np.int64)
    for h in range(H):
        base = h * HD
        perm[base:base + HD // 2] = base + np.arange(0, HD, 2)
        perm[base + HD // 2:base + HD] = base + np.arange(1, HD, 2)
    return perm


def make_in_maps(x, wq, wk, wv, wo, freqs_cos, freqs_sin):
    x = np.asarray(x, np.float32).reshape(S, DM)
    wq = np.asarray(wq, np.float32)
    wk = np.asarray(wk, np.float32)
    wv = np.asarray(wv, np.float32)
    wo = np.asarray(wo, np.float32)
    xT = np.ascontiguousarray(x.T).astype(NPBF16)
    perm = _rope_perm()
    wq_p = wq[:, perm]
    wk_p = wk[:, perm]
    # transposed rope tables: C = [cosT; cosT], S' = [-sinT; sinT]
    cosT = np.asarray(freqs_cos, np.float32).T  # [64, S]
    sinT = np.asarray(freqs_sin, np.float32).T
    cosT2 = np.concatenate([cosT, cosT], axis=0).astype(NPBF16)
    sinT2 = np.concatenate([-sinT, sinT], axis=0).astype(NPBF16)
    # mask[i, u*QT + j] = 1 if i <= j - 128*u else 0  (keep kj <= qi)
    i_idx = np.arange(P)[:, None]
    j_idx = np.arange(QT)[None, :]
    mask = np.concatenate(
        [(i_idx <= j_idx - P * u) for u in range(4)], axis=1
    ).astype(NPBF16)
    in_maps = []
    for c in range(N_CORES):
        cs = slice(c * CW, (c + 1) * CW)
        h0 = slice(c * CW, c * CW + HD)
        h1 = slice(c * CW + HD, (c + 1) * CW)
        # device column order: [q-h0, k-h0, v, q-h1, k-h1]
        w_all = np.concatenate(
            [wq_p[:, h0], wk_p[:, h0], wv[:, cs],
             wq_p[:, h1], wk_p[:, h1]], axis=1).astype(NPBF16)
        wo_c = np.ascontiguousarray(wo[cs, :]).astype(NPBF16)
        in_maps.append({
            "xT": xT,
            "w_all": np.ascontiguousarray(w_all),
            "wo_c": wo_c,
            "cosT2": cosT2,
            "sinT2": sinT2,
            "mask": mask,
        })
    return in_maps


def assemble_output(results):
    acc = results[0]["outT"].astype(np.float32)
    for r in results[1:]:
        acc += np.asarray(r["outT"]).astype(np.float32)
    return np.ascontiguousarray(acc.T).reshape(1, S, DM).astype(np.float32)


def kernel(x, wq, wk, wv, wo, freqs_cos, freqs_sin):
    nc = _get_nc()
    in_maps = make_in_maps(x, wq, wk, wv, wo, freqs_cos, freqs_sin)
    res = run_bass_kernel_spmd(nc, in_maps, core_ids=list(range(N_CORES)))
    return assemble_output(res.results)


if __name__ == "__main__":
    rng = np.random.default_rng(0)
    ins = {
        "x": rng.standard_normal((1, S, DM), np.float32),
        "wq": rng.standard_normal((DM, DM), np.float32) / np.sqrt(DM),
        "wk": rng.standard_normal((DM, DM), np.float32) / np.sqrt(DM),
        "wv": rng.standard_normal((DM, DM), np.float32) / np.sqrt(DM),
        "wo": rng.standard_normal((DM, DM), np.float32) / np.sqrt(DM),
        "freqs_cos": rng.standard_normal((S, HD // 2), np.float32),
        "freqs_sin": rng.standard_normal((S, HD // 2), np.float32),
    }
    out = kernel(**ins)
    print("out", out.shape, out.dtype, np.abs(out).mean())


# revision 45
# speedup vs baseline: 1.0307x; 1.0036x over previous
"""Trainium2 Bass kernel for Llama-like attention (16 heads, tanh softcap, RoPE).

Sharding: tensor-parallel over heads, fully collective-free. Each of the 8
cores computes 2 heads end-to-end and a *partial* output projection
(o_local @ wo_rows_local)^T; the host sums the 8 partial outputs. With no
on-device collective, each core's NEFF span is pure local compute — no
cross-core rendezvous.

Per-core pipeline (engine-balanced against the ~165us PE floor):
  - q^T/k^T computed directly in transposed layout ([head_dim, s]) via
    matmul(w_slice^T, x^T): no PE transposes. Weight columns of wq/wk are
    pre-permuted on the host to de-interleave even/odd rotary pairs (the
    permutation cancels inside q.k).
  - RoPE in transposed layout straight out of PSUM: rot = A*C + swap(A)*S'
    with C = [cosT; cosT], S' = [-sinT; sinT]. The partition-half swap is
    two half-height Vector multiplies reading PSUM at a partition offset
    (Pool cannot touch PSUM); the all-SBUF add runs on Pool. ACT stays
    free for the softmax chain.
  - attention with scores transposed ([kj, qi]) so softmaxed probabilities
    feed the PV matmul directly as the moving operand. tanh softcap bounds
    scores, so softmax needs no row-max pass: p = exp(50*tanh(.)),
    l = ones-row matmul, o = p@v / l. Head 0's ACT-bound window is filled
    with head 1's q/k projection and the tail v chunks; head 1's windows
    are filled with the output-projection pieces for the q-tile that just
    finished.
  - output projection pieces acc[oc(128), st(512)] += wo_h[:, oc]^T @ oT_h
    accumulated over the 2 local heads, copied to SBUF bf16 (ACT/DVE
    alternating) and DMA'd per piece. Host sums partials and transposes.
"""

import os
import sys

for _p in ("/root/.axon_site/_ro/trn_rl_repo", "/opt/trn_rl_repo"):
    if os.path.isdir(_p) and _p not in sys.path:
        sys.path.append(_p)

import numpy as np
import ml_dtypes
from contextlib import ExitStack

import concourse.bass as bass
import concourse.bacc as bacc
import concourse.mybir as mybir
import concourse.tile as tile
from concourse.bass_utils import run_bass_kernel_spmd

BF16 = mybir.dt.bfloat16
F32 = mybir.dt.float32
NPBF16 = ml_dtypes.bfloat16

N_CORES = 8
S = 2048          # sequence length
DM = 2048         # model dim
H = 16            # heads
HD = 128          # head dim
HPC = H // N_CORES  # heads per core = 2
CW = HPC * HD     # per-core projection width = 256
P = 128
HW = HD // 2      # 64
QT = 512          # query tile (free dim of attention matmuls)
NQT = S // QT     # 4 query tiles per head
NSC = S // P      # 16 sequence chunks
NKC = DM // P     # 16 contraction chunks
NST = S // QT     # 4 s-tiles
SOFTCAP = 50.0
C1 = 1.0 / (SOFTCAP * np.sqrt(HD))

Tanh = mybir.ActivationFunctionType.Tanh
Exp = mybir.ActivationFunctionType.Exp


def build_nc(reps=1, single=False):
    nc = bacc.Bacc("TRN2", target_bir_lowering=False, num_devices=N_CORES)

    xT_d = nc.dram_tensor("xT", [DM, S], BF16, kind="ExternalInput")
    w_d = nc.dram_tensor("w_all", [DM, 3 * CW], BF16, kind="ExternalInput")
    wo_d = nc.dram_tensor("wo_c", [CW, DM], BF16, kind="ExternalInput")
    cos_d = nc.dram_tensor("cosT2", [P, S], BF16, kind="ExternalInput")
    sin_d = nc.dram_tensor("sinT2", [P, S], BF16, kind="ExternalInput")
    mask_d = nc.dram_tensor("mask", [P, 4 * QT], BF16, kind="ExternalInput")
    out_d = nc.dram_tensor("outT", [DM, S], BF16, kind="ExternalOutput")

    with tile.TileContext(nc) as tc:
        for _rep in range(reps):
            _emit_body(nc, tc, xT_d, w_d, wo_d, cos_d, sin_d, mask_d, out_d)
    nc.compile()
    return nc


def _emit_body(nc, tc, xT_d, w_d, wo_d, cos_d, sin_d, mask_d, out_d):
    with ExitStack() as ctx:
        # ---------- persistent SBUF ----------
        persist = ctx.enter_context(tc.tile_pool(name="persist", bufs=1))
        qT = [persist.tile([P, S], BF16, name=f"qT{h}") for h in range(HPC)]
        kT = [persist.tile([P, S], BF16, name=f"kT{h}") for h in range(HPC)]
        v_sb = [persist.tile([P, S], BF16, name=f"v{h}") for h in range(HPC)]
        oT = [persist.tile([P, S], BF16, name=f"oT{h}") for h in range(HPC)]
        mask_sb = persist.tile([P, 4 * QT], BF16, name="mask")
        ones_bf = persist.tile([P, 1], BF16, name="ones")
        cos_sb = persist.tile([P, S], BF16, name="cosT2")
        sin_sb = persist.tile([P, S], BF16, name="sinT2")
        wo_sb = [persist.tile([P, DM], BF16, name=f"wo{h}") for h in range(HPC)]
        xp = ctx.enter_context(tc.tile_pool(name="xT", bufs=1))
        wp = ctx.enter_context(tc.tile_pool(name="w", bufs=1))
        rp = ctx.enter_context(tc.tile_pool(name="rope", bufs=4))

        # DMA priority order (HWDGE + the transfer engines serialize, so
        # issue order IS arrival order): rope tables, then the pre-phase
        # wavefront (w cols [q0|k0|v] + x first halves, k-interleaved),
        # then x second halves, mask, w cols [q1|k1], wo. w_all columns
        # are host-reordered to [q0, k0, v, q1, k1] to enable the split.
        # Batched loads: HWDGE issue bandwidth (~0.63us per DMA) is the
        # startup bottleneck, so w/x load as 4-k-group DMAs via 3-D tiles
        # and partition-inner DRAM views; x additionally splits into
        # column halves so the first s-tiles unblock early.
        wt_all = wp.tile([P, NKC, 3 * CW], BF16, name="wt")
        xt_all = xp.tile([P, NKC, S], BF16, name="xt")
        w_r = w_d.rearrange("(k p) c -> p k c", p=P)
        x_r = xT_d.rearrange("(k p) c -> p k c", p=P)
        HS = S // 2
        WA = 2 * P + CW  # wavefront w cols: q0 | k0 | v
        # first k-group split in two so the very first matmuls start ~3us
        nc.sync.dma_start(out=wt_all[:, 0:2, 0:WA], in_=w_r[:, 0:2, 0:WA])
        nc.scalar.dma_start(out=xt_all[:, 0:2, 0:HS], in_=x_r[:, 0:2, 0:HS])
        nc.sync.dma_start(out=wt_all[:, 2:4, 0:WA], in_=w_r[:, 2:4, 0:WA])
        nc.scalar.dma_start(out=xt_all[:, 2:4, 0:HS], in_=x_r[:, 2:4, 0:HS])
        # rope tables: needed ~10us in, after the first k-group
        nc.scalar.dma_start(out=cos_sb[:], in_=cos_d[:])
        nc.scalar.dma_start(out=sin_sb[:], in_=sin_d[:])
        for g in range(4, NKC, 4):
            nc.sync.dma_start(
                out=wt_all[:, g:g + 4, 0:WA], in_=w_r[:, g:g + 4, 0:WA])
            nc.scalar.dma_start(
                out=xt_all[:, g:g + 4, 0:HS], in_=x_r[:, g:g + 4, 0:HS])
        for g in range(0, NKC, 4):
            (nc.sync if g % 8 == 0 else nc.scalar).dma_start(
                out=xt_all[:, g:g + 4, HS:S], in_=x_r[:, g:g + 4, HS:S])
        for g in range(0, NKC, 8):  # q1|k1 w cols, first needed mid-B0
            nc.sync.dma_start(
                out=wt_all[:, g:g + 8, WA:3 * CW],
                in_=w_r[:, g:g + 8, WA:3 * CW])
        nc.sync.dma_start(out=mask_sb[:], in_=mask_d[:])
        nc.vector.memset(ones_bf[:], 1.0)
        for h in range(HPC):
            nc.sync.dma_start(out=wo_sb[h][:], in_=wo_d[h * P:(h + 1) * P, :])

        # w_all column offsets after host reorder [q0, k0, v, q1, k1]
        W_OFF = {0: 0, 2: P, 1: 2 * P + CW, 3: 3 * P + CW}
        V_OFF = 2 * P

        def qk_chunks(pool, c, st):
            """q/k feature chunk c (0: q-h0, 1: q-h1, 2: k-h0, 3: k-h1),
            s-tile st, transposed layout + fused rope, as 4 PE micro-steps."""
            dst = (qT, kT)[c // HPC][c % HPC]
            wo_ = W_OFF[c]
            state = {}

            def mm(k0):
                def f():
                    if k0 == 0:
                        state["ps"] = pool.tile([P, QT], F32, name="f")
                    ps = state["ps"]
                    for k in range(k0, k0 + 4):
                        nc.tensor.matmul(
                            ps[:],
                            wt_all[:, k, wo_:wo_ + P],
                            xt_all[:, k, st * QT:(st + 1) * QT],
                            start=(k == 0), stop=(k == NKC - 1),
                        )
                    if k0 == NKC - 4:
                        ps = state["ps"]
                        cs = slice(st * QT, (st + 1) * QT)
                        # Pool cannot read PSUM: both rope multiplies run
                        # on DVE; the all-SBUF add goes to Pool.
                        t1 = rp.tile([P, QT], BF16, name="t1")
                        nc.vector.tensor_mul(t1[:], ps[:], cos_sb[:, cs])
                        t2 = rp.tile([P, QT], BF16, name="t2")
                        nc.vector.tensor_mul(
                            t2[0:HW, :], ps[HW:HD, :], sin_sb[0:HW, cs])
                        nc.vector.tensor_mul(
                            t2[HW:HD, :], ps[0:HW, :], sin_sb[HW:HD, cs])
                        nc.gpsimd.tensor_add(dst[:, cs], t1[:], t2[:])
                return f
            return [mm(k0) for k0 in range(0, NKC, 4)]

        def v_chunks(pool, sc):
            """v s-chunk sc in natural layout, as 2 PE micro-steps."""
            state = {}

            def mm(k0):
                def f():
                    if k0 == 0:
                        state["ps"] = pool.tile([P, QT], F32, name="f")
                    ps = state["ps"]
                    for k in range(k0, k0 + 8):
                        nc.tensor.matmul(
                            ps[:, 0:CW],
                            xt_all[:, k, sc * P:(sc + 1) * P],
                            wt_all[:, k, V_OFF:V_OFF + CW],
                            start=(k == 0), stop=(k == NKC - 1),
                        )
                    if k0 == NKC - 8:
                        for h in range(HPC):
                            nc.vector.tensor_copy(
                                v_sb[h][:, sc * P:(sc + 1) * P],
                                ps[:, h * HD:(h + 1) * HD],
                            )
                return f
            return [mm(0), mm(8)]

        o_r = out_d.rearrange("(o p) s -> p o s", p=P)

        def c_chunks(st, outp, c_ps, alt=False):
            """output-projection pieces for s-tile st, 1 PE micro-step each;
            results stage into 4-oc-wide tiles DMA'd as one transfer."""
            state = {}

            def piece(oc):
                def f():
                    acc = c_ps.tile([P, QT], F32, name="f")
                    for h in range(HPC):
                        nc.tensor.matmul(
                            acc[:],
                            wo_sb[h][:, oc * P:(oc + 1) * P],
                            oT[h][:, st * QT:(st + 1) * QT],
                            start=(h == 0), stop=(h == HPC - 1),
                        )
                    if oc % 4 == 0:
                        state["osb"] = outp.tile([P, 4, QT], BF16, name="osb")
                    osb = state["osb"]
                    # ACT carries the B1 tanh/exp chain: only 1 in 4 copies
                    # goes there — except in the drain round (alt), where
                    # ACT is free and copies alternate 50/50
                    if (oc % 2 == 0) if alt else (oc % 4 == 0):
                        nc.scalar.copy(osb[:, oc % 4, :], acc[:])
                    else:
                        nc.vector.tensor_copy(osb[:, oc % 4, :], acc[:])
                    if oc % 4 == 3:
                        nc.sync.dma_start(
                            out=o_r[:, oc - 3:oc + 1,
                                    st * QT:(st + 1) * QT],
                            in_=osb[:])
                return f
            return [piece(oc) for oc in range(NKC)]

        class Feeder:
            """Doles out independent PE micro-steps to hide ACT latency."""
            def __init__(self):
                self.chunks = []

            def add(self, chunks):
                self.chunks.extend(chunks)

            def step(self, n):
                for _ in range(n):
                    if self.chunks:
                        self.chunks.pop(0)()

            def drain(self):
                self.step(len(self.chunks))

        def emit_attn(h, t, pools, feeder, per_pair):
            s_ps, o_ps, l_ps, thp, pp, np_ = pools
            o_acc = o_ps.tile([P, QT], F32, name="o_acc")
            l_acc = l_ps.tile([1, QT], F32, name="l_acc")
            npair = 2 * t + 2
            q_ap = qT[h][:, t * QT:(t + 1) * QT]

            def emit_pv(pT, p, last):
                for i in range(2):
                    kc = 2 * p + i
                    nc.tensor.matmul(
                        o_acc[:],
                        v_sb[h][:, kc * P:(kc + 1) * P],
                        pT[:, i * QT:(i + 1) * QT],
                        start=(kc == 0), stop=(last and i == 1),
                    )
                    nc.tensor.matmul(
                        l_acc[:], ones_bf[:, 0:1],
                        pT[:, i * QT:(i + 1) * QT],
                        start=(kc == 0), stop=(last and i == 1),
                    )

            prev = None
            for p in range(npair):
                sp = s_ps.tile([P, 2 * QT], F32, name="sp")
                for i in range(2):
                    kc = 2 * p + i
                    nc.tensor.matmul(
                        sp[:, i * QT:(i + 1) * QT],
                        kT[h][:, kc * P:(kc + 1) * P], q_ap,
                        start=True, stop=True,
                    )
                feeder.step(per_pair)
                th = thp.tile([P, 2 * QT], F32, name="th")
                nc.scalar.activation(th[:], sp[:], Tanh, scale=C1)
                pT = pp.tile([P, 2 * QT], BF16, name="pTt")
                nc.scalar.activation(pT[:], th[:], Exp, scale=SOFTCAP)
                # masked pairs are the last two: p==2t (u=0,1), p==2t+1 (u=2,3)
                u0 = 2 * (p - 2 * t)
                if u0 >= 0:
                    nc.vector.tensor_mul(
                        pT[:], pT[:], mask_sb[:, u0 * QT:(u0 + 2) * QT])
                if prev is not None:
                    emit_pv(prev[0], prev[1], last=False)
                prev = (pT, p)
            emit_pv(prev[0], prev[1], last=True)
            recip = np_.tile([1, QT], F32, name="recip")
            nc.vector.reciprocal(recip[:], l_acc[:])
            bcast = np_.tile([P, QT], F32, name="bcast")
            nc.gpsimd.partition_broadcast(bcast[:], recip[:])
            nc.vector.tensor_mul(
                oT[h][:, t * QT:(t + 1) * QT], o_acc[:], bcast[:])

        # ---------- phase A (pre-attention part) ----------
        # head 0's q/k + the first 4 v chunks. Tiles needing only the x
        # first halves come first, k-interleaved within 3-tile windows so
        # the PE tracks the DMA wavefront instead of stalling on one tile.
        def interleave(units):
            out = []
            for step in range(max(len(u) for u in units)):
                for u in units:
                    if step < len(u):
                        out.append(u[step])
            return out

        # The A phase is DMA-bound (~35us of input wavefront), so all v
        # chunks ride along in its PE bubbles, ordered by which x quarter
        # they need.
        with ExitStack() as ctxA:
            qkA = ctxA.enter_context(
                tc.tile_pool(name="qkA", bufs=4, space="PSUM"))
            pre = Feeder()
            pre.add(interleave([qk_chunks(qkA, 0, 0), qk_chunks(qkA, 2, 0)]))
            pre.add(interleave([qk_chunks(qkA, 0, 1), qk_chunks(qkA, 2, 1)]))
            for sc in range(0, 8):
                pre.add(v_chunks(qkA, sc))
            pre.add(interleave([qk_chunks(qkA, 0, 2), qk_chunks(qkA, 2, 2)]))
            for sc in range(8, 12):
                pre.add(v_chunks(qkA, sc))
            pre.add(interleave([qk_chunks(qkA, 0, 3), qk_chunks(qkA, 2, 3)]))
            for sc in range(12, 16):
                pre.add(v_chunks(qkA, sc))
            pre.drain()

        # ---------- phase B0: head-0 attention + A-fill ----------
        # shared fill/output-projection PSUM pool (one tag, 2 banks);
        # created below the B pools so those can close before the drain
        fps = ctx.enter_context(tc.tile_pool(name="fps", bufs=2, space="PSUM"))
        outp = ctx.enter_context(tc.tile_pool(name="out", bufs=4))
        fill = Feeder()
        with ExitStack() as ctxB:
            s_ps = ctxB.enter_context(
                tc.tile_pool(name="s_ps", bufs=2, space="PSUM"))
            o_ps = ctxB.enter_context(
                tc.tile_pool(name="o_ps", bufs=1, space="PSUM"))
            l_ps = ctxB.enter_context(
                tc.tile_pool(name="l_ps", bufs=1, space="PSUM"))
            thp = ctxB.enter_context(tc.tile_pool(name="tanh", bufs=3))
            pp = ctxB.enter_context(tc.tile_pool(name="pT", bufs=3))
            np_ = ctxB.enter_context(tc.tile_pool(name="norm", bufs=2))
            bpools = (s_ps, o_ps, l_ps, thp, pp, np_)

            for st in (0, 1):
                fill.add(qk_chunks(fps, 1, st))
                fill.add(qk_chunks(fps, 3, st))
            for t in range(NQT):
                emit_attn(0, t, bpools, fill, per_pair=2)
            # q1/k1 st2/st3 are first needed by B1 t2/t3: defer them into
            # the otherwise-unfilled B1 t0/t1 windows.
            for st in (2, 3):
                fill.add(qk_chunks(fps, 1, st))
                fill.add(qk_chunks(fps, 3, st))

            # ---------- phase B1 + C: attention + output projection ----
            for t, per in zip(range(NQT), (6, 5, 3, 2)):
                emit_attn(1, t, bpools, fill, per_pair=per)
                if t < NQT - 1:
                    fill.add(c_chunks(t, outp, fps))
        # drain round: B pools are closed, give the last output-projection
        # round a wide PSUM pool so its pieces pipeline
        cD = ctx.enter_context(tc.tile_pool(name="cD", bufs=5, space="PSUM"))
        fill.add(c_chunks(NQT - 1, outp, cD, alt=True))
        fill.drain()


_NC_CACHE = None


def _get_nc():
    global _NC_CACHE
    if _NC_CACHE is None:
        _NC_CACHE = build_nc()
    return _NC_CACHE


def _rope_perm():
    """per-head column permutation de-interleaving rotary pairs"""
    perm = np.zeros(DM, np.int64)
    for h in range(H):
        base = h * HD
        perm[base:base + HD // 2] = base + np.arange(0, HD, 2)
        perm[base + HD // 2:base + HD] = base + np.arange(1, HD, 2)
    return perm


def make_in_maps(x, wq, wk, wv, wo, freqs_cos, freqs_sin):
    x = np.asarray(x, np.float32).reshape(S, DM)
    wq = np.asarray(wq, np.float32)
    wk = np.asarray(wk, np.float32)
    wv = np.asarray(wv, np.float32)
    wo = np.asarray(wo, np.float32)
    xT = np.ascontiguousarray(x.T).astype(NPBF16)
    perm = _rope_perm()
    wq_p = wq[:, perm]
    wk_p = wk[:, perm]
    # transposed rope tables: C = [cosT; cosT], S' = [-sinT; sinT]
    cosT = np.asarray(freqs_cos, np.float32).T  # [64, S]
    sinT = np.asarray(freqs_sin, np.float32).T
    cosT2 = np.concatenate([cosT, cosT], axis=0).astype(NPBF16)
    sinT2 = np.concatenate([-sinT, sinT], axis=0).astype(NPBF16)
    # mask[i, u*QT + j] = 1 if i <= j - 128*u else 0  (keep kj <= qi)
    i_idx = np.arange(P)[:, None]
    j_idx = np.arange(QT)[None, :]
    mask = np.concatenate(
        [(i_idx <= j_idx - P * u) for u in range(4)], axis=1
    ).astype(NPBF16)
    in_maps = []
    for c in range(N_CORES):
        cs = slice(c * CW, (c + 1) * CW)
        h0 = slice(c * CW, c * CW + HD)
        h1 = slice(c * CW + HD, (c + 1) * CW)
        # device column order: [q-h0, k-h0, v, q-h1, k-h1]
        w_all = np.concatenate(
            [wq_p[:, h0], wk_p[:, h0], wv[:, cs],
             wq_p[:, h1], wk_p[:, h1]], axis=1).astype(NPBF16)
        wo_c = np.ascontiguousarray(wo[cs, :]).astype(NPBF16)
        in_maps.append({
            "xT": xT,
            "w_all": np.ascontiguousarray(w_all),
            "wo_c": wo_c,
            "cosT2": cosT2,
            "sinT2": sinT2,
            "mask": mask,
        })
    return in_maps


def assemble_output(results):
    acc = results[0]["outT"].astype(np.float32)
    for r in results[1:]:
        acc += np.asarray(r["outT"]).astype(np.float32)
    return np.ascontiguousarray(acc.T).reshape(1, S, DM).astype(np.float32)


def kernel(x, wq, wk, wv, wo, freqs_cos, freqs_sin):
    nc = _get_nc()
    in_maps = make_in_maps(x, wq, wk, wv, wo, freqs_cos, freqs_sin)
    res = run_bass_kernel_spmd(nc, in_maps, core_ids=list(range(N_CORES)))
    return assemble_output(res.results)


if __name__ == "__main__":
    rng = np.random.default_rng(0)
    ins = {
        "x": rng.standard_normal((1, S, DM), np.float32),
        "wq": rng.standard_normal((DM, DM), np.float32) / np.sqrt(DM),
        "wk": rng.standard_normal((DM, DM), np.float32) / np.sqrt(DM),
        "wv": rng.standard_normal((DM, DM), np.float32) / np.sqrt(DM),
        "wo": rng.standard_normal((DM, DM), np.float32) / np.sqrt(DM),
        "freqs_cos": rng.standard_normal((S, HD // 2), np.float32),
        "freqs_sin": rng.standard_normal((S, HD // 2), np.float32),
    }
    out = kernel(**ins)
    print("out", out.shape, out.dtype, np.abs(out).mean())


# revision 57
# speedup vs baseline: 1.0349x; 1.0040x over previous
"""Trainium2 Bass kernel for Llama-like attention (16 heads, tanh softcap, RoPE).

Sharding: tensor-parallel over heads, fully collective-free. Each of the 8
cores computes 2 heads end-to-end and a *partial* output projection
(o_local @ wo_rows_local)^T; the host sums the 8 partial outputs. With no
on-device collective, each core's NEFF span is pure local compute — no
cross-core rendezvous.

Per-core pipeline (engine-balanced against the ~165us PE floor):
  - q^T/k^T computed directly in transposed layout ([head_dim, s]) via
    matmul(w_slice^T, x^T): no PE transposes. Weight columns of wq/wk are
    pre-permuted on the host to de-interleave even/odd rotary pairs (the
    permutation cancels inside q.k).
  - RoPE in transposed layout straight out of PSUM: rot = A*C + swap(A)*S'
    with C = [cosT; cosT], S' = [-sinT; sinT]. The partition-half swap is
    two half-height Vector multiplies reading PSUM at a partition offset
    (Pool cannot touch PSUM); the all-SBUF add runs on Pool. ACT stays
    free for the softmax chain.
  - attention with scores transposed ([kj, qi]) so softmaxed probabilities
    feed the PV matmul directly as the moving operand. tanh softcap bounds
    scores, so softmax needs no row-max pass: p = exp(50*tanh(.)),
    l = ones-row matmul, o = p@v / l. Head 0's ACT-bound window is filled
    with head 1's q/k projection and the tail v chunks; head 1's windows
    are filled with the output-projection pieces for the q-tile that just
    finished.
  - output projection pieces acc[oc(128), st(512)] += wo_h[:, oc]^T @ oT_h
    accumulated over the 2 local heads, copied to SBUF bf16 (ACT/DVE
    alternating) and DMA'd per piece. Host sums partials and transposes.
"""

import os
import sys

for _p in ("/root/.axon_site/_ro/trn_rl_repo", "/opt/trn_rl_repo"):
    if os.path.isdir(_p) and _p not in sys.path:
        sys.path.append(_p)

import numpy as np
import ml_dtypes
from contextlib import ExitStack

import concourse.bass as bass
import concourse.bacc as bacc
import concourse.mybir as mybir
import concourse.tile as tile
from concourse.bass_utils import run_bass_kernel_spmd

BF16 = mybir.dt.bfloat16
F32 = mybir.dt.float32
NPBF16 = ml_dtypes.bfloat16

N_CORES = 8
S = 2048          # sequence length
DM = 2048         # model dim
H = 16            # heads
HD = 128          # head dim
HPC = H // N_CORES  # heads per core = 2
CW = HPC * HD     # per-core projection width = 256
P = 128
HW = HD // 2      # 64
QT = 512          # query tile (free dim of attention matmuls)
NQT = S // QT     # 4 query tiles per head
NSC = S // P      # 16 sequence chunks
NKC = DM // P     # 16 contraction chunks
NST = S // QT     # 4 s-tiles
SOFTCAP = 50.0
C1 = 1.0 / (SOFTCAP * np.sqrt(HD))

Tanh = mybir.ActivationFunctionType.Tanh
Exp = mybir.ActivationFunctionType.Exp


def build_nc(reps=1, single=False):
    nc = bacc.Bacc("TRN2", target_bir_lowering=False, num_devices=N_CORES)

    xT_d = nc.dram_tensor("xT", [DM, S], BF16, kind="ExternalInput")
    w_d = nc.dram_tensor("w_all", [DM, 3 * CW], BF16, kind="ExternalInput")
    wo_d = nc.dram_tensor("wo_c", [CW, DM], BF16, kind="ExternalInput")
    cos_d = nc.dram_tensor("cosT2", [P, S], BF16, kind="ExternalInput")
    sin_d = nc.dram_tensor("sinT2", [P, S], BF16, kind="ExternalInput")
    mask_d = nc.dram_tensor("mask", [P, 4 * QT], BF16, kind="ExternalInput")
    out_d = nc.dram_tensor("outT", [DM, S], BF16, kind="ExternalOutput")

    with tile.TileContext(nc) as tc:
        for _rep in range(reps):
            _emit_body(nc, tc, xT_d, w_d, wo_d, cos_d, sin_d, mask_d, out_d)
    nc.compile()
    return nc


def _emit_body(nc, tc, xT_d, w_d, wo_d, cos_d, sin_d, mask_d, out_d):
    with ExitStack() as ctx:
        # ---------- persistent SBUF ----------
        persist = ctx.enter_context(tc.tile_pool(name="persist", bufs=1))
        qT = [persist.tile([P, S], BF16, name=f"qT{h}") for h in range(HPC)]
        kT = [persist.tile([P, S], BF16, name=f"kT{h}") for h in range(HPC)]
        v_sb = [persist.tile([P, S], BF16, name=f"v{h}") for h in range(HPC)]
        oT = [persist.tile([P, S], BF16, name=f"oT{h}") for h in range(HPC)]
        mask_sb = persist.tile([P, 4 * QT], BF16, name="mask")
        ones_bf = persist.tile([P, 1], BF16, name="ones")
        cos_sb = persist.tile([P, S], BF16, name="cosT2")
        sin_sb = persist.tile([P, S], BF16, name="sinT2")
        wo_sb = [persist.tile([P, DM], BF16, name=f"wo{h}") for h in range(HPC)]
        xp = ctx.enter_context(tc.tile_pool(name="xT", bufs=1))
        wp = ctx.enter_context(tc.tile_pool(name="w", bufs=1))
        rp = ctx.enter_context(tc.tile_pool(name="rope", bufs=4))

        # DMA priority order (HWDGE + the transfer engines serialize, so
        # issue order IS arrival order): rope tables, then the pre-phase
        # wavefront (w cols [q0|k0|v] + x first halves, k-interleaved),
        # then x second halves, mask, w cols [q1|k1], wo. w_all columns
        # are host-reordered to [q0, k0, v, q1, k1] to enable the split.
        # Batched loads: HWDGE issue bandwidth (~0.63us per DMA) is the
        # startup bottleneck, so w/x load as 4-k-group DMAs via 3-D tiles
        # and partition-inner DRAM views; x additionally splits into
        # column halves so the first s-tiles unblock early.
        wt_all = wp.tile([P, NKC, 3 * CW], BF16, name="wt")
        xt_all = xp.tile([P, NKC, S], BF16, name="xt")
        w_r = w_d.rearrange("(k p) c -> p k c", p=P)
        x_r = xT_d.rearrange("(k p) c -> p k c", p=P)
        HS = S // 2
        WA = 2 * P + CW  # wavefront w cols: q0 | k0 | v
        # first k-group split in two so the very first matmuls start ~3us
        nc.sync.dma_start(out=wt_all[:, 0:2, 0:WA], in_=w_r[:, 0:2, 0:WA])
        nc.scalar.dma_start(out=xt_all[:, 0:2, 0:HS], in_=x_r[:, 0:2, 0:HS])
        nc.sync.dma_start(out=wt_all[:, 2:4, 0:WA], in_=w_r[:, 2:4, 0:WA])
        nc.scalar.dma_start(out=xt_all[:, 2:4, 0:HS], in_=x_r[:, 2:4, 0:HS])
        # rope tables: needed ~10us in, after the first k-group
        nc.scalar.dma_start(out=cos_sb[:], in_=cos_d[:])
        nc.scalar.dma_start(out=sin_sb[:], in_=sin_d[:])
        for g in range(4, NKC, 4):
            nc.sync.dma_start(
                out=wt_all[:, g:g + 4, 0:WA], in_=w_r[:, g:g + 4, 0:WA])
            nc.scalar.dma_start(
                out=xt_all[:, g:g + 4, 0:HS], in_=x_r[:, g:g + 4, 0:HS])
        for g in range(0, NKC, 4):
            (nc.sync if g % 8 == 0 else nc.scalar).dma_start(
                out=xt_all[:, g:g + 4, HS:S], in_=x_r[:, g:g + 4, HS:S])
        for g in range(0, NKC, 8):  # q1|k1 w cols, first needed mid-B0
            nc.sync.dma_start(
                out=wt_all[:, g:g + 8, WA:3 * CW],
                in_=w_r[:, g:g + 8, WA:3 * CW])
        nc.sync.dma_start(out=mask_sb[:], in_=mask_d[:])
        nc.vector.memset(ones_bf[:], 1.0)
        for h in range(HPC):
            nc.sync.dma_start(out=wo_sb[h][:], in_=wo_d[h * P:(h + 1) * P, :])

        # w_all column offsets after host reorder [q0, k0, v, q1, k1]
        W_OFF = {0: 0, 2: P, 1: 2 * P + CW, 3: 3 * P + CW}
        V_OFF = 2 * P

        def qk_chunks(pool, c, st):
            """q/k feature chunk c (0: q-h0, 1: q-h1, 2: k-h0, 3: k-h1),
            s-tile st, transposed layout + fused rope, as 4 PE micro-steps."""
            dst = (qT, kT)[c // HPC][c % HPC]
            wo_ = W_OFF[c]
            state = {}

            def mm(k0):
                def f():
                    if k0 == 0:
                        state["ps"] = pool.tile([P, QT], F32, name="f")
                    ps = state["ps"]
                    for k in range(k0, k0 + 4):
                        nc.tensor.matmul(
                            ps[:],
                            wt_all[:, k, wo_:wo_ + P],
                            xt_all[:, k, st * QT:(st + 1) * QT],
                            start=(k == 0), stop=(k == NKC - 1),
                        )
                    if k0 == NKC - 4:
                        ps = state["ps"]
                        cs = slice(st * QT, (st + 1) * QT)
                        # Pool cannot read PSUM: both rope multiplies run
                        # on DVE; the all-SBUF add goes to Pool.
                        t1 = rp.tile([P, QT], BF16, name="t1")
                        nc.vector.tensor_mul(t1[:], ps[:], cos_sb[:, cs])
                        t2 = rp.tile([P, QT], BF16, name="t2")
                        nc.vector.tensor_mul(
                            t2[0:HW, :], ps[HW:HD, :], sin_sb[0:HW, cs])
                        nc.vector.tensor_mul(
                            t2[HW:HD, :], ps[0:HW, :], sin_sb[HW:HD, cs])
                        nc.gpsimd.tensor_add(dst[:, cs], t1[:], t2[:])
                return f
            return [mm(k0) for k0 in range(0, NKC, 4)]

        def v_chunks(pool, sc):
            """v s-chunk sc in natural layout, as 2 PE micro-steps."""
            state = {}

            def mm(k0):
                def f():
                    if k0 == 0:
                        state["ps"] = pool.tile([P, QT], F32, name="f")
                    ps = state["ps"]
                    for k in range(k0, k0 + 8):
                        nc.tensor.matmul(
                            ps[:, 0:CW],
                            xt_all[:, k, sc * P:(sc + 1) * P],
                            wt_all[:, k, V_OFF:V_OFF + CW],
                            start=(k == 0), stop=(k == NKC - 1),
                        )
                    if k0 == NKC - 8:
                        for h in range(HPC):
                            nc.vector.tensor_copy(
                                v_sb[h][:, sc * P:(sc + 1) * P],
                                ps[:, h * HD:(h + 1) * HD],
                            )
                return f
            return [mm(0), mm(8)]

        o_r = out_d.rearrange("(o p) s -> p o s", p=P)

        def c_chunks(st, outp, c_ps, alt=False):
            """output-projection pieces for s-tile st, 1 PE micro-step each;
            results stage into 4-oc-wide tiles DMA'd as one transfer."""
            state = {}

            def piece(oc):
                def f():
                    acc = c_ps.tile([P, QT], F32, name="f")
                    for h in range(HPC):
                        nc.tensor.matmul(
                            acc[:],
                            wo_sb[h][:, oc * P:(oc + 1) * P],
                            oT[h][:, st * QT:(st + 1) * QT],
                            start=(h == 0), stop=(h == HPC - 1),
                        )
                    if oc % 4 == 0:
                        state["osb"] = outp.tile([P, 4, QT], BF16, name="osb")
                    osb = state["osb"]
                    # ACT carries the B1 tanh/exp chain: only 1 in 4 copies
                    # goes there — except in the drain round (alt), where
                    # ACT is free and copies alternate 50/50
                    if (oc % 2 == 0) if alt else (oc % 4 == 0):
                        nc.scalar.copy(osb[:, oc % 4, :], acc[:])
                    else:
                        nc.vector.tensor_copy(osb[:, oc % 4, :], acc[:])
                    if oc % 4 == 3:
                        nc.sync.dma_start(
                            out=o_r[:, oc - 3:oc + 1,
                                    st * QT:(st + 1) * QT],
                            in_=osb[:])
                return f
            return [piece(oc) for oc in range(NKC)]

        class Feeder:
            """Doles out independent PE micro-steps to hide ACT latency."""
            def __init__(self):
                self.chunks = []

            def add(self, chunks):
                self.chunks.extend(chunks)

            def step(self, n):
                for _ in range(n):
                    if self.chunks:
                        self.chunks.pop(0)()

            def drain(self):
                self.step(len(self.chunks))

        def emit_attn(h, t, pools, feeder, per_pair):
            s_ps, o_ps, l_ps, thp, pp, np_ = pools
            o_acc = o_ps.tile([P, QT], F32, name="o_acc")
            l_acc = l_ps.tile([1, QT], F32, name="l_acc")
            npair = 2 * t + 2
            q_ap = qT[h][:, t * QT:(t + 1) * QT]

            def emit_pv(pT, p, last):
                for i in range(2):
                    kc = 2 * p + i
                    nc.tensor.matmul(
                        o_acc[:],
                        v_sb[h][:, kc * P:(kc + 1) * P],
                        pT[:, i * QT:(i + 1) * QT],
                        start=(kc == 0), stop=(last and i == 1),
                    )
                    nc.tensor.matmul(
                        l_acc[:], ones_bf[:, 0:1],
                        pT[:, i * QT:(i + 1) * QT],
                        start=(kc == 0), stop=(last and i == 1),
                    )

            prev = None
            for p in range(npair):
                sp = s_ps.tile([P, 2 * QT], F32, name="sp")
                for i in range(2):
                    kc = 2 * p + i
                    nc.tensor.matmul(
                        sp[:, i * QT:(i + 1) * QT],
                        kT[h][:, kc * P:(kc + 1) * P], q_ap,
                        start=True, stop=True,
                    )
                feeder.step(per_pair)
                th = thp.tile([P, 2 * QT], F32, name="th")
                nc.scalar.activation(th[:], sp[:], Tanh, scale=C1)
                pT = pp.tile([P, 2 * QT], BF16, name="pTt")
                nc.scalar.activation(pT[:], th[:], Exp, scale=SOFTCAP)
                # masked pairs are the last two: p==2t (u=0,1), p==2t+1 (u=2,3)
                u0 = 2 * (p - 2 * t)
                if u0 >= 0:
                    nc.vector.tensor_mul(
                        pT[:], pT[:], mask_sb[:, u0 * QT:(u0 + 2) * QT])
                if prev is not None:
                    emit_pv(prev[0], prev[1], last=False)
                prev = (pT, p)
            emit_pv(prev[0], prev[1], last=True)
            recip = np_.tile([1, QT], F32, name="recip")
            nc.vector.reciprocal(recip[:], l_acc[:])
            bcast = np_.tile([P, QT], F32, name="bcast")
            nc.gpsimd.partition_broadcast(bcast[:], recip[:])
            nc.vector.tensor_mul(
                oT[h][:, t * QT:(t + 1) * QT], o_acc[:], bcast[:])

        # ---------- phase A (pre-attention part) ----------
        # head 0's q/k + the first 4 v chunks. Tiles needing only the x
        # first halves come first, k-interleaved within 3-tile windows so
        # the PE tracks the DMA wavefront instead of stalling on one tile.
        def interleave(units):
            out = []
            for step in range(max(len(u) for u in units)):
                for u in units:
                    if step < len(u):
                        out.append(u[step])
            return out

        # The A phase is DMA-bound (~35us of input wavefront), so all v
        # chunks ride along in its PE bubbles, ordered by which x quarter
        # they need.
        with ExitStack() as ctxA:
            qkA = ctxA.enter_context(
                tc.tile_pool(name="qkA", bufs=4, space="PSUM"))
            pre = Feeder()
            pre.add(interleave([qk_chunks(qkA, 0, 0), qk_chunks(qkA, 2, 0)]))
            pre.add(interleave([qk_chunks(qkA, 0, 1), qk_chunks(qkA, 2, 1)]))
            for sc in range(0, 8):
                pre.add(v_chunks(qkA, sc))
            pre.add(interleave([qk_chunks(qkA, 0, 2), qk_chunks(qkA, 2, 2)]))
            for sc in range(8, 12):
                pre.add(v_chunks(qkA, sc))
            pre.add(interleave([qk_chunks(qkA, 0, 3), qk_chunks(qkA, 2, 3)]))
            for sc in range(12, 16):
                pre.add(v_chunks(qkA, sc))
            pre.drain()

        # ---------- phase B0: head-0 attention + A-fill ----------
        # shared fill/output-projection PSUM pool (one tag, 2 banks);
        # created below the B pools so those can close before the drain
        fps = ctx.enter_context(tc.tile_pool(name="fps", bufs=2, space="PSUM"))
        outp = ctx.enter_context(tc.tile_pool(name="out", bufs=4))
        fill = Feeder()
        with ExitStack() as ctxB:
            s_ps = ctxB.enter_context(
                tc.tile_pool(name="s_ps", bufs=2, space="PSUM"))
            o_ps = ctxB.enter_context(
                tc.tile_pool(name="o_ps", bufs=1, space="PSUM"))
            l_ps = ctxB.enter_context(
                tc.tile_pool(name="l_ps", bufs=1, space="PSUM"))
            thp = ctxB.enter_context(tc.tile_pool(name="tanh", bufs=3))
            pp = ctxB.enter_context(tc.tile_pool(name="pT", bufs=3))
            np_ = ctxB.enter_context(tc.tile_pool(name="norm", bufs=2))
            bpools = (s_ps, o_ps, l_ps, thp, pp, np_)

            for st in (0, 1):
                fill.add(qk_chunks(fps, 1, st))
                fill.add(qk_chunks(fps, 3, st))
            for t in range(NQT):
                emit_attn(0, t, bpools, fill, per_pair=3)
            # q1/k1 st2/st3 are first needed by B1 t2/t3: defer them into
            # the otherwise-unfilled B1 t0/t1 windows.
            for st in (2, 3):
                fill.add(qk_chunks(fps, 1, st))
                fill.add(qk_chunks(fps, 3, st))

            # ---------- phase B1 + C: attention + output projection ----
            for t, per in zip(range(NQT), (6, 5, 3, 2)):
                emit_attn(1, t, bpools, fill, per_pair=per)
                if t < NQT - 1:
                    fill.add(c_chunks(t, outp, fps))
        # drain round: B pools are closed, give the last output-projection
        # round a wide PSUM pool so its pieces pipeline
        cD = ctx.enter_context(tc.tile_pool(name="cD", bufs=5, space="PSUM"))
        fill.add(c_chunks(NQT - 1, outp, cD, alt=True))
        fill.drain()


_NC_CACHE = None


def _get_nc():
    global _NC_CACHE
    if _NC_CACHE is None:
        _NC_CACHE = build_nc()
    return _NC_CACHE


def _rope_perm():
    """per-head column permutation de-interleaving rotary pairs"""
    perm = np.zeros(DM, np.int64)
    for h in range(H):
        base = h * HD
        perm[base:base + HD // 2] = base + np.arange(0, HD, 2)
        perm[base + HD // 2:base + HD] = base + np.arange(1, HD, 2)
    return perm


def make_in_maps(x, wq, wk, wv, wo, freqs_cos, freqs_sin):
    x = np.asarray(x, np.float32).reshape(S, DM)
    wq = np.asarray(wq, np.float32)
    wk = np.asarray(wk, np.float32)
    wv = np.asarray(wv, np.float32)
    wo = np.asarray(wo, np.float32)
    xT = np.ascontiguousarray(x.T).astype(NPBF16)
    perm = _rope_perm()
    wq_p = wq[:, perm]
    wk_p = wk[:, perm]
    # transposed rope tables: C = [cosT; cosT], S' = [-sinT; sinT]
    cosT = np.asarray(freqs_cos, np.float32).T  # [64, S]
    sinT = np.asarray(freqs_sin, np.float32).T
    cosT2 = np.concatenate([cosT, cosT], axis=0).astype(NPBF16)
    sinT2 = np.concatenate([-sinT, sinT], axis=0).astype(NPBF16)
    # mask[i, u*QT + j] = 1 if i <= j - 128*u else 0  (keep kj <= qi)
    i_idx = np.arange(P)[:, None]
    j_idx = np.arange(QT)[None, :]
    mask = np.concatenate(
        [(i_idx <= j_idx - P * u) for u in range(4)], axis=1
    ).astype(NPBF16)
    in_maps = []
    for c in range(N_CORES):
        cs = slice(c * CW, (c + 1) * CW)
        h0 = slice(c * CW, c * CW + HD)
        h1 = slice(c * CW + HD, (c + 1) * CW)
        # device column order: [q-h0, k-h0, v, q-h1, k-h1]
        w_all = np.concatenate(
            [wq_p[:, h0], wk_p[:, h0], wv[:, cs],
             wq_p[:, h1], wk_p[:, h1]], axis=1).astype(NPBF16)
        wo_c = np.ascontiguousarray(wo[cs, :]).astype(NPBF16)
        in_maps.append({
            "xT": xT,
            "w_all": np.ascontiguousarray(w_all),
            "wo_c": wo_c,
            "cosT2": cosT2,
            "sinT2": sinT2,
            "mask": mask,
        })
    return in_maps


def assemble_output(results):
    acc = results[0]["outT"].astype(np.float32)
    for r in results[1:]:
        acc += np.asarray(r["outT"]).astype(np.float32)
    return np.ascontiguousarray(acc.T).reshape(1, S, DM).astype(np.float32)


def kernel(x, wq, wk, wv, wo, freqs_cos, freqs_sin):
    nc = _get_nc()
    in_maps = make_in_maps(x, wq, wk, wv, wo, freqs_cos, freqs_sin)
    res = run_bass_kernel_spmd(nc, in_maps, core_ids=list(range(N_CORES)))
    return assemble_output(res.results)


if __name__ == "__main__":
    rng = np.random.default_rng(0)
    ins = {
        "x": rng.standard_normal((1, S, DM), np.float32),
        "wq": rng.standard_normal((DM, DM), np.float32) / np.sqrt(DM),
        "wk": rng.standard_normal((DM, DM), np.float32) / np.sqrt(DM),
        "wv": rng.standard_normal((DM, DM), np.float32) / np.sqrt(DM),
        "wo": rng.standard_normal((DM, DM), np.float32) / np.sqrt(DM),
        "freqs_cos": rng.standard_normal((S, HD // 2), np.float32),
        "freqs_sin": rng.standard_normal((S, HD // 2), np.float32),
    }
    out = kernel(**ins)
    print("out", out.shape, out.dtype, np.abs(out).mean())


# revision 69
# speedup vs baseline: 1.0450x; 1.0098x over previous
"""Trainium2 Bass kernel for Llama-like attention (16 heads, tanh softcap, RoPE).

Sharding: tensor-parallel over heads, fully collective-free. Each of the 8
cores computes 2 heads end-to-end and a *partial* output projection
(o_local @ wo_rows_local)^T; the host sums the 8 partial outputs. With no
on-device collective, each core's NEFF span is pure local compute — no
cross-core rendezvous.

Per-core pipeline (engine-balanced against the ~165us PE floor):
  - q^T/k^T computed directly in transposed layout ([head_dim, s]) via
    matmul(w_slice^T, x^T): no PE transposes. Weight columns of wq/wk are
    pre-permuted on the host to de-interleave even/odd rotary pairs (the
    permutation cancels inside q.k).
  - RoPE in transposed layout straight out of PSUM: rot = A*C + swap(A)*S'
    with C = [cosT; cosT], S' = [-sinT; sinT]. The partition-half swap is
    two half-height Vector multiplies reading PSUM at a partition offset
    (Pool cannot touch PSUM); the all-SBUF add runs on Pool. ACT stays
    free for the softmax chain.
  - attention with scores transposed ([kj, qi]) so softmaxed probabilities
    feed the PV matmul directly as the moving operand. tanh softcap bounds
    scores, so softmax needs no row-max pass: p = exp(50*tanh(.)),
    l = ones-row matmul, o = p@v / l. Head 0's ACT-bound window is filled
    with head 1's q/k projection and the tail v chunks; head 1's windows
    are filled with the output-projection pieces for the q-tile that just
    finished.
  - output projection pieces acc[oc(128), st(512)] += wo_h[:, oc]^T @ oT_h
    accumulated over the 2 local heads, copied to SBUF bf16 (ACT/DVE
    alternating) and DMA'd per piece. Host sums partials and transposes.
"""

import os
import sys

for _p in ("/root/.axon_site/_ro/trn_rl_repo", "/opt/trn_rl_repo"):
    if os.path.isdir(_p) and _p not in sys.path:
        sys.path.append(_p)

import numpy as np
import ml_dtypes
from contextlib import ExitStack

import concourse.bass as bass
import concourse.bacc as bacc
import concourse.mybir as mybir
import concourse.tile as tile
from concourse.bass_utils import run_bass_kernel_spmd

BF16 = mybir.dt.bfloat16
F32 = mybir.dt.float32
NPBF16 = ml_dtypes.bfloat16

N_CORES = 8
S = 2048          # sequence length
DM = 2048         # model dim
H = 16            # heads
HD = 128          # head dim
HPC = H // N_CORES  # heads per core = 2
CW = HPC * HD     # per-core projection width = 256
P = 128
HW = HD // 2      # 64
QT = 512          # query tile (free dim of attention matmuls)
NQT = S // QT     # 4 query tiles per head
NSC = S // P      # 16 sequence chunks
NKC = DM // P     # 16 contraction chunks
NST = S // QT     # 4 s-tiles
SOFTCAP = 50.0
C1 = 1.0 / (SOFTCAP * np.sqrt(HD))

Tanh = mybir.ActivationFunctionType.Tanh
Exp = mybir.ActivationFunctionType.Exp


def build_nc(reps=1, single=False):
    nc = bacc.Bacc("TRN2", target_bir_lowering=False, num_devices=N_CORES)

    xT_d = nc.dram_tensor("xT", [DM, S], BF16, kind="ExternalInput")
    w_d = nc.dram_tensor("w_all", [DM, 3 * CW], BF16, kind="ExternalInput")
    wo_d = nc.dram_tensor("wo_c", [CW, DM], BF16, kind="ExternalInput")
    cos_d = nc.dram_tensor("cosT2", [P, S], BF16, kind="ExternalInput")
    sin_d = nc.dram_tensor("sinT2", [P, S], BF16, kind="ExternalInput")
    mask_d = nc.dram_tensor("mask", [P, 4 * QT], BF16, kind="ExternalInput")
    out_d = nc.dram_tensor("outT", [DM, S], BF16, kind="ExternalOutput")

    with tile.TileContext(nc) as tc:
        for _rep in range(reps):
            _emit_body(nc, tc, xT_d, w_d, wo_d, cos_d, sin_d, mask_d, out_d)
    nc.compile()
    return nc


def _emit_body(nc, tc, xT_d, w_d, wo_d, cos_d, sin_d, mask_d, out_d):
    with ExitStack() as ctx:
        # ---------- persistent SBUF ----------
        persist = ctx.enter_context(tc.tile_pool(name="persist", bufs=1))
        qT = [persist.tile([P, S], BF16, name=f"qT{h}") for h in range(HPC)]
        kT = [persist.tile([P, S], BF16, name=f"kT{h}") for h in range(HPC)]
        v_sb = [persist.tile([P, S], BF16, name=f"v{h}") for h in range(HPC)]
        oT = [persist.tile([P, S], BF16, name=f"oT{h}") for h in range(HPC)]
        mask_sb = persist.tile([P, 4 * QT], BF16, name="mask")
        ones_bf = persist.tile([P, 1], BF16, name="ones")
        cos_sb = persist.tile([P, S], BF16, name="cosT2")
        sin_sb = persist.tile([P, S], BF16, name="sinT2")
        wo_sb = [persist.tile([P, DM], BF16, name=f"wo{h}") for h in range(HPC)]
        xp = ctx.enter_context(tc.tile_pool(name="xT", bufs=1))
        wp = ctx.enter_context(tc.tile_pool(name="w", bufs=1))
        rp = ctx.enter_context(tc.tile_pool(name="rope", bufs=8))

        # DMA priority order (HWDGE + the transfer engines serialize, so
        # issue order IS arrival order): rope tables, then the pre-phase
        # wavefront (w cols [q0|k0|v] + x first halves, k-interleaved),
        # then x second halves, mask, w cols [q1|k1], wo. w_all columns
        # are host-reordered to [q0, k0, v, q1, k1] to enable the split.
        # Batched loads: HWDGE issue bandwidth (~0.63us per DMA) is the
        # startup bottleneck, so w/x load as 4-k-group DMAs via 3-D tiles
        # and partition-inner DRAM views; x additionally splits into
        # column halves so the first s-tiles unblock early.
        wt_all = wp.tile([P, NKC, 3 * CW], BF16, name="wt")
        xt_all = xp.tile([P, NKC, S], BF16, name="xt")
        w_r = w_d.rearrange("(k p) c -> p k c", p=P)
        x_r = xT_d.rearrange("(k p) c -> p k c", p=P)
        HS = S // 2
        WA = 2 * P + CW  # wavefront w cols: q0 | k0 | v
        # first k-group split in two so the very first matmuls start ~3us
        nc.sync.dma_start(out=wt_all[:, 0:2, 0:WA], in_=w_r[:, 0:2, 0:WA])
        nc.scalar.dma_start(out=xt_all[:, 0:2, 0:HS], in_=x_r[:, 0:2, 0:HS])
        nc.sync.dma_start(out=wt_all[:, 2:4, 0:WA], in_=w_r[:, 2:4, 0:WA])
        nc.scalar.dma_start(out=xt_all[:, 2:4, 0:HS], in_=x_r[:, 2:4, 0:HS])
        # rope tables: needed ~10us in, after the first k-group
        nc.scalar.dma_start(out=cos_sb[:], in_=cos_d[:])
        nc.scalar.dma_start(out=sin_sb[:], in_=sin_d[:])
        for g in range(4, NKC, 4):
            nc.sync.dma_start(
                out=wt_all[:, g:g + 4, 0:WA], in_=w_r[:, g:g + 4, 0:WA])
            nc.scalar.dma_start(
                out=xt_all[:, g:g + 4, 0:HS], in_=x_r[:, g:g + 4, 0:HS])
        for g in range(0, NKC, 4):
            (nc.sync if g % 8 == 0 else nc.scalar).dma_start(
                out=xt_all[:, g:g + 4, HS:S], in_=x_r[:, g:g + 4, HS:S])
        for g in range(0, NKC, 8):  # q1|k1 w cols, first needed mid-B0
            nc.sync.dma_start(
                out=wt_all[:, g:g + 8, WA:3 * CW],
                in_=w_r[:, g:g + 8, WA:3 * CW])
        nc.sync.dma_start(out=mask_sb[:], in_=mask_d[:])
        nc.vector.memset(ones_bf[:], 1.0)
        for h in range(HPC):
            nc.sync.dma_start(out=wo_sb[h][:], in_=wo_d[h * P:(h + 1) * P, :])

        # w_all column offsets after host reorder [q0, k0, v, q1, k1]
        W_OFF = {0: 0, 2: P, 1: 2 * P + CW, 3: 3 * P + CW}
        V_OFF = 2 * P

        def qk_chunks(pool, c, st):
            """q/k feature chunk c (0: q-h0, 1: q-h1, 2: k-h0, 3: k-h1),
            s-tile st, transposed layout + fused rope, as 4 PE micro-steps."""
            dst = (qT, kT)[c // HPC][c % HPC]
            wo_ = W_OFF[c]
            state = {}

            def mm(k0):
                def f():
                    if k0 == 0:
                        state["ps"] = pool.tile([P, QT], F32, name="f")
                    ps = state["ps"]
                    for k in range(k0, k0 + 4):
                        nc.tensor.matmul(
                            ps[:],
                            wt_all[:, k, wo_:wo_ + P],
                            xt_all[:, k, st * QT:(st + 1) * QT],
                            start=(k == 0), stop=(k == NKC - 1),
                        )
                    if k0 == NKC - 4:
                        ps = state["ps"]
                        cs = slice(st * QT, (st + 1) * QT)
                        # Pool cannot read PSUM: both rope multiplies run
                        # on DVE; the all-SBUF add goes to Pool.
                        t1 = rp.tile([P, QT], BF16, name="t1")
                        nc.vector.tensor_mul(t1[:], ps[:], cos_sb[:, cs])
                        t2 = rp.tile([P, QT], BF16, name="t2")
                        nc.vector.tensor_mul(
                            t2[0:HW, :], ps[HW:HD, :], sin_sb[0:HW, cs])
                        nc.vector.tensor_mul(
                            t2[HW:HD, :], ps[0:HW, :], sin_sb[HW:HD, cs])
                        nc.gpsimd.tensor_add(dst[:, cs], t1[:], t2[:])
                return f
            return [mm(k0) for k0 in range(0, NKC, 4)]

        def v_chunks(pool, sc):
            """v s-chunk sc in natural layout, as 2 PE micro-steps."""
            state = {}

            def mm(k0):
                def f():
                    if k0 == 0:
                        state["ps"] = pool.tile([P, QT], F32, name="f")
                    ps = state["ps"]
                    for k in range(k0, k0 + 8):
                        nc.tensor.matmul(
                            ps[:, 0:CW],
                            xt_all[:, k, sc * P:(sc + 1) * P],
                            wt_all[:, k, V_OFF:V_OFF + CW],
                            start=(k == 0), stop=(k == NKC - 1),
                        )
                    if k0 == NKC - 8:
                        for h in range(HPC):
                            nc.vector.tensor_copy(
                                v_sb[h][:, sc * P:(sc + 1) * P],
                                ps[:, h * HD:(h + 1) * HD],
                            )
                return f
            return [mm(0), mm(8)]

        o_r = out_d.rearrange("(o p) s -> p o s", p=P)

        def c_chunks(st, outp, c_ps, alt=False):
            """output-projection pieces for s-tile st, 1 PE micro-step each;
            results stage into 4-oc-wide tiles DMA'd as one transfer."""
            state = {}

            def piece(oc):
                def f():
                    acc = c_ps.tile([P, QT], F32, name="f")
                    for h in range(HPC):
                        nc.tensor.matmul(
                            acc[:],
                            wo_sb[h][:, oc * P:(oc + 1) * P],
                            oT[h][:, st * QT:(st + 1) * QT],
                            start=(h == 0), stop=(h == HPC - 1),
                        )
                    if oc % 4 == 0:
                        state["osb"] = outp.tile([P, 4, QT], BF16, name="osb")
                    osb = state["osb"]
                    # ACT carries the B1 tanh/exp chain: only 1 in 4 copies
                    # goes there — except in the drain round (alt), where
                    # ACT is free and copies alternate 50/50
                    if (oc % 2 == 0) if alt else (oc % 4 == 0):
                        nc.scalar.copy(osb[:, oc % 4, :], acc[:])
                    else:
                        nc.vector.tensor_copy(osb[:, oc % 4, :], acc[:])
                    if alt and oc % 2 == 1:
                        # drain round: ship half-tiles so the final
                        # transfer after the last copy is shorter
                        nc.sync.dma_start(
                            out=o_r[:, oc - 1:oc + 1,
                                    st * QT:(st + 1) * QT],
                            in_=osb[:, (oc % 4) - 1:(oc % 4) + 1, :])
                    elif not alt and oc % 4 == 3:
                        nc.sync.dma_start(
                            out=o_r[:, oc - 3:oc + 1,
                                    st * QT:(st + 1) * QT],
                            in_=osb[:])
                return f
            return [piece(oc) for oc in range(NKC)]

        class Feeder:
            """Doles out independent PE micro-steps to hide ACT latency."""
            def __init__(self):
                self.chunks = []

            def add(self, chunks):
                self.chunks.extend(chunks)

            def step(self, n):
                for _ in range(n):
                    if self.chunks:
                        self.chunks.pop(0)()

            def drain(self):
                self.step(len(self.chunks))

        def emit_attn(h, t, pools, feeder, per_pair):
            s_ps, o_ps, l_ps, thp, pp, np_ = pools
            o_acc = o_ps.tile([P, QT], F32, name="o_acc")
            l_acc = l_ps.tile([1, QT], F32, name="l_acc")
            npair = 2 * t + 2
            q_ap = qT[h][:, t * QT:(t + 1) * QT]

            def emit_pv(pT, p, last):
                for i in range(2):
                    kc = 2 * p + i
                    nc.tensor.matmul(
                        o_acc[:],
                        v_sb[h][:, kc * P:(kc + 1) * P],
                        pT[:, i * QT:(i + 1) * QT],
                        start=(kc == 0), stop=(last and i == 1),
                    )
                    nc.tensor.matmul(
                        l_acc[:], ones_bf[:, 0:1],
                        pT[:, i * QT:(i + 1) * QT],
                        start=(kc == 0), stop=(last and i == 1),
                    )

            prev = None
            for p in range(npair):
                sp = s_ps.tile([P, 2 * QT], F32, name="sp")
                for i in range(2):
                    kc = 2 * p + i
                    nc.tensor.matmul(
                        sp[:, i * QT:(i + 1) * QT],
                        kT[h][:, kc * P:(kc + 1) * P], q_ap,
                        start=True, stop=True,
                    )
                feeder.step(per_pair)
                th = thp.tile([P, 2 * QT], F32, name="th")
                nc.scalar.activation(th[:], sp[:], Tanh, scale=C1)
                pT = pp.tile([P, 2 * QT], BF16, name="pTt")
                nc.scalar.activation(pT[:], th[:], Exp, scale=SOFTCAP)
                # masked pairs are the last two: p==2t (u=0,1), p==2t+1 (u=2,3)
                u0 = 2 * (p - 2 * t)
                if u0 >= 0:
                    nc.vector.tensor_mul(
                        pT[:], pT[:], mask_sb[:, u0 * QT:(u0 + 2) * QT])
                if prev is not None:
                    emit_pv(prev[0], prev[1], last=False)
                prev = (pT, p)
            emit_pv(prev[0], prev[1], last=True)
            recip = np_.tile([1, QT], F32, name="recip")
            nc.vector.reciprocal(recip[:], l_acc[:])
            bcast = np_.tile([P, QT], F32, name="bcast")
            nc.gpsimd.partition_broadcast(bcast[:], recip[:])
            nc.vector.tensor_mul(
                oT[h][:, t * QT:(t + 1) * QT], o_acc[:], bcast[:])

        # ---------- phase A (pre-attention part) ----------
        # head 0's q/k + the first 4 v chunks. Tiles needing only the x
        # first halves come first, k-interleaved within 3-tile windows so
        # the PE tracks the DMA wavefront instead of stalling on one tile.
        def interleave(units):
            out = []
            for step in range(max(len(u) for u in units)):
                for u in units:
                    if step < len(u):
                        out.append(u[step])
            return out

        # The A phase is DMA-bound (~35us of input wavefront), so all v
        # chunks ride along in its PE bubbles, ordered by which x quarter
        # they need.
        with ExitStack() as ctxA:
            qkA = ctxA.enter_context(
                tc.tile_pool(name="qkA", bufs=6, space="PSUM"))
            pre = Feeder()
            pre.add(interleave([qk_chunks(qkA, 0, 0), qk_chunks(qkA, 2, 0)]))
            pre.add(interleave([qk_chunks(qkA, 0, 1), qk_chunks(qkA, 2, 1)]))
            for sc in range(0, 8):
                pre.add(v_chunks(qkA, sc))
            pre.add(interleave([qk_chunks(qkA, 0, 2), qk_chunks(qkA, 2, 2)]))
            for sc in range(8, 12):
                pre.add(v_chunks(qkA, sc))
            pre.add(interleave([qk_chunks(qkA, 0, 3), qk_chunks(qkA, 2, 3)]))
            for sc in range(12, 16):
                pre.add(v_chunks(qkA, sc))
            pre.drain()

        # ---------- phase B0: head-0 attention + A-fill ----------
        # shared fill/output-projection PSUM pool (one tag, 2 banks);
        # created below the B pools so those can close before the drain
        fps = ctx.enter_context(tc.tile_pool(name="fps", bufs=2, space="PSUM"))
        outp = ctx.enter_context(tc.tile_pool(name="out", bufs=4))
        fill = Feeder()
        with ExitStack() as ctxB:
            s_ps = ctxB.enter_context(
                tc.tile_pool(name="s_ps", bufs=2, space="PSUM"))
            o_ps = ctxB.enter_context(
                tc.tile_pool(name="o_ps", bufs=1, space="PSUM"))
            l_ps = ctxB.enter_context(
                tc.tile_pool(name="l_ps", bufs=1, space="PSUM"))
            thp = ctxB.enter_context(tc.tile_pool(name="tanh", bufs=3))
            pp = ctxB.enter_context(tc.tile_pool(name="pT", bufs=3))
            np_ = ctxB.enter_context(tc.tile_pool(name="norm", bufs=4))
            bpools = (s_ps, o_ps, l_ps, thp, pp, np_)

            for st in (0, 1):
                fill.add(qk_chunks(fps, 1, st))
                fill.add(qk_chunks(fps, 3, st))
            for t in range(NQT):
                emit_attn(0, t, bpools, fill, per_pair=3)
            # q1/k1 st2/st3 are first needed by B1 t2/t3: defer them into
            # the otherwise-unfilled B1 t0/t1 windows.
            for st in (2, 3):
                fill.add(qk_chunks(fps, 1, st))
                fill.add(qk_chunks(fps, 3, st))

            # ---------- phase B1 + C: attention + output projection ----
            for t, per in zip(range(NQT), (6, 5, 3, 2)):
                emit_attn(1, t, bpools, fill, per_pair=per)
                if t < NQT - 1:
                    fill.add(c_chunks(t, outp, fps))
        # drain round: B pools are closed, give the last output-projection
        # round a wide PSUM pool so its pieces pipeline
        cD = ctx.enter_context(tc.tile_pool(name="cD", bufs=5, space="PSUM"))
        fill.add(c_chunks(NQT - 1, outp, cD, alt=True))
        fill.drain()


_NC_CACHE = None


def _get_nc():
    global _NC_CACHE
    if _NC_CACHE is None:
        _NC_CACHE = build_nc()
    return _NC_CACHE


def _rope_perm():
    """per-head column permutation de-interleaving rotary pairs"""
    perm = np.zeros(DM, np.int64)
    for h in range(H):
        base = h * HD
        perm[base:base + HD // 2] = base + np.arange(0, HD, 2)
        perm[base + HD // 2:base + HD] = base + np.arange(1, HD, 2)
    return perm


def make_in_maps(x, wq, wk, wv, wo, freqs_cos, freqs_sin):
    x = np.asarray(x, np.float32).reshape(S, DM)
    wq = np.asarray(wq, np.float32)
    wk = np.asarray(wk, np.float32)
    wv = np.asarray(wv, np.float32)
    wo = np.asarray(wo, np.float32)
    xT = np.ascontiguousarray(x.T).astype(NPBF16)
    perm = _rope_perm()
    wq_p = wq[:, perm]
    wk_p = wk[:, perm]
    # transposed rope tables: C = [cosT; cosT], S' = [-sinT; sinT]
    cosT = np.asarray(freqs_cos, np.float32).T  # [64, S]
    sinT = np.asarray(freqs_sin, np.float32).T
    cosT2 = np.concatenate([cosT, cosT], axis=0).astype(NPBF16)
    sinT2 = np.concatenate([-sinT, sinT], axis=0).astype(NPBF16)
    # mask[i, u*QT + j] = 1 if i <= j - 128*u else 0  (keep kj <= qi)
    i_idx = np.arange(P)[:, None]
    j_idx = np.arange(QT)[None, :]
    mask = np.concatenate(
        [(i_idx <= j_idx - P * u) for u in range(4)], axis=1
    ).astype(NPBF16)
    in_maps = []
    for c in range(N_CORES):
        cs = slice(c * CW, (c + 1) * CW)
        h0 = slice(c * CW, c * CW + HD)
        h1 = slice(c * CW + HD, (c + 1) * CW)
        # device column order: [q-h0, k-h0, v, q-h1, k-h1]
        w_all = np.concatenate(
            [wq_p[:, h0], wk_p[:, h0], wv[:, cs],
             wq_p[:, h1], wk_p[:, h1]], axis=1).astype(NPBF16)
        wo_c = np.ascontiguousarray(wo[cs, :]).astype(NPBF16)
        in_maps.append({
            "xT": xT,
            "w_all": np.ascontiguousarray(w_all),
            "wo_c": wo_c,
            "cosT2": cosT2,
            "sinT2": sinT2,
            "mask": mask,
        })
    return in_maps


def assemble_output(results):
    acc = results[0]["outT"].astype(np.float32)
    for r in results[1:]:
        acc += np.asarray(r["outT"]).astype(np.float32)
    return np.ascontiguousarray(acc.T).reshape(1, S, DM).astype(np.float32)


def kernel(x, wq, wk, wv, wo, freqs_cos, freqs_sin):
    nc = _get_nc()
    in_maps = make_in_maps(x, wq, wk, wv, wo, freqs_cos, freqs_sin)
    res = run_bass_kernel_spmd(nc, in_maps, core_ids=list(range(N_CORES)))
    return assemble_output(res.results)


if __name__ == "__main__":
    rng = np.random.default_rng(0)
    ins = {
        "x": rng.standard_normal((1, S, DM), np.float32),
        "wq": rng.standard_normal((DM, DM), np.float32) / np.sqrt(DM),
        "wk": rng.standard_normal((DM, DM), np.float32) / np.sqrt(DM),
        "wv": rng.standard_normal((DM, DM), np.float32) / np.sqrt(DM),
        "wo": rng.standard_normal((DM, DM), np.float32) / np.sqrt(DM),
        "freqs_cos": rng.standard_normal((S, HD // 2), np.float32),
        "freqs_sin": rng.standard_normal((S, HD // 2), np.float32),
    }
    out = kernel(**ins)
    print("out", out.shape, out.dtype, np.abs(out).mean())


# revision 75
# speedup vs baseline: 1.0665x; 1.0205x over previous
"""Trainium2 Bass kernel for Llama-like attention (16 heads, tanh softcap, RoPE).

Sharding: tensor-parallel over heads, fully collective-free. Each of the 8
cores computes 2 heads end-to-end and a *partial* output projection
(o_local @ wo_rows_local)^T; the host sums the 8 partial outputs. With no
on-device collective, each core's NEFF span is pure local compute — no
cross-core rendezvous.

Per-core pipeline (engine-balanced against the ~165us PE floor):
  - q^T/k^T computed directly in transposed layout ([head_dim, s]) via
    matmul(w_slice^T, x^T): no PE transposes. Weight columns of wq/wk are
    pre-permuted on the host to de-interleave even/odd rotary pairs (the
    permutation cancels inside q.k).
  - RoPE in transposed layout straight out of PSUM: rot = A*C + swap(A)*S'
    with C = [cosT; cosT], S' = [-sinT; sinT]. The partition-half swap is
    two half-height Vector multiplies reading PSUM at a partition offset
    (Pool cannot touch PSUM); the all-SBUF add runs on Pool. ACT stays
    free for the softmax chain.
  - attention with scores transposed ([kj, qi]) so softmaxed probabilities
    feed the PV matmul directly as the moving operand. tanh softcap bounds
    scores, so softmax needs no row-max pass: p = exp(50*tanh(.)),
    l = ones-row matmul, o = p@v / l. Head 0's ACT-bound window is filled
    with head 1's q/k projection and the tail v chunks; head 1's windows
    are filled with the output-projection pieces for the q-tile that just
    finished.
  - output projection pieces acc[oc(128), st(512)] += wo_h[:, oc]^T @ oT_h
    accumulated over the 2 local heads, copied to SBUF bf16 (ACT/DVE
    alternating) and DMA'd per piece. Host sums partials and transposes.
"""

import os
import sys

for _p in ("/root/.axon_site/_ro/trn_rl_repo", "/opt/trn_rl_repo"):
    if os.path.isdir(_p) and _p not in sys.path:
        sys.path.append(_p)

import numpy as np
import ml_dtypes
from contextlib import ExitStack

import concourse.bass as bass
import concourse.bacc as bacc
import concourse.mybir as mybir
import concourse.tile as tile
from concourse.bass_utils import run_bass_kernel_spmd

BF16 = mybir.dt.bfloat16
F32 = mybir.dt.float32
NPBF16 = ml_dtypes.bfloat16

N_CORES = 8
S = 2048          # sequence length
DM = 2048         # model dim
H = 16            # heads
HD = 128          # head dim
HPC = H // N_CORES  # heads per core = 2
CW = HPC * HD     # per-core projection width = 256
P = 128
HW = HD // 2      # 64
QT = 512          # query tile (free dim of attention matmuls)
NQT = S // QT     # 4 query tiles per head
NSC = S // P      # 16 sequence chunks
NKC = DM // P     # 16 contraction chunks
NST = S // QT     # 4 s-tiles
SOFTCAP = 50.0
C1 = 1.0 / (SOFTCAP * np.sqrt(HD))

Tanh = mybir.ActivationFunctionType.Tanh
Exp = mybir.ActivationFunctionType.Exp


def build_nc(reps=1, single=False):
    nc = bacc.Bacc("TRN2", target_bir_lowering=False, num_devices=N_CORES)

    xT_d = nc.dram_tensor("xT", [DM, S], BF16, kind="ExternalInput")
    w_d = nc.dram_tensor("w_all", [DM, 3 * CW], BF16, kind="ExternalInput")
    wo_d = nc.dram_tensor("wo_c", [CW, DM], BF16, kind="ExternalInput")
    cos_d = nc.dram_tensor("cosT2", [P, S], BF16, kind="ExternalInput")
    sin_d = nc.dram_tensor("sinT2", [P, S], BF16, kind="ExternalInput")
    mask_d = nc.dram_tensor("mask", [P, 4 * QT], BF16, kind="ExternalInput")
    out_d = nc.dram_tensor("outT", [DM, S], BF16, kind="ExternalOutput")

    with tile.TileContext(nc) as tc:
        for _rep in range(reps):
            _emit_body(nc, tc, xT_d, w_d, wo_d, cos_d, sin_d, mask_d, out_d)
    nc.compile()
    return nc


def _emit_body(nc, tc, xT_d, w_d, wo_d, cos_d, sin_d, mask_d, out_d):
    with ExitStack() as ctx:
        # ---------- persistent SBUF ----------
        persist = ctx.enter_context(tc.tile_pool(name="persist", bufs=1))
        qT = [persist.tile([P, S], BF16, name=f"qT{h}") for h in range(HPC)]
        kT = [persist.tile([P, S], BF16, name=f"kT{h}") for h in range(HPC)]
        v_sb = [persist.tile([P, S], BF16, name=f"v{h}") for h in range(HPC)]
        oT = [persist.tile([P, S], BF16, name=f"oT{h}") for h in range(HPC)]
        mask_sb = persist.tile([P, 4 * QT], BF16, name="mask")
        ones_bf = persist.tile([P, 1], BF16, name="ones")
        cos_sb = persist.tile([P, S], BF16, name="cosT2")
        sin_sb = persist.tile([P, S], BF16, name="sinT2")
        wo_sb = [persist.tile([P, DM], BF16, name=f"wo{h}") for h in range(HPC)]
        xp = ctx.enter_context(tc.tile_pool(name="xT", bufs=1))
        wp = ctx.enter_context(tc.tile_pool(name="w", bufs=1))
        rp = ctx.enter_context(tc.tile_pool(name="rope", bufs=8))

        # DMA priority order (HWDGE + the transfer engines serialize, so
        # issue order IS arrival order): rope tables, then the pre-phase
        # wavefront (w cols [q0|k0|v] + x first halves, k-interleaved),
        # then x second halves, mask, w cols [q1|k1], wo. w_all columns
        # are host-reordered to [q0, k0, v, q1, k1] to enable the split.
        # Batched loads: HWDGE issue bandwidth (~0.63us per DMA) is the
        # startup bottleneck, so w/x load as 4-k-group DMAs via 3-D tiles
        # and partition-inner DRAM views; x additionally splits into
        # column halves so the first s-tiles unblock early.
        wt_all = wp.tile([P, NKC, 3 * CW], BF16, name="wt")
        xt_all = xp.tile([P, NKC, S], BF16, name="xt")
        w_r = w_d.rearrange("(k p) c -> p k c", p=P)
        x_r = xT_d.rearrange("(k p) c -> p k c", p=P)
        HS = S // 2
        WA = 2 * P + CW  # wavefront w cols: q0 | k0 | v
        # first k-group split in two so the very first matmuls start ~3us;
        # all four head-of-line issues go via SP — the scalar queue is
        # blocked ~1.3us at t=0 by the activation-table load
        nc.sync.dma_start(out=wt_all[:, 0:2, 0:WA], in_=w_r[:, 0:2, 0:WA])
        nc.sync.dma_start(out=xt_all[:, 0:2, 0:HS], in_=x_r[:, 0:2, 0:HS])
        nc.sync.dma_start(out=wt_all[:, 2:4, 0:WA], in_=w_r[:, 2:4, 0:WA])
        nc.sync.dma_start(out=xt_all[:, 2:4, 0:HS], in_=x_r[:, 2:4, 0:HS])
        # rope tables: needed ~10us in, after the first k-group
        nc.scalar.dma_start(out=cos_sb[:], in_=cos_d[:])
        nc.scalar.dma_start(out=sin_sb[:], in_=sin_d[:])
        for g in range(4, NKC, 4):
            nc.sync.dma_start(
                out=wt_all[:, g:g + 4, 0:WA], in_=w_r[:, g:g + 4, 0:WA])
            nc.scalar.dma_start(
                out=xt_all[:, g:g + 4, 0:HS], in_=x_r[:, g:g + 4, 0:HS])
        for g in range(0, NKC, 4):
            (nc.sync if g % 8 == 0 else nc.scalar).dma_start(
                out=xt_all[:, g:g + 4, HS:S], in_=x_r[:, g:g + 4, HS:S])
        for g in range(0, NKC, 8):  # q1|k1 w cols, first needed mid-B0
            nc.sync.dma_start(
                out=wt_all[:, g:g + 8, WA:3 * CW],
                in_=w_r[:, g:g + 8, WA:3 * CW])
        nc.sync.dma_start(out=mask_sb[:], in_=mask_d[:])
        nc.vector.memset(ones_bf[:], 1.0)
        for h in range(HPC):
            nc.sync.dma_start(out=wo_sb[h][:], in_=wo_d[h * P:(h + 1) * P, :])

        # w_all column offsets after host reorder [q0, k0, v, q1, k1]
        W_OFF = {0: 0, 2: P, 1: 2 * P + CW, 3: 3 * P + CW}
        V_OFF = 2 * P

        def qk_chunks(pool, c, st):
            """q/k feature chunk c (0: q-h0, 1: q-h1, 2: k-h0, 3: k-h1),
            s-tile st, transposed layout + fused rope, as 4 PE micro-steps."""
            dst = (qT, kT)[c // HPC][c % HPC]
            wo_ = W_OFF[c]
            state = {}

            def mm(k0):
                def f():
                    if k0 == 0:
                        state["ps"] = pool.tile([P, QT], F32, name="f")
                    ps = state["ps"]
                    for k in range(k0, k0 + 4):
                        nc.tensor.matmul(
                            ps[:],
                            wt_all[:, k, wo_:wo_ + P],
                            xt_all[:, k, st * QT:(st + 1) * QT],
                            start=(k == 0), stop=(k == NKC - 1),
                        )
                    if k0 == NKC - 4:
                        ps = state["ps"]
                        cs = slice(st * QT, (st + 1) * QT)
                        # Pool cannot read PSUM: both rope multiplies run
                        # on DVE; the all-SBUF add goes to Pool.
                        t1 = rp.tile([P, QT], BF16, name="t1")
                        nc.vector.tensor_mul(t1[:], ps[:], cos_sb[:, cs])
                        t2 = rp.tile([P, QT], BF16, name="t2")
                        nc.vector.tensor_mul(
                            t2[0:HW, :], ps[HW:HD, :], sin_sb[0:HW, cs])
                        nc.vector.tensor_mul(
                            t2[HW:HD, :], ps[0:HW, :], sin_sb[HW:HD, cs])
                        nc.gpsimd.tensor_add(dst[:, cs], t1[:], t2[:])
                return f
            return [mm(k0) for k0 in range(0, NKC, 4)]

        def v_chunks(pool, sc):
            """v s-chunk sc in natural layout, as 2 PE micro-steps."""
            state = {}

            def mm(k0):
                def f():
                    if k0 == 0:
                        state["ps"] = pool.tile([P, QT], F32, name="f")
                    ps = state["ps"]
                    for k in range(k0, k0 + 8):
                        nc.tensor.matmul(
                            ps[:, 0:CW],
                            xt_all[:, k, sc * P:(sc + 1) * P],
                            wt_all[:, k, V_OFF:V_OFF + CW],
                            start=(k == 0), stop=(k == NKC - 1),
                        )
                    if k0 == NKC - 8:
                        for h in range(HPC):
                            nc.vector.tensor_copy(
                                v_sb[h][:, sc * P:(sc + 1) * P],
                                ps[:, h * HD:(h + 1) * HD],
                            )
                return f
            return [mm(0), mm(8)]

        o_r = out_d.rearrange("(o p) s -> p o s", p=P)

        def c_chunks(st, outp, c_ps, alt=False):
            """output-projection pieces for s-tile st, 1 PE micro-step each;
            results stage into 4-oc-wide tiles DMA'd as one transfer."""
            state = {}

            def piece(oc):
                def f():
                    acc = c_ps.tile([P, QT], F32, name="f")
                    for h in range(HPC):
                        nc.tensor.matmul(
                            acc[:],
                            wo_sb[h][:, oc * P:(oc + 1) * P],
                            oT[h][:, st * QT:(st + 1) * QT],
                            start=(h == 0), stop=(h == HPC - 1),
                        )
                    if oc % 4 == 0:
                        state["osb"] = outp.tile([P, 4, QT], BF16, name="osb")
                    osb = state["osb"]
                    # ACT carries the B1 tanh/exp chain: only 1 in 4 copies
                    # goes there — except in the drain round (alt), where
                    # ACT is free and copies alternate 50/50
                    if (oc % 2 == 0) if alt else (oc % 4 == 0):
                        nc.scalar.copy(osb[:, oc % 4, :], acc[:])
                    else:
                        nc.vector.tensor_copy(osb[:, oc % 4, :], acc[:])
                    if alt and oc % 2 == 1:
                        # drain round: ship half-tiles so the final
                        # transfer after the last copy is shorter
                        nc.sync.dma_start(
                            out=o_r[:, oc - 1:oc + 1,
                                    st * QT:(st + 1) * QT],
                            in_=osb[:, (oc % 4) - 1:(oc % 4) + 1, :])
                    elif not alt and oc % 4 == 3:
                        nc.sync.dma_start(
                            out=o_r[:, oc - 3:oc + 1,
                                    st * QT:(st + 1) * QT],
                            in_=osb[:])
                return f
            return [piece(oc) for oc in range(NKC)]

        class Feeder:
            """Doles out independent PE micro-steps to hide ACT latency."""
            def __init__(self):
                self.chunks = []

            def add(self, chunks):
                self.chunks.extend(chunks)

            def step(self, n):
                for _ in range(n):
                    if self.chunks:
                        self.chunks.pop(0)()

            def drain(self):
                self.step(len(self.chunks))

        def emit_attn(h, t, pools, feeder, per_pair):
            s_ps, o_ps, l_ps, thp, pp, np_ = pools
            o_acc = o_ps.tile([P, QT], F32, name="o_acc")
            l_acc = l_ps.tile([1, QT], F32, name="l_acc")
            npair = 2 * t + 2
            q_ap = qT[h][:, t * QT:(t + 1) * QT]

            def emit_pv(pT, p, last):
                for i in range(2):
                    kc = 2 * p + i
                    nc.tensor.matmul(
                        o_acc[:],
                        v_sb[h][:, kc * P:(kc + 1) * P],
                        pT[:, i * QT:(i + 1) * QT],
                        start=(kc == 0), stop=(last and i == 1),
                    )
                    nc.tensor.matmul(
                        l_acc[:], ones_bf[:, 0:1],
                        pT[:, i * QT:(i + 1) * QT],
                        start=(kc == 0), stop=(last and i == 1),
                    )

            prev = None
            for p in range(npair):
                sp = s_ps.tile([P, 2 * QT], F32, name="sp")
                for i in range(2):
                    kc = 2 * p + i
                    nc.tensor.matmul(
                        sp[:, i * QT:(i + 1) * QT],
                        kT[h][:, kc * P:(kc + 1) * P], q_ap,
                        start=True, stop=True,
                    )
                feeder.step(per_pair)
                th = thp.tile([P, 2 * QT], F32, name="th")
                nc.scalar.activation(th[:], sp[:], Tanh, scale=C1)
                pT = pp.tile([P, 2 * QT], BF16, name="pTt")
                nc.scalar.activation(pT[:], th[:], Exp, scale=SOFTCAP)
                # masked pairs are the last two: p==2t (u=0,1), p==2t+1 (u=2,3)
                u0 = 2 * (p - 2 * t)
                if u0 >= 0:
                    nc.vector.tensor_mul(
                        pT[:], pT[:], mask_sb[:, u0 * QT:(u0 + 2) * QT])
                if prev is not None:
                    emit_pv(prev[0], prev[1], last=False)
                prev = (pT, p)
            emit_pv(prev[0], prev[1], last=True)
            recip = np_.tile([1, QT], F32, name="recip")
            nc.vector.reciprocal(recip[:], l_acc[:])
            bcast = np_.tile([P, QT], F32, name="bcast")
            nc.gpsimd.partition_broadcast(bcast[:], recip[:])
            nc.vector.tensor_mul(
                oT[h][:, t * QT:(t + 1) * QT], o_acc[:], bcast[:])

        # ---------- phase A (pre-attention part) ----------
        # head 0's q/k + the first 4 v chunks. Tiles needing only the x
        # first halves come first, k-interleaved within 3-tile windows so
        # the PE tracks the DMA wavefront instead of stalling on one tile.
        def interleave(units):
            out = []
            for step in range(max(len(u) for u in units)):
                for u in units:
                    if step < len(u):
                        out.append(u[step])
            return out

        # The A phase is DMA-bound (~35us of input wavefront), so all v
        # chunks ride along in its PE bubbles, ordered by which x quarter
        # they need.
        with ExitStack() as ctxA:
            qkA = ctxA.enter_context(
                tc.tile_pool(name="qkA", bufs=6, space="PSUM"))
            pre = Feeder()
            pre.add(interleave([qk_chunks(qkA, 0, 0), qk_chunks(qkA, 2, 0)]))
            pre.add(interleave([qk_chunks(qkA, 0, 1), qk_chunks(qkA, 2, 1)]))
            for sc in range(0, 8):
                pre.add(v_chunks(qkA, sc))
            pre.add(interleave([qk_chunks(qkA, 0, 2), qk_chunks(qkA, 2, 2)]))
            for sc in range(8, 12):
                pre.add(v_chunks(qkA, sc))
            pre.add(interleave([qk_chunks(qkA, 0, 3), qk_chunks(qkA, 2, 3)]))
            for sc in range(12, 16):
                pre.add(v_chunks(qkA, sc))
            pre.drain()

        # ---------- phase B0: head-0 attention + A-fill ----------
        # shared fill/output-projection PSUM pool (one tag, 2 banks);
        # created below the B pools so those can close before the drain
        fps = ctx.enter_context(tc.tile_pool(name="fps", bufs=2, space="PSUM"))
        outp = ctx.enter_context(tc.tile_pool(name="out", bufs=4))
        fill = Feeder()
        with ExitStack() as ctxB:
            s_ps = ctxB.enter_context(
                tc.tile_pool(name="s_ps", bufs=2, space="PSUM"))
            o_ps = ctxB.enter_context(
                tc.tile_pool(name="o_ps", bufs=1, space="PSUM"))
            l_ps = ctxB.enter_context(
                tc.tile_pool(name="l_ps", bufs=1, space="PSUM"))
            thp = ctxB.enter_context(tc.tile_pool(name="tanh", bufs=3))
            pp = ctxB.enter_context(tc.tile_pool(name="pT", bufs=3))
            np_ = ctxB.enter_context(tc.tile_pool(name="norm", bufs=4))
            bpools = (s_ps, o_ps, l_ps, thp, pp, np_)

            for st in (0, 1):
                fill.add(qk_chunks(fps, 1, st))
                fill.add(qk_chunks(fps, 3, st))
            for t in range(NQT):
                emit_attn(0, t, bpools, fill, per_pair=3)
            # q1/k1 st2/st3 are first needed by B1 t2/t3: defer them into
            # the otherwise-unfilled B1 t0/t1 windows.
            for st in (2, 3):
                fill.add(qk_chunks(fps, 1, st))
                fill.add(qk_chunks(fps, 3, st))

            # ---------- phase B1 + C: attention + output projection ----
            for t, per in zip(range(NQT), (6, 5, 3, 2)):
                emit_attn(1, t, bpools, fill, per_pair=per)
                if t < NQT - 1:
                    fill.add(c_chunks(t, outp, fps))
        # drain round: B pools are closed, give the last output-projection
        # round a wide PSUM pool so its pieces pipeline
        cD = ctx.enter_context(tc.tile_pool(name="cD", bufs=5, space="PSUM"))
        fill.add(c_chunks(NQT - 1, outp, cD, alt=True))
        fill.drain()


_NC_CACHE = None


def _get_nc():
    global _NC_CACHE
    if _NC_CACHE is None:
        _NC_CACHE = build_nc()
    return _NC_CACHE


def _rope_perm():
    """per-head column permutation de-interleaving rotary pairs"""
    perm = np.zeros(DM, np.int64)
    for h in range(H):
        base = h * HD
        perm[base:base + HD // 2] = base + np.arange(0, HD, 2)
        perm[base + HD // 2:base + HD] = base + np.arange(1, HD, 2)
    return perm


def make_in_maps(x, wq, wk, wv, wo, freqs_cos, freqs_sin):
    x = np.asarray(x, np.float32).reshape(S, DM)
    wq = np.asarray(wq, np.float32)
    wk = np.asarray(wk, np.float32)
    wv = np.asarray(wv, np.float32)
    wo = np.asarray(wo, np.float32)
    xT = np.ascontiguousarray(x.T).astype(NPBF16)
    perm = _rope_perm()
    wq_p = wq[:, perm]
    wk_p = wk[:, perm]
    # transposed rope tables: C = [cosT; cosT], S' = [-sinT; sinT]
    cosT = np.asarray(freqs_cos, np.float32).T  # [64, S]
    sinT = np.asarray(freqs_sin, np.float32).T
    cosT2 = np.concatenate([cosT, cosT], axis=0).astype(NPBF16)
    sinT2 = np.concatenate([-sinT, sinT], axis=0).astype(NPBF16)
    # mask[i, u*QT + j] = 1 if i <= j - 128*u else 0  (keep kj <= qi)
    i_idx = np.arange(P)[:, None]
    j_idx = np.arange(QT)[None, :]
    mask = np.concatenate(
        [(i_idx <= j_idx - P * u) for u in range(4)], axis=1
    ).astype(NPBF16)
    in_maps = []
    for c in range(N_CORES):
        cs = slice(c * CW, (c + 1) * CW)
        h0 = slice(c * CW, c * CW + HD)
        h1 = slice(c * CW + HD, (c + 1) * CW)
        # device column order: [q-h0, k-h0, v, q-h1, k-h1]
        w_all = np.concatenate(
            [wq_p[:, h0], wk_p[:, h0], wv[:, cs],
             wq_p[:, h1], wk_p[:, h1]], axis=1).astype(NPBF16)
        wo_c = np.ascontiguousarray(wo[cs, :]).astype(NPBF16)
        in_maps.append({
            "xT": xT,
            "w_all": np.ascontiguousarray(w_all),
            "wo_c": wo_c,
            "cosT2": cosT2,
            "sinT2": sinT2,
            "mask": mask,
        })
    return in_maps


def assemble_output(results):
    acc = results[0]["outT"].astype(np.float32)
    for r in results[1:]:
        acc += np.asarray(r["outT"]).astype(np.float32)
    return np.ascontiguousarray(acc.T).reshape(1, S, DM).astype(np.float32)


def kernel(x, wq, wk, wv, wo, freqs_cos, freqs_sin):
    nc = _get_nc()
    in_maps = make_in_maps(x, wq, wk, wv, wo, freqs_cos, freqs_sin)
    res = run_bass_kernel_spmd(nc, in_maps, core_ids=list(range(N_CORES)))
    return assemble_output(res.results)


if __name__ == "__main__":
    rng = np.random.default_rng(0)
    ins = {
        "x": rng.standard_normal((1, S, DM), np.float32),
        "wq": rng.standard_normal((DM, DM), np.float32) / np.sqrt(DM),
        "wk": rng.standard_normal((DM, DM), np.float32) / np.sqrt(DM),
        "wv": rng.standard_normal((DM, DM), np.float32) / np.sqrt(DM),
        "wo": rng.standard_normal((DM, DM), np.float32) / np.sqrt(DM),
        "freqs_cos": rng.standard_normal((S, HD // 2), np.float32),
        "freqs_sin": rng.standard_normal((S, HD // 2), np.float32),
    }
    out = kernel(**ins)
    print("out", out.shape, out.dtype, np.abs(out).mean())


# revision 76
# speedup vs baseline: 1.0776x; 1.0105x over previous
"""Trainium2 Bass kernel for Llama-like attention (16 heads, tanh softcap, RoPE).

Sharding: tensor-parallel over heads, fully collective-free. Each of the 8
cores computes 2 heads end-to-end and a *partial* output projection
(o_local @ wo_rows_local)^T; the host sums the 8 partial outputs. With no
on-device collective, each core's NEFF span is pure local compute — no
cross-core rendezvous.

Per-core pipeline (engine-balanced against the ~165us PE floor):
  - q^T/k^T computed directly in transposed layout ([head_dim, s]) via
    matmul(w_slice^T, x^T): no PE transposes. Weight columns of wq/wk are
    pre-permuted on the host to de-interleave even/odd rotary pairs (the
    permutation cancels inside q.k).
  - RoPE in transposed layout straight out of PSUM: rot = A*C + swap(A)*S'
    with C = [cosT; cosT], S' = [-sinT; sinT]. The partition-half swap is
    two half-height Vector multiplies reading PSUM at a partition offset
    (Pool cannot touch PSUM); the all-SBUF add runs on Pool. ACT stays
    free for the softmax chain.
  - attention with scores transposed ([kj, qi]) so softmaxed probabilities
    feed the PV matmul directly as the moving operand. tanh softcap bounds
    scores, so softmax needs no row-max pass: p = exp(50*tanh(.)),
    l = ones-row matmul, o = p@v / l. Head 0's ACT-bound window is filled
    with head 1's q/k projection and the tail v chunks; head 1's windows
    are filled with the output-projection pieces for the q-tile that just
    finished.
  - output projection pieces acc[oc(128), st(512)] += wo_h[:, oc]^T @ oT_h
    accumulated over the 2 local heads, copied to SBUF bf16 (ACT/DVE
    alternating) and DMA'd per piece. Host sums partials and transposes.
"""

import os
import sys

for _p in ("/root/.axon_site/_ro/trn_rl_repo", "/opt/trn_rl_repo"):
    if os.path.isdir(_p) and _p not in sys.path:
        sys.path.append(_p)

import numpy as np
import ml_dtypes
from contextlib import ExitStack

import concourse.bass as bass
import concourse.bacc as bacc
import concourse.mybir as mybir
import concourse.tile as tile
from concourse.bass_utils import run_bass_kernel_spmd

BF16 = mybir.dt.bfloat16
F32 = mybir.dt.float32
NPBF16 = ml_dtypes.bfloat16

N_CORES = 8
S = 2048          # sequence length
DM = 2048         # model dim
H = 16            # heads
HD = 128          # head dim
HPC = H // N_CORES  # heads per core = 2
CW = HPC * HD     # per-core projection width = 256
P = 128
HW = HD // 2      # 64
QT = 512          # query tile (free dim of attention matmuls)
NQT = S // QT     # 4 query tiles per head
NSC = S // P      # 16 sequence chunks
NKC = DM // P     # 16 contraction chunks
NST = S // QT     # 4 s-tiles
SOFTCAP = 50.0
C1 = 1.0 / (SOFTCAP * np.sqrt(HD))

Tanh = mybir.ActivationFunctionType.Tanh
Exp = mybir.ActivationFunctionType.Exp


def build_nc(reps=1, single=False):
    nc = bacc.Bacc("TRN2", target_bir_lowering=False, num_devices=N_CORES)

    xT_d = nc.dram_tensor("xT", [DM, S], BF16, kind="ExternalInput")
    w_d = nc.dram_tensor("w_all", [DM, 3 * CW], BF16, kind="ExternalInput")
    wo_d = nc.dram_tensor("wo_c", [CW, DM], BF16, kind="ExternalInput")
    cos_d = nc.dram_tensor("cosT2", [P, S], BF16, kind="ExternalInput")
    sin_d = nc.dram_tensor("sinT2", [P, S], BF16, kind="ExternalInput")
    mask_d = nc.dram_tensor("mask", [P, 4 * QT], BF16, kind="ExternalInput")
    out_d = nc.dram_tensor("outT", [DM, S], BF16, kind="ExternalOutput")

    with tile.TileContext(nc) as tc:
        for _rep in range(reps):
            _emit_body(nc, tc, xT_d, w_d, wo_d, cos_d, sin_d, mask_d, out_d)
    nc.compile()
    return nc


def _emit_body(nc, tc, xT_d, w_d, wo_d, cos_d, sin_d, mask_d, out_d):
    with ExitStack() as ctx:
        # ---------- persistent SBUF ----------
        persist = ctx.enter_context(tc.tile_pool(name="persist", bufs=1))
        qT = [persist.tile([P, S], BF16, name=f"qT{h}") for h in range(HPC)]
        kT = [persist.tile([P, S], BF16, name=f"kT{h}") for h in range(HPC)]
        v_sb = [persist.tile([P, S], BF16, name=f"v{h}") for h in range(HPC)]
        oT = [persist.tile([P, S], BF16, name=f"oT{h}") for h in range(HPC)]
        mask_sb = persist.tile([P, 4 * QT], BF16, name="mask")
        ones_bf = persist.tile([P, 1], BF16, name="ones")
        cos_sb = persist.tile([P, S], BF16, name="cosT2")
        sin_sb = persist.tile([P, S], BF16, name="sinT2")
        wo_sb = [persist.tile([P, DM], BF16, name=f"wo{h}") for h in range(HPC)]
        xp = ctx.enter_context(tc.tile_pool(name="xT", bufs=1))
        wp = ctx.enter_context(tc.tile_pool(name="w", bufs=1))
        rp = ctx.enter_context(tc.tile_pool(name="rope", bufs=8))

        # DMA priority order (HWDGE + the transfer engines serialize, so
        # issue order IS arrival order): rope tables, then the pre-phase
        # wavefront (w cols [q0|k0|v] + x first halves, k-interleaved),
        # then x second halves, mask, w cols [q1|k1], wo. w_all columns
        # are host-reordered to [q0, k0, v, q1, k1] to enable the split.
        # Batched loads: HWDGE issue bandwidth (~0.63us per DMA) is the
        # startup bottleneck, so w/x load as 4-k-group DMAs via 3-D tiles
        # and partition-inner DRAM views; x additionally splits into
        # column halves so the first s-tiles unblock early.
        wt_all = wp.tile([P, NKC, 3 * CW], BF16, name="wt")
        xt_all = xp.tile([P, NKC, S], BF16, name="xt")
        w_r = w_d.rearrange("(k p) c -> p k c", p=P)
        x_r = xT_d.rearrange("(k p) c -> p k c", p=P)
        HS = S // 2
        WA = 2 * P + CW  # wavefront w cols: q0 | k0 | v
        # first k-group split in two so the very first matmuls start ~3us;
        # all four head-of-line issues go via SP — the scalar queue is
        # blocked ~1.3us at t=0 by the activation-table load
        nc.sync.dma_start(out=wt_all[:, 0:1, 0:WA], in_=w_r[:, 0:1, 0:WA])
        nc.sync.dma_start(out=xt_all[:, 0:1, 0:HS], in_=x_r[:, 0:1, 0:HS])
        nc.sync.dma_start(out=wt_all[:, 1:2, 0:WA], in_=w_r[:, 1:2, 0:WA])
        nc.sync.dma_start(out=xt_all[:, 1:2, 0:HS], in_=x_r[:, 1:2, 0:HS])
        nc.sync.dma_start(out=wt_all[:, 2:4, 0:WA], in_=w_r[:, 2:4, 0:WA])
        nc.sync.dma_start(out=xt_all[:, 2:4, 0:HS], in_=x_r[:, 2:4, 0:HS])
        # rope tables: needed ~10us in, after the first k-group
        nc.scalar.dma_start(out=cos_sb[:], in_=cos_d[:])
        nc.scalar.dma_start(out=sin_sb[:], in_=sin_d[:])
        for g in range(4, NKC, 4):
            nc.sync.dma_start(
                out=wt_all[:, g:g + 4, 0:WA], in_=w_r[:, g:g + 4, 0:WA])
            nc.scalar.dma_start(
                out=xt_all[:, g:g + 4, 0:HS], in_=x_r[:, g:g + 4, 0:HS])
        for g in range(0, NKC, 4):
            (nc.sync if g % 8 == 0 else nc.scalar).dma_start(
                out=xt_all[:, g:g + 4, HS:S], in_=x_r[:, g:g + 4, HS:S])
        for g in range(0, NKC, 8):  # q1|k1 w cols, first needed mid-B0
            nc.sync.dma_start(
                out=wt_all[:, g:g + 8, WA:3 * CW],
                in_=w_r[:, g:g + 8, WA:3 * CW])
        nc.sync.dma_start(out=mask_sb[:], in_=mask_d[:])
        nc.vector.memset(ones_bf[:], 1.0)
        for h in range(HPC):
            nc.sync.dma_start(out=wo_sb[h][:], in_=wo_d[h * P:(h + 1) * P, :])

        # w_all column offsets after host reorder [q0, k0, v, q1, k1]
        W_OFF = {0: 0, 2: P, 1: 2 * P + CW, 3: 3 * P + CW}
        V_OFF = 2 * P

        def qk_chunks(pool, c, st):
            """q/k feature chunk c (0: q-h0, 1: q-h1, 2: k-h0, 3: k-h1),
            s-tile st, transposed layout + fused rope, as 4 PE micro-steps."""
            dst = (qT, kT)[c // HPC][c % HPC]
            wo_ = W_OFF[c]
            state = {}

            def mm(k0):
                def f():
                    if k0 == 0:
                        state["ps"] = pool.tile([P, QT], F32, name="f")
                    ps = state["ps"]
                    for k in range(k0, k0 + 4):
                        nc.tensor.matmul(
                            ps[:],
                            wt_all[:, k, wo_:wo_ + P],
                            xt_all[:, k, st * QT:(st + 1) * QT],
                            start=(k == 0), stop=(k == NKC - 1),
                        )
                    if k0 == NKC - 4:
                        ps = state["ps"]
                        cs = slice(st * QT, (st + 1) * QT)
                        # Pool cannot read PSUM: both rope multiplies run
                        # on DVE; the all-SBUF add goes to Pool.
                        t1 = rp.tile([P, QT], BF16, name="t1")
                        nc.vector.tensor_mul(t1[:], ps[:], cos_sb[:, cs])
                        t2 = rp.tile([P, QT], BF16, name="t2")
                        nc.vector.tensor_mul(
                            t2[0:HW, :], ps[HW:HD, :], sin_sb[0:HW, cs])
                        nc.vector.tensor_mul(
                            t2[HW:HD, :], ps[0:HW, :], sin_sb[HW:HD, cs])
                        nc.gpsimd.tensor_add(dst[:, cs], t1[:], t2[:])
                return f
            return [mm(k0) for k0 in range(0, NKC, 4)]

        def v_chunks(pool, sc):
            """v s-chunk sc in natural layout, as 2 PE micro-steps."""
            state = {}

            def mm(k0):
                def f():
                    if k0 == 0:
                        state["ps"] = pool.tile([P, QT], F32, name="f")
                    ps = state["ps"]
                    for k in range(k0, k0 + 8):
                        nc.tensor.matmul(
                            ps[:, 0:CW],
                            xt_all[:, k, sc * P:(sc + 1) * P],
                            wt_all[:, k, V_OFF:V_OFF + CW],
                            start=(k == 0), stop=(k == NKC - 1),
                        )
                    if k0 == NKC - 8:
                        for h in range(HPC):
                            nc.vector.tensor_copy(
                                v_sb[h][:, sc * P:(sc + 1) * P],
                                ps[:, h * HD:(h + 1) * HD],
                            )
                return f
            return [mm(0), mm(8)]

        o_r = out_d.rearrange("(o p) s -> p o s", p=P)

        def c_chunks(st, outp, c_ps, alt=False):
            """output-projection pieces for s-tile st, 1 PE micro-step each;
            results stage into 4-oc-wide tiles DMA'd as one transfer."""
            state = {}

            def piece(oc):
                def f():
                    acc = c_ps.tile([P, QT], F32, name="f")
                    for h in range(HPC):
                        nc.tensor.matmul(
                            acc[:],
                            wo_sb[h][:, oc * P:(oc + 1) * P],
                            oT[h][:, st * QT:(st + 1) * QT],
                            start=(h == 0), stop=(h == HPC - 1),
                        )
                    if oc % 4 == 0:
                        state["osb"] = outp.tile([P, 4, QT], BF16, name="osb")
                    osb = state["osb"]
                    # ACT carries the B1 tanh/exp chain: only 1 in 4 copies
                    # goes there — except in the drain round (alt), where
                    # ACT is free and copies alternate 50/50
                    if (oc % 2 == 0) if alt else (oc % 4 == 0):
                        nc.scalar.copy(osb[:, oc % 4, :], acc[:])
                    else:
                        nc.vector.tensor_copy(osb[:, oc % 4, :], acc[:])
                    if alt and oc % 2 == 1:
                        # drain round: ship half-tiles so the final
                        # transfer after the last copy is shorter
                        nc.sync.dma_start(
                            out=o_r[:, oc - 1:oc + 1,
                                    st * QT:(st + 1) * QT],
                            in_=osb[:, (oc % 4) - 1:(oc % 4) + 1, :])
                    elif not alt and oc % 4 == 3:
                        nc.sync.dma_start(
                            out=o_r[:, oc - 3:oc + 1,
                                    st * QT:(st + 1) * QT],
                            in_=osb[:])
                return f
            return [piece(oc) for oc in range(NKC)]

        class Feeder:
            """Doles out independent PE micro-steps to hide ACT latency."""
            def __init__(self):
                self.chunks = []

            def add(self, chunks):
                self.chunks.extend(chunks)

            def step(self, n):
                for _ in range(n):
                    if self.chunks:
                        self.chunks.pop(0)()

            def drain(self):
                self.step(len(self.chunks))

        def emit_attn(h, t, pools, feeder, per_pair):
            s_ps, o_ps, l_ps, thp, pp, np_ = pools
            o_acc = o_ps.tile([P, QT], F32, name="o_acc")
            l_acc = l_ps.tile([1, QT], F32, name="l_acc")
            npair = 2 * t + 2
            q_ap = qT[h][:, t * QT:(t + 1) * QT]

            def emit_pv(pT, p, last):
                for i in range(2):
                    kc = 2 * p + i
                    nc.tensor.matmul(
                        o_acc[:],
                        v_sb[h][:, kc * P:(kc + 1) * P],
                        pT[:, i * QT:(i + 1) * QT],
                        start=(kc == 0), stop=(last and i == 1),
                    )
                    nc.tensor.matmul(
                        l_acc[:], ones_bf[:, 0:1],
                        pT[:, i * QT:(i + 1) * QT],
                        start=(kc == 0), stop=(last and i == 1),
                    )

            prev = None
            for p in range(npair):
                sp = s_ps.tile([P, 2 * QT], F32, name="sp")
                for i in range(2):
                    kc = 2 * p + i
                    nc.tensor.matmul(
                        sp[:, i * QT:(i + 1) * QT],
                        kT[h][:, kc * P:(kc + 1) * P], q_ap,
                        start=True, stop=True,
                    )
                feeder.step(per_pair)
                th = thp.tile([P, 2 * QT], F32, name="th")
                nc.scalar.activation(th[:], sp[:], Tanh, scale=C1)
                pT = pp.tile([P, 2 * QT], BF16, name="pTt")
                nc.scalar.activation(pT[:], th[:], Exp, scale=SOFTCAP)
                # masked pairs are the last two: p==2t (u=0,1), p==2t+1 (u=2,3)
                u0 = 2 * (p - 2 * t)
                if u0 >= 0:
                    nc.vector.tensor_mul(
                        pT[:], pT[:], mask_sb[:, u0 * QT:(u0 + 2) * QT])
                if prev is not None:
                    emit_pv(prev[0], prev[1], last=False)
                prev = (pT, p)
            emit_pv(prev[0], prev[1], last=True)
            recip = np_.tile([1, QT], F32, name="recip")
            nc.vector.reciprocal(recip[:], l_acc[:])
            bcast = np_.tile([P, QT], F32, name="bcast")
            nc.gpsimd.partition_broadcast(bcast[:], recip[:])
            nc.vector.tensor_mul(
                oT[h][:, t * QT:(t + 1) * QT], o_acc[:], bcast[:])

        # ---------- phase A (pre-attention part) ----------
        # head 0's q/k + the first 4 v chunks. Tiles needing only the x
        # first halves come first, k-interleaved within 3-tile windows so
        # the PE tracks the DMA wavefront instead of stalling on one tile.
        def interleave(units):
            out = []
            for step in range(max(len(u) for u in units)):
                for u in units:
                    if step < len(u):
                        out.append(u[step])
            return out

        # The A phase is DMA-bound (~35us of input wavefront), so all v
        # chunks ride along in its PE bubbles, ordered by which x quarter
        # they need.
        with ExitStack() as ctxA:
            qkA = ctxA.enter_context(
                tc.tile_pool(name="qkA", bufs=6, space="PSUM"))
            pre = Feeder()
            pre.add(interleave([qk_chunks(qkA, 0, 0), qk_chunks(qkA, 2, 0)]))
            pre.add(interleave([qk_chunks(qkA, 0, 1), qk_chunks(qkA, 2, 1)]))
            for sc in range(0, 8):
                pre.add(v_chunks(qkA, sc))
            pre.add(interleave([qk_chunks(qkA, 0, 2), qk_chunks(qkA, 2, 2)]))
            for sc in range(8, 12):
                pre.add(v_chunks(qkA, sc))
            pre.add(interleave([qk_chunks(qkA, 0, 3), qk_chunks(qkA, 2, 3)]))
            for sc in range(12, 16):
                pre.add(v_chunks(qkA, sc))
            pre.drain()

        # ---------- phase B0: head-0 attention + A-fill ----------
        # shared fill/output-projection PSUM pool (one tag, 2 banks);
        # created below the B pools so those can close before the drain
        fps = ctx.enter_context(tc.tile_pool(name="fps", bufs=2, space="PSUM"))
        outp = ctx.enter_context(tc.tile_pool(name="out", bufs=4))
        fill = Feeder()
        with ExitStack() as ctxB:
            s_ps = ctxB.enter_context(
                tc.tile_pool(name="s_ps", bufs=2, space="PSUM"))
            o_ps = ctxB.enter_context(
                tc.tile_pool(name="o_ps", bufs=1, space="PSUM"))
            l_ps = ctxB.enter_context(
                tc.tile_pool(name="l_ps", bufs=1, space="PSUM"))
            thp = ctxB.enter_context(tc.tile_pool(name="tanh", bufs=3))
            pp = ctxB.enter_context(tc.tile_pool(name="pT", bufs=3))
            np_ = ctxB.enter_context(tc.tile_pool(name="norm", bufs=4))
            bpools = (s_ps, o_ps, l_ps, thp, pp, np_)

            for st in (0, 1):
                fill.add(qk_chunks(fps, 1, st))
                fill.add(qk_chunks(fps, 3, st))
            for t in range(NQT):
                emit_attn(0, t, bpools, fill, per_pair=3)
            # q1/k1 st2/st3 are first needed by B1 t2/t3: defer them into
            # the otherwise-unfilled B1 t0/t1 windows.
            for st in (2, 3):
                fill.add(qk_chunks(fps, 1, st))
                fill.add(qk_chunks(fps, 3, st))

            # ---------- phase B1 + C: attention + output projection ----
            for t, per in zip(range(NQT), (6, 5, 3, 2)):
                emit_attn(1, t, bpools, fill, per_pair=per)
                if t < NQT - 1:
                    fill.add(c_chunks(t, outp, fps))
        # drain round: B pools are closed, give the last output-projection
        # round a wide PSUM pool so its pieces pipeline
        cD = ctx.enter_context(tc.tile_pool(name="cD", bufs=5, space="PSUM"))
        fill.add(c_chunks(NQT - 1, outp, cD, alt=True))
        fill.drain()


_NC_CACHE = None


def _get_nc():
    global _NC_CACHE
    if _NC_CACHE is None:
        _NC_CACHE = build_nc()
    return _NC_CACHE


def _rope_perm():
    """per-head column permutation de-interleaving rotary pairs"""
    perm = np.zeros(DM, np.int64)
    for h in range(H):
        base = h * HD
        perm[base:base + HD // 2] = base + np.arange(0, HD, 2)
        perm[base + HD // 2:base + HD] = base + np.arange(1, HD, 2)
    return perm


def make_in_maps(x, wq, wk, wv, wo, freqs_cos, freqs_sin):
    x = np.asarray(x, np.float32).reshape(S, DM)
    wq = np.asarray(wq, np.float32)
    wk = np.asarray(wk, np.float32)
    wv = np.asarray(wv, np.float32)
    wo = np.asarray(wo, np.float32)
    xT = np.ascontiguousarray(x.T).astype(NPBF16)
    perm = _rope_perm()
    wq_p = wq[:, perm]
    wk_p = wk[:, perm]
    # transposed rope tables: C = [cosT; cosT], S' = [-sinT; sinT]
    cosT = np.asarray(freqs_cos, np.float32).T  # [64, S]
    sinT = np.asarray(freqs_sin, np.float32).T
    cosT2 = np.concatenate([cosT, cosT], axis=0).astype(NPBF16)
    sinT2 = np.concatenate([-sinT, sinT], axis=0).astype(NPBF16)
    # mask[i, u*QT + j] = 1 if i <= j - 128*u else 0  (keep kj <= qi)
    i_idx = np.arange(P)[:, None]
    j_idx = np.arange(QT)[None, :]
    mask = np.concatenate(
        [(i_idx <= j_idx - P * u) for u in range(4)], axis=1
    ).astype(NPBF16)
    in_maps = []
    for c in range(N_CORES):
        cs = slice(c * CW, (c + 1) * CW)
        h0 = slice(c * CW, c * CW + HD)
        h1 = slice(c * CW + HD, (c + 1) * CW)
        # device column order: [q-h0, k-h0, v, q-h1, k-h1]
        w_all = np.concatenate(
            [wq_p[:, h0], wk_p[:, h0], wv[:, cs],
             wq_p[:, h1], wk_p[:, h1]], axis=1).astype(NPBF16)
        wo_c = np.ascontiguousarray(wo[cs, :]).astype(NPBF16)
        in_maps.append({
            "xT": xT,
            "w_all": np.ascontiguousarray(w_all),
            "wo_c": wo_c,
            "cosT2": cosT2,
            "sinT2": sinT2,
            "mask": mask,
        })
    return in_maps


def assemble_output(results):
    acc = results[0]["outT"].astype(np.float32)
    for r in results[1:]:
        acc += np.asarray(r["outT"]).astype(np.float32)
    return np.ascontiguousarray(acc.T).reshape(1, S, DM).astype(np.float32)


def kernel(x, wq, wk, wv, wo, freqs_cos, freqs_sin):
    nc = _get_nc()
    in_maps = make_in_maps(x, wq, wk, wv, wo, freqs_cos, freqs_sin)
    res = run_bass_kernel_spmd(nc, in_maps, core_ids=list(range(N_CORES)))
    return assemble_output(res.results)


if __name__ == "__main__":
    rng = np.random.default_rng(0)
    ins = {
        "x": rng.standard_normal((1, S, DM), np.float32),
        "wq": rng.standard_normal((DM, DM), np.float32) / np.sqrt(DM),
        "wk": rng.standard_normal((DM, DM), np.float32) / np.sqrt(DM),
        "wv": rng.standard_normal((DM, DM), np.float32) / np.sqrt(DM),
        "wo": rng.standard_normal((DM, DM), np.float32) / np.sqrt(DM),
        "freqs_cos": rng.standard_normal((S, HD // 2), np.float32),
        "freqs_sin": rng.standard_normal((S, HD // 2), np.float32),
    }
    out = kernel(**ins)
    print("out", out.shape, out.dtype, np.abs(out).mean())


# revision 77
# speedup vs baseline: 1.0858x; 1.0076x over previous
"""Trainium2 Bass kernel for Llama-like attention (16 heads, tanh softcap, RoPE).

Sharding: tensor-parallel over heads, fully collective-free. Each of the 8
cores computes 2 heads end-to-end and a *partial* output projection
(o_local @ wo_rows_local)^T; the host sums the 8 partial outputs. With no
on-device collective, each core's NEFF span is pure local compute — no
cross-core rendezvous.

Per-core pipeline (engine-balanced against the ~165us PE floor):
  - q^T/k^T computed directly in transposed layout ([head_dim, s]) via
    matmul(w_slice^T, x^T): no PE transposes. Weight columns of wq/wk are
    pre-permuted on the host to de-interleave even/odd rotary pairs (the
    permutation cancels inside q.k).
  - RoPE in transposed layout straight out of PSUM: rot = A*C + swap(A)*S'
    with C = [cosT; cosT], S' = [-sinT; sinT]. The partition-half swap is
    two half-height Vector multiplies reading PSUM at a partition offset
    (Pool cannot touch PSUM); the all-SBUF add runs on Pool. ACT stays
    free for the softmax chain.
  - attention with scores transposed ([kj, qi]) so softmaxed probabilities
    feed the PV matmul directly as the moving operand. tanh softcap bounds
    scores, so softmax needs no row-max pass: p = exp(50*tanh(.)),
    l = ones-row matmul, o = p@v / l. Head 0's ACT-bound window is filled
    with head 1's q/k projection and the tail v chunks; head 1's windows
    are filled with the output-projection pieces for the q-tile that just
    finished.
  - output projection pieces acc[oc(128), st(512)] += wo_h[:, oc]^T @ oT_h
    accumulated over the 2 local heads, copied to SBUF bf16 (ACT/DVE
    alternating) and DMA'd per piece. Host sums partials and transposes.
"""

import os
import sys

for _p in ("/root/.axon_site/_ro/trn_rl_repo", "/opt/trn_rl_repo"):
    if os.path.isdir(_p) and _p not in sys.path:
        sys.path.append(_p)

import numpy as np
import ml_dtypes
from contextlib import ExitStack

import concourse.bass as bass
import concourse.bacc as bacc
import concourse.mybir as mybir
import concourse.tile as tile
from concourse.bass_utils import run_bass_kernel_spmd

BF16 = mybir.dt.bfloat16
F32 = mybir.dt.float32
NPBF16 = ml_dtypes.bfloat16

N_CORES = 8
S = 2048          # sequence length
DM = 2048         # model dim
H = 16            # heads
HD = 128          # head dim
HPC = H // N_CORES  # heads per core = 2
CW = HPC * HD     # per-core projection width = 256
P = 128
HW = HD // 2      # 64
QT = 512          # query tile (free dim of attention matmuls)
NQT = S // QT     # 4 query tiles per head
NSC = S // P      # 16 sequence chunks
NKC = DM // P     # 16 contraction chunks
NST = S // QT     # 4 s-tiles
SOFTCAP = 50.0
C1 = 1.0 / (SOFTCAP * np.sqrt(HD))

Tanh = mybir.ActivationFunctionType.Tanh
Exp = mybir.ActivationFunctionType.Exp


def build_nc(reps=1, single=False):
    nc = bacc.Bacc("TRN2", target_bir_lowering=False, num_devices=N_CORES)

    xT_d = nc.dram_tensor("xT", [DM, S], BF16, kind="ExternalInput")
    w_d = nc.dram_tensor("w_all", [DM, 3 * CW], BF16, kind="ExternalInput")
    wo_d = nc.dram_tensor("wo_c", [CW, DM], BF16, kind="ExternalInput")
    cos_d = nc.dram_tensor("cosT2", [P, S], BF16, kind="ExternalInput")
    sin_d = nc.dram_tensor("sinT2", [P, S], BF16, kind="ExternalInput")
    mask_d = nc.dram_tensor("mask", [P, 4 * QT], BF16, kind="ExternalInput")
    out_d = nc.dram_tensor("outT", [DM, S], BF16, kind="ExternalOutput")

    with tile.TileContext(nc) as tc:
        for _rep in range(reps):
            _emit_body(nc, tc, xT_d, w_d, wo_d, cos_d, sin_d, mask_d, out_d)
    nc.compile()
    return nc


def _emit_body(nc, tc, xT_d, w_d, wo_d, cos_d, sin_d, mask_d, out_d):
    with ExitStack() as ctx:
        # ---------- persistent SBUF ----------
        persist = ctx.enter_context(tc.tile_pool(name="persist", bufs=1))
        qT = [persist.tile([P, S], BF16, name=f"qT{h}") for h in range(HPC)]
        kT = [persist.tile([P, S], BF16, name=f"kT{h}") for h in range(HPC)]
        v_sb = [persist.tile([P, S], BF16, name=f"v{h}") for h in range(HPC)]
        oT = [persist.tile([P, S], BF16, name=f"oT{h}") for h in range(HPC)]
        mask_sb = persist.tile([P, 4 * QT], BF16, name="mask")
        ones_bf = persist.tile([P, 1], BF16, name="ones")
        cos_sb = persist.tile([P, S], BF16, name="cosT2")
        sin_sb = persist.tile([P, S], BF16, name="sinT2")
        wo_sb = [persist.tile([P, DM], BF16, name=f"wo{h}") for h in range(HPC)]
        xp = ctx.enter_context(tc.tile_pool(name="xT", bufs=1))
        wp = ctx.enter_context(tc.tile_pool(name="w", bufs=1))
        rp = ctx.enter_context(tc.tile_pool(name="rope", bufs=8))

        # DMA priority order (HWDGE + the transfer engines serialize, so
        # issue order IS arrival order): rope tables, then the pre-phase
        # wavefront (w cols [q0|k0|v] + x first halves, k-interleaved),
        # then x second halves, mask, w cols [q1|k1], wo. w_all columns
        # are host-reordered to [q0, k0, v, q1, k1] to enable the split.
        # Batched loads: HWDGE issue bandwidth (~0.63us per DMA) is the
        # startup bottleneck, so w/x load as 4-k-group DMAs via 3-D tiles
        # and partition-inner DRAM views; x additionally splits into
        # column halves so the first s-tiles unblock early.
        wt_all = wp.tile([P, NKC, 3 * CW], BF16, name="wt")
        xt_all = xp.tile([P, NKC, S], BF16, name="xt")
        w_r = w_d.rearrange("(k p) c -> p k c", p=P)
        x_r = xT_d.rearrange("(k p) c -> p k c", p=P)
        HS = S // 2
        WA = 2 * P + CW  # wavefront w cols: q0 | k0 | v
        # first k-group split in two so the very first matmuls start ~3us;
        # all four head-of-line issues go via SP — the scalar queue is
        # blocked ~1.3us at t=0 by the activation-table load
        nc.sync.dma_start(out=wt_all[:, 0:1, 0:WA], in_=w_r[:, 0:1, 0:WA])
        nc.sync.dma_start(out=xt_all[:, 0:1, 0:HS], in_=x_r[:, 0:1, 0:HS])
        nc.sync.dma_start(out=wt_all[:, 1:2, 0:WA], in_=w_r[:, 1:2, 0:WA])
        nc.sync.dma_start(out=xt_all[:, 1:2, 0:HS], in_=x_r[:, 1:2, 0:HS])
        nc.sync.dma_start(out=wt_all[:, 2:4, 0:WA], in_=w_r[:, 2:4, 0:WA])
        nc.sync.dma_start(out=xt_all[:, 2:4, 0:HS], in_=x_r[:, 2:4, 0:HS])
        # rope tables: needed ~10us in, after the first k-group
        nc.scalar.dma_start(out=cos_sb[:], in_=cos_d[:])
        nc.scalar.dma_start(out=sin_sb[:], in_=sin_d[:])
        for g in range(4, NKC, 4):
            nc.sync.dma_start(
                out=wt_all[:, g:g + 4, 0:WA], in_=w_r[:, g:g + 4, 0:WA])
            nc.scalar.dma_start(
                out=xt_all[:, g:g + 4, 0:HS], in_=x_r[:, g:g + 4, 0:HS])
        for g in range(0, NKC, 4):
            (nc.sync if g % 8 == 0 else nc.scalar).dma_start(
                out=xt_all[:, g:g + 4, HS:S], in_=x_r[:, g:g + 4, HS:S])
        for g in range(0, NKC, 8):  # q1|k1 w cols, first needed mid-B0
            nc.sync.dma_start(
                out=wt_all[:, g:g + 8, WA:3 * CW],
                in_=w_r[:, g:g + 8, WA:3 * CW])
        nc.sync.dma_start(out=mask_sb[:], in_=mask_d[:])
        nc.vector.memset(ones_bf[:], 1.0)
        for h in range(HPC):
            nc.sync.dma_start(out=wo_sb[h][:], in_=wo_d[h * P:(h + 1) * P, :])

        # w_all column offsets after host reorder [q0, k0, v, q1, k1]
        W_OFF = {0: 0, 2: P, 1: 2 * P + CW, 3: 3 * P + CW}
        V_OFF = 2 * P

        def qk_chunks(pool, c, st):
            """q/k feature chunk c (0: q-h0, 1: q-h1, 2: k-h0, 3: k-h1),
            s-tile st, transposed layout + fused rope, as 4 PE micro-steps."""
            dst = (qT, kT)[c // HPC][c % HPC]
            wo_ = W_OFF[c]
            state = {}

            def mm(k0):
                def f():
                    if k0 == 0:
                        state["ps"] = pool.tile([P, QT], F32, name="f")
                    ps = state["ps"]
                    for k in range(k0, k0 + 4):
                        nc.tensor.matmul(
                            ps[:],
                            wt_all[:, k, wo_:wo_ + P],
                            xt_all[:, k, st * QT:(st + 1) * QT],
                            start=(k == 0), stop=(k == NKC - 1),
                        )
                    if k0 == NKC - 4:
                        ps = state["ps"]
                        cs = slice(st * QT, (st + 1) * QT)
                        # Pool cannot read PSUM: both rope multiplies run
                        # on DVE; the all-SBUF add goes to Pool.
                        t1 = rp.tile([P, QT], BF16, name="t1")
                        nc.vector.tensor_mul(t1[:], ps[:], cos_sb[:, cs])
                        t2 = rp.tile([P, QT], BF16, name="t2")
                        nc.vector.tensor_mul(
                            t2[0:HW, :], ps[HW:HD, :], sin_sb[0:HW, cs])
                        nc.vector.tensor_mul(
                            t2[HW:HD, :], ps[0:HW, :], sin_sb[HW:HD, cs])
                        nc.gpsimd.tensor_add(dst[:, cs], t1[:], t2[:])
                return f
            return [mm(k0) for k0 in range(0, NKC, 4)]

        def v_chunks(pool, sc):
            """v s-chunk sc in natural layout, as 2 PE micro-steps."""
            state = {}

            def mm(k0):
                def f():
                    if k0 == 0:
                        state["ps"] = pool.tile([P, QT], F32, name="f")
                    ps = state["ps"]
                    for k in range(k0, k0 + 8):
                        nc.tensor.matmul(
                            ps[:, 0:CW],
                            xt_all[:, k, sc * P:(sc + 1) * P],
                            wt_all[:, k, V_OFF:V_OFF + CW],
                            start=(k == 0), stop=(k == NKC - 1),
                        )
                    if k0 == NKC - 8:
                        for h in range(HPC):
                            nc.vector.tensor_copy(
                                v_sb[h][:, sc * P:(sc + 1) * P],
                                ps[:, h * HD:(h + 1) * HD],
                            )
                return f
            return [mm(0), mm(8)]

        o_r = out_d.rearrange("(o p) s -> p o s", p=P)

        def c_chunks(st, outp, c_ps, alt=False):
            """output-projection pieces for s-tile st, 1 PE micro-step each;
            results stage into 4-oc-wide tiles DMA'd as one transfer."""
            state = {}

            def piece(oc):
                def f():
                    acc = c_ps.tile([P, QT], F32, name="f")
                    for h in range(HPC):
                        nc.tensor.matmul(
                            acc[:],
                            wo_sb[h][:, oc * P:(oc + 1) * P],
                            oT[h][:, st * QT:(st + 1) * QT],
                            start=(h == 0), stop=(h == HPC - 1),
                        )
                    if oc % 4 == 0:
                        state["osb"] = outp.tile([P, 4, QT], BF16, name="osb")
                    osb = state["osb"]
                    # ACT carries the B1 tanh/exp chain: only 1 in 4 copies
                    # goes there — except in the drain round (alt), where
                    # ACT is free and copies alternate 50/50
                    if (oc % 2 == 0) if alt else (oc % 4 == 0):
                        nc.scalar.copy(osb[:, oc % 4, :], acc[:])
                    else:
                        nc.vector.tensor_copy(osb[:, oc % 4, :], acc[:])
                    if alt and oc % 2 == 1:
                        # drain round: ship half-tiles so the final
                        # transfer after the last copy is shorter
                        nc.sync.dma_start(
                            out=o_r[:, oc - 1:oc + 1,
                                    st * QT:(st + 1) * QT],
                            in_=osb[:, (oc % 4) - 1:(oc % 4) + 1, :])
                    elif not alt and oc % 4 == 3:
                        nc.sync.dma_start(
                            out=o_r[:, oc - 3:oc + 1,
                                    st * QT:(st + 1) * QT],
                            in_=osb[:])
                return f
            return [piece(oc) for oc in range(NKC)]

        class Feeder:
            """Doles out independent PE micro-steps to hide ACT latency."""
            def __init__(self):
                self.chunks = []

            def add(self, chunks):
                self.chunks.extend(chunks)

            def step(self, n):
                for _ in range(n):
                    if self.chunks:
                        self.chunks.pop(0)()

            def drain(self):
                self.step(len(self.chunks))

        def emit_attn(h, t, pools, feeder, per_pair):
            s_ps, o_ps, l_ps, thp, pp, np_ = pools
            o_acc = o_ps.tile([P, QT], F32, name="o_acc")
            l_acc = l_ps.tile([1, QT], F32, name="l_acc")
            npair = 2 * t + 2
            q_ap = qT[h][:, t * QT:(t + 1) * QT]

            def emit_pv(pT, p, last):
                for i in range(2):
                    kc = 2 * p + i
                    nc.tensor.matmul(
                        o_acc[:],
                        v_sb[h][:, kc * P:(kc + 1) * P],
                        pT[:, i * QT:(i + 1) * QT],
                        start=(kc == 0), stop=(last and i == 1),
                    )
                    nc.tensor.matmul(
                        l_acc[:], ones_bf[:, 0:1],
                        pT[:, i * QT:(i + 1) * QT],
                        start=(kc == 0), stop=(last and i == 1),
                    )

            prev = None
            for p in range(npair):
                sp = s_ps.tile([P, 2 * QT], F32, name="sp")
                for i in range(2):
                    kc = 2 * p + i
                    nc.tensor.matmul(
                        sp[:, i * QT:(i + 1) * QT],
                        kT[h][:, kc * P:(kc + 1) * P], q_ap,
                        start=True, stop=True,
                    )
                feeder.step(per_pair)
                th = thp.tile([P, 2 * QT], F32, name="th")
                nc.scalar.activation(th[:], sp[:], Tanh, scale=C1)
                pT = pp.tile([P, 2 * QT], BF16, name="pTt")
                nc.scalar.activation(pT[:], th[:], Exp, scale=SOFTCAP)
                # masked pairs are the last two: p==2t (u=0,1), p==2t+1 (u=2,3)
                u0 = 2 * (p - 2 * t)
                if u0 >= 0:
                    nc.vector.tensor_mul(
                        pT[:], pT[:], mask_sb[:, u0 * QT:(u0 + 2) * QT])
                if prev is not None:
                    emit_pv(prev[0], prev[1], last=False)
                prev = (pT, p)
            emit_pv(prev[0], prev[1], last=True)
            recip = np_.tile([1, QT], F32, name="recip")
            nc.vector.reciprocal(recip[:], l_acc[:])
            bcast = np_.tile([P, QT], F32, name="bcast")
            nc.gpsimd.partition_broadcast(bcast[:], recip[:])
            nc.vector.tensor_mul(
                oT[h][:, t * QT:(t + 1) * QT], o_acc[:], bcast[:])

        # ---------- phase A (pre-attention part) ----------
        # head 0's q/k + the first 4 v chunks. Tiles needing only the x
        # first halves come first, k-interleaved within 3-tile windows so
        # the PE tracks the DMA wavefront instead of stalling on one tile.
        def interleave(units):
            out = []
            for step in range(max(len(u) for u in units)):
                for u in units:
                    if step < len(u):
                        out.append(u[step])
            return out

        # The A phase is DMA-bound (~35us of input wavefront), so all v
        # chunks ride along in its PE bubbles, ordered by which x quarter
        # they need.
        with ExitStack() as ctxA:
            qkA = ctxA.enter_context(
                tc.tile_pool(name="qkA", bufs=6, space="PSUM"))
            pre = Feeder()
            pre.add(interleave([qk_chunks(qkA, 0, 0), qk_chunks(qkA, 2, 0)]))
            pre.add(interleave([qk_chunks(qkA, 0, 1), qk_chunks(qkA, 2, 1)]))
            for sc in range(0, 8):
                pre.add(v_chunks(qkA, sc))
            pre.add(interleave([qk_chunks(qkA, 0, 2), qk_chunks(qkA, 2, 2)]))
            for sc in range(8, 12):
                pre.add(v_chunks(qkA, sc))
            pre.add(interleave([qk_chunks(qkA, 0, 3), qk_chunks(qkA, 2, 3)]))
            for sc in range(12, 16):
                pre.add(v_chunks(qkA, sc))
            pre.drain()

        # ---------- phase B0: head-0 attention + A-fill ----------
        # shared fill/output-projection PSUM pool (one tag, 2 banks);
        # created below the B pools so those can close before the drain
        fps = ctx.enter_context(tc.tile_pool(name="fps", bufs=2, space="PSUM"))
        outp = ctx.enter_context(tc.tile_pool(name="out", bufs=4))
        fill = Feeder()
        with ExitStack() as ctxB:
            s_ps = ctxB.enter_context(
                tc.tile_pool(name="s_ps", bufs=2, space="PSUM"))
            o_ps = ctxB.enter_context(
                tc.tile_pool(name="o_ps", bufs=1, space="PSUM"))
            l_ps = ctxB.enter_context(
                tc.tile_pool(name="l_ps", bufs=1, space="PSUM"))
            thp = ctxB.enter_context(tc.tile_pool(name="tanh", bufs=3))
            pp = ctxB.enter_context(tc.tile_pool(name="pT", bufs=3))
            np_ = ctxB.enter_context(tc.tile_pool(name="norm", bufs=4))
            bpools = (s_ps, o_ps, l_ps, thp, pp, np_)

            for st in (0, 1):
                fill.add(qk_chunks(fps, 1, st))
                fill.add(qk_chunks(fps, 3, st))
            for t in range(NQT):
                emit_attn(0, t, bpools, fill, per_pair=3)
            # q1/k1 st2/st3 are first needed by B1 t2/t3: defer them into
            # the otherwise-unfilled B1 t0/t1 windows.
            for st in (2, 3):
                fill.add(qk_chunks(fps, 1, st))
                fill.add(qk_chunks(fps, 3, st))

            # ---------- phase B1 + C: attention + output projection ----
            for t, per in zip(range(NQT), (6, 5, 3, 1)):
                emit_attn(1, t, bpools, fill, per_pair=per)
                if t < NQT - 1:
                    fill.add(c_chunks(t, outp, fps))
        # drain round: B pools are closed, give the last output-projection
        # round a wide PSUM pool so its pieces pipeline
        cD = ctx.enter_context(tc.tile_pool(name="cD", bufs=5, space="PSUM"))
        fill.add(c_chunks(NQT - 1, outp, cD, alt=True))
        fill.drain()


_NC_CACHE = None


def _get_nc():
    global _NC_CACHE
    if _NC_CACHE is None:
        _NC_CACHE = build_nc()
    return _NC_CACHE


def _rope_perm():
    """per-head column permutation de-interleaving rotary pairs"""
    perm = np.zeros(DM, np.int64)
    for h in range(H):
        base = h * HD
        perm[base:base + HD // 2] = base + np.arange(0, HD, 2)
        perm[base + HD // 2:base + HD] = base + np.arange(1, HD, 2)
    return perm


def make_in_maps(x, wq, wk, wv, wo, freqs_cos, freqs_sin):
    x = np.asarray(x, np.float32).reshape(S, DM)
    wq = np.asarray(wq, np.float32)
    wk = np.asarray(wk, np.float32)
    wv = np.asarray(wv, np.float32)
    wo = np.asarray(wo, np.float32)
    xT = np.ascontiguousarray(x.T).astype(NPBF16)
    perm = _rope_perm()
    wq_p = wq[:, perm]
    wk_p = wk[:, perm]
    # transposed rope tables: C = [cosT; cosT], S' = [-sinT; sinT]
    cosT = np.asarray(freqs_cos, np.float32).T  # [64, S]
    sinT = np.asarray(freqs_sin, np.float32).T
    cosT2 = np.concatenate([cosT, cosT], axis=0).astype(NPBF16)
    sinT2 = np.concatenate([-sinT, sinT], axis=0).astype(NPBF16)
    # mask[i, u*QT + j] = 1 if i <= j - 128*u else 0  (keep kj <= qi)
    i_idx = np.arange(P)[:, None]
    j_idx = np.arange(QT)[None, :]
    mask = np.concatenate(
        [(i_idx <= j_idx - P * u) for u in range(4)], axis=1
    ).astype(NPBF16)
    in_maps = []
    for c in range(N_CORES):
        cs = slice(c * CW, (c + 1) * CW)
        h0 = slice(c * CW, c * CW + HD)
        h1 = slice(c * CW + HD, (c + 1) * CW)
        # device column order: [q-h0, k-h0, v, q-h1, k-h1]
        w_all = np.concatenate(
            [wq_p[:, h0], wk_p[:, h0], wv[:, cs],
             wq_p[:, h1], wk_p[:, h1]], axis=1).astype(NPBF16)
        wo_c = np.ascontiguousarray(wo[cs, :]).astype(NPBF16)
        in_maps.append({
            "xT": xT,
            "w_all": np.ascontiguousarray(w_all),
            "wo_c": wo_c,
            "cosT2": cosT2,
            "sinT2": sinT2,
            "mask": mask,
        })
    return in_maps


def assemble_output(results):
    acc = results[0]["outT"].astype(np.float32)
    for r in results[1:]:
        acc += np.asarray(r["outT"]).astype(np.float32)
    return np.ascontiguousarray(acc.T).reshape(1, S, DM).astype(np.float32)


def kernel(x, wq, wk, wv, wo, freqs_cos, freqs_sin):
    nc = _get_nc()
    in_maps = make_in_maps(x, wq, wk, wv, wo, freqs_cos, freqs_sin)
    res = run_bass_kernel_spmd(nc, in_maps, core_ids=list(range(N_CORES)))
    return assemble_output(res.results)


if __name__ == "__main__":
    rng = np.random.default_rng(0)
    ins = {
        "x": rng.standard_normal((1, S, DM), np.float32),
        "wq": rng.standard_normal((DM, DM), np.float32) / np.sqrt(DM),
        "wk": rng.standard_normal((DM, DM), np.float32) / np.sqrt(DM),
        "wv": rng.standard_normal((DM, DM), np.float32) / np.sqrt(DM),
        "wo": rng.standard_normal((DM, DM), np.float32) / np.sqrt(DM),
        "freqs_cos": rng.standard_normal((S, HD // 2), np.float32),
        "freqs_sin": rng.standard_normal((S, HD // 2), np.float32),
    }
    out = kernel(**ins)
    print("out", out.shape, out.dtype, np.abs(out).mean())


# revision 78
# speedup vs baseline: 1.0980x; 1.0112x over previous
"""Trainium2 Bass kernel for Llama-like attention (16 heads, tanh softcap, RoPE).

Sharding: tensor-parallel over heads, fully collective-free. Each of the 8
cores computes 2 heads end-to-end and a *partial* output projection
(o_local @ wo_rows_local)^T; the host sums the 8 partial outputs. With no
on-device collective, each core's NEFF span is pure local compute — no
cross-core rendezvous.

Per-core pipeline (engine-balanced against the ~165us PE floor):
  - q^T/k^T computed directly in transposed layout ([head_dim, s]) via
    matmul(w_slice^T, x^T): no PE transposes. Weight columns of wq/wk are
    pre-permuted on the host to de-interleave even/odd rotary pairs (the
    permutation cancels inside q.k).
  - RoPE in transposed layout straight out of PSUM: rot = A*C + swap(A)*S'
    with C = [cosT; cosT], S' = [-sinT; sinT]. The partition-half swap is
    two half-height Vector multiplies reading PSUM at a partition offset
    (Pool cannot touch PSUM); the all-SBUF add runs on Pool. ACT stays
    free for the softmax chain.
  - attention with scores transposed ([kj, qi]) so softmaxed probabilities
    feed the PV matmul directly as the moving operand. tanh softcap bounds
    scores, so softmax needs no row-max pass: p = exp(50*tanh(.)),
    l = ones-row matmul, o = p@v / l. Head 0's ACT-bound window is filled
    with head 1's q/k projection and the tail v chunks; head 1's windows
    are filled with the output-projection pieces for the q-tile that just
    finished.
  - output projection pieces acc[oc(128), st(512)] += wo_h[:, oc]^T @ oT_h
    accumulated over the 2 local heads, copied to SBUF bf16 (ACT/DVE
    alternating) and DMA'd per piece. Host sums partials and transposes.
"""

import os
import sys

for _p in ("/root/.axon_site/_ro/trn_rl_repo", "/opt/trn_rl_repo"):
    if os.path.isdir(_p) and _p not in sys.path:
        sys.path.append(_p)

import numpy as np
import ml_dtypes
from contextlib import ExitStack

import concourse.bass as bass
import concourse.bacc as bacc
import concourse.mybir as mybir
import concourse.tile as tile
from concourse.bass_utils import run_bass_kernel_spmd

BF16 = mybir.dt.bfloat16
F32 = mybir.dt.float32
NPBF16 = ml_dtypes.bfloat16

N_CORES = 8
S = 2048          # sequence length
DM = 2048         # model dim
H = 16            # heads
HD = 128          # head dim
HPC = H // N_CORES  # heads per core = 2
CW = HPC * HD     # per-core projection width = 256
P = 128
HW = HD // 2      # 64
QT = 512          # query tile (free dim of attention matmuls)
NQT = S // QT     # 4 query tiles per head
NSC = S // P      # 16 sequence chunks
NKC = DM // P     # 16 contraction chunks
NST = S // QT     # 4 s-tiles
SOFTCAP = 50.0
C1 = 1.0 / (SOFTCAP * np.sqrt(HD))

Tanh = mybir.ActivationFunctionType.Tanh
Exp = mybir.ActivationFunctionType.Exp


def build_nc(reps=1, single=False):
    nc = bacc.Bacc("TRN2", target_bir_lowering=False, num_devices=N_CORES)

    xT_d = nc.dram_tensor("xT", [DM, S], BF16, kind="ExternalInput")
    w_d = nc.dram_tensor("w_all", [DM, 3 * CW], BF16, kind="ExternalInput")
    wo_d = nc.dram_tensor("wo_c", [CW, DM], BF16, kind="ExternalInput")
    cos_d = nc.dram_tensor("cosT2", [P, S], BF16, kind="ExternalInput")
    sin_d = nc.dram_tensor("sinT2", [P, S], BF16, kind="ExternalInput")
    mask_d = nc.dram_tensor("mask", [P, 4 * QT], BF16, kind="ExternalInput")
    out_d = nc.dram_tensor("outT", [DM, S], BF16, kind="ExternalOutput")

    with tile.TileContext(nc) as tc:
        for _rep in range(reps):
            _emit_body(nc, tc, xT_d, w_d, wo_d, cos_d, sin_d, mask_d, out_d)
    nc.compile()
    return nc


def _emit_body(nc, tc, xT_d, w_d, wo_d, cos_d, sin_d, mask_d, out_d):
    with ExitStack() as ctx:
        # ---------- persistent SBUF ----------
        persist = ctx.enter_context(tc.tile_pool(name="persist", bufs=1))
        qT = [persist.tile([P, S], BF16, name=f"qT{h}") for h in range(HPC)]
        kT = [persist.tile([P, S], BF16, name=f"kT{h}") for h in range(HPC)]
        v_sb = [persist.tile([P, S], BF16, name=f"v{h}") for h in range(HPC)]
        oT = [persist.tile([P, S], BF16, name=f"oT{h}") for h in range(HPC)]
        mask_sb = persist.tile([P, 4 * QT], BF16, name="mask")
        ones_bf = persist.tile([P, 1], BF16, name="ones")
        cos_sb = persist.tile([P, S], BF16, name="cosT2")
        sin_sb = persist.tile([P, S], BF16, name="sinT2")
        wo_sb = [persist.tile([P, DM], BF16, name=f"wo{h}") for h in range(HPC)]
        xp = ctx.enter_context(tc.tile_pool(name="xT", bufs=1))
        wp = ctx.enter_context(tc.tile_pool(name="w", bufs=1))
        rp = ctx.enter_context(tc.tile_pool(name="rope", bufs=8))

        # DMA priority order (HWDGE + the transfer engines serialize, so
        # issue order IS arrival order): rope tables, then the pre-phase
        # wavefront (w cols [q0|k0|v] + x first halves, k-interleaved),
        # then x second halves, mask, w cols [q1|k1], wo. w_all columns
        # are host-reordered to [q0, k0, v, q1, k1] to enable the split.
        # Batched loads: HWDGE issue bandwidth (~0.63us per DMA) is the
        # startup bottleneck, so w/x load as 4-k-group DMAs via 3-D tiles
        # and partition-inner DRAM views; x additionally splits into
        # column halves so the first s-tiles unblock early.
        wt_all = wp.tile([P, NKC, 3 * CW], BF16, name="wt")
        xt_all = xp.tile([P, NKC, S], BF16, name="xt")
        w_r = w_d.rearrange("(k p) c -> p k c", p=P)
        x_r = xT_d.rearrange("(k p) c -> p k c", p=P)
        HS = S // 2
        WA = 2 * P + CW  # wavefront w cols: q0 | k0 | v
        # first k-group split in two so the very first matmuls start ~3us;
        # all four head-of-line issues go via SP — the scalar queue is
        # blocked ~1.3us at t=0 by the activation-table load
        QX = QT  # first x quarter: columns the st0 tiles and v0..3 need
        nc.sync.dma_start(out=wt_all[:, 0:1, 0:WA], in_=w_r[:, 0:1, 0:WA])
        nc.sync.dma_start(out=xt_all[:, 0:1, 0:QX], in_=x_r[:, 0:1, 0:QX])
        nc.sync.dma_start(out=wt_all[:, 1:2, 0:WA], in_=w_r[:, 1:2, 0:WA])
        nc.sync.dma_start(out=xt_all[:, 1:2, 0:QX], in_=x_r[:, 1:2, 0:QX])
        nc.sync.dma_start(out=wt_all[:, 2:4, 0:WA], in_=w_r[:, 2:4, 0:WA])
        nc.sync.dma_start(out=xt_all[:, 2:4, 0:QX], in_=x_r[:, 2:4, 0:QX])
        # rope tables: needed ~10us in, after the first k-group
        nc.scalar.dma_start(out=cos_sb[:], in_=cos_d[:])
        nc.scalar.dma_start(out=sin_sb[:], in_=sin_d[:])
        for g in range(4, NKC, 4):
            nc.sync.dma_start(
                out=wt_all[:, g:g + 4, 0:WA], in_=w_r[:, g:g + 4, 0:WA])
            nc.scalar.dma_start(
                out=xt_all[:, g:g + 4, 0:QX], in_=x_r[:, g:g + 4, 0:QX])
        for g in range(0, NKC, 4):  # x second quarter (st1 tiles)
            (nc.sync if g % 8 == 0 else nc.scalar).dma_start(
                out=xt_all[:, g:g + 4, QX:HS], in_=x_r[:, g:g + 4, QX:HS])
        for g in range(0, NKC, 4):
            (nc.sync if g % 8 == 0 else nc.scalar).dma_start(
                out=xt_all[:, g:g + 4, HS:S], in_=x_r[:, g:g + 4, HS:S])
        for g in range(0, NKC, 8):  # q1|k1 w cols, first needed mid-B0
            nc.sync.dma_start(
                out=wt_all[:, g:g + 8, WA:3 * CW],
                in_=w_r[:, g:g + 8, WA:3 * CW])
        nc.sync.dma_start(out=mask_sb[:], in_=mask_d[:])
        nc.vector.memset(ones_bf[:], 1.0)
        for h in range(HPC):
            nc.sync.dma_start(out=wo_sb[h][:], in_=wo_d[h * P:(h + 1) * P, :])

        # w_all column offsets after host reorder [q0, k0, v, q1, k1]
        W_OFF = {0: 0, 2: P, 1: 2 * P + CW, 3: 3 * P + CW}
        V_OFF = 2 * P

        def qk_chunks(pool, c, st):
            """q/k feature chunk c (0: q-h0, 1: q-h1, 2: k-h0, 3: k-h1),
            s-tile st, transposed layout + fused rope, as 4 PE micro-steps."""
            dst = (qT, kT)[c // HPC][c % HPC]
            wo_ = W_OFF[c]
            state = {}

            def mm(k0):
                def f():
                    if k0 == 0:
                        state["ps"] = pool.tile([P, QT], F32, name="f")
                    ps = state["ps"]
                    for k in range(k0, k0 + 4):
                        nc.tensor.matmul(
                            ps[:],
                            wt_all[:, k, wo_:wo_ + P],
                            xt_all[:, k, st * QT:(st + 1) * QT],
                            start=(k == 0), stop=(k == NKC - 1),
                        )
                    if k0 == NKC - 4:
                        ps = state["ps"]
                        cs = slice(st * QT, (st + 1) * QT)
                        # Pool cannot read PSUM: both rope multiplies run
                        # on DVE; the all-SBUF add goes to Pool.
                        t1 = rp.tile([P, QT], BF16, name="t1")
                        nc.vector.tensor_mul(t1[:], ps[:], cos_sb[:, cs])
                        t2 = rp.tile([P, QT], BF16, name="t2")
                        nc.vector.tensor_mul(
                            t2[0:HW, :], ps[HW:HD, :], sin_sb[0:HW, cs])
                        nc.vector.tensor_mul(
                            t2[HW:HD, :], ps[0:HW, :], sin_sb[HW:HD, cs])
                        nc.gpsimd.tensor_add(dst[:, cs], t1[:], t2[:])
                return f
            return [mm(k0) for k0 in range(0, NKC, 4)]

        def v_chunks(pool, sc):
            """v s-chunk sc in natural layout, as 2 PE micro-steps."""
            state = {}

            def mm(k0):
                def f():
                    if k0 == 0:
                        state["ps"] = pool.tile([P, QT], F32, name="f")
                    ps = state["ps"]
                    for k in range(k0, k0 + 8):
                        nc.tensor.matmul(
                            ps[:, 0:CW],
                            xt_all[:, k, sc * P:(sc + 1) * P],
                            wt_all[:, k, V_OFF:V_OFF + CW],
                            start=(k == 0), stop=(k == NKC - 1),
                        )
                    if k0 == NKC - 8:
                        for h in range(HPC):
                            nc.vector.tensor_copy(
                                v_sb[h][:, sc * P:(sc + 1) * P],
                                ps[:, h * HD:(h + 1) * HD],
                            )
                return f
            return [mm(0), mm(8)]

        o_r = out_d.rearrange("(o p) s -> p o s", p=P)

        def c_chunks(st, outp, c_ps, alt=False):
            """output-projection pieces for s-tile st, 1 PE micro-step each;
            results stage into 4-oc-wide tiles DMA'd as one transfer."""
            state = {}

            def piece(oc):
                def f():
                    acc = c_ps.tile([P, QT], F32, name="f")
                    for h in range(HPC):
                        nc.tensor.matmul(
                            acc[:],
                            wo_sb[h][:, oc * P:(oc + 1) * P],
                            oT[h][:, st * QT:(st + 1) * QT],
                            start=(h == 0), stop=(h == HPC - 1),
                        )
                    if oc % 4 == 0:
                        state["osb"] = outp.tile([P, 4, QT], BF16, name="osb")
                    osb = state["osb"]
                    # ACT carries the B1 tanh/exp chain: only 1 in 4 copies
                    # goes there — except in the drain round (alt), where
                    # ACT is free and copies alternate 50/50
                    if (oc % 2 == 0) if alt else (oc % 4 == 0):
                        nc.scalar.copy(osb[:, oc % 4, :], acc[:])
                    else:
                        nc.vector.tensor_copy(osb[:, oc % 4, :], acc[:])
                    if alt and oc % 2 == 1:
                        # drain round: ship half-tiles so the final
                        # transfer after the last copy is shorter
                        nc.sync.dma_start(
                            out=o_r[:, oc - 1:oc + 1,
                                    st * QT:(st + 1) * QT],
                            in_=osb[:, (oc % 4) - 1:(oc % 4) + 1, :])
                    elif not alt and oc % 4 == 3:
                        nc.sync.dma_start(
                            out=o_r[:, oc - 3:oc + 1,
                                    st * QT:(st + 1) * QT],
                            in_=osb[:])
                return f
            return [piece(oc) for oc in range(NKC)]

        class Feeder:
            """Doles out independent PE micro-steps to hide ACT latency."""
            def __init__(self):
                self.chunks = []

            def add(self, chunks):
                self.chunks.extend(chunks)

            def step(self, n):
                for _ in range(n):
                    if self.chunks:
                        self.chunks.pop(0)()

            def drain(self):
                self.step(len(self.chunks))

        def emit_attn(h, t, pools, feeder, per_pair):
            s_ps, o_ps, l_ps, thp, pp, np_ = pools
            o_acc = o_ps.tile([P, QT], F32, name="o_acc")
            l_acc = l_ps.tile([1, QT], F32, name="l_acc")
            npair = 2 * t + 2
            q_ap = qT[h][:, t * QT:(t + 1) * QT]

            def emit_pv(pT, p, last):
                for i in range(2):
                    kc = 2 * p + i
                    nc.tensor.matmul(
                        o_acc[:],
                        v_sb[h][:, kc * P:(kc + 1) * P],
                        pT[:, i * QT:(i + 1) * QT],
                        start=(kc == 0), stop=(last and i == 1),
                    )
                    nc.tensor.matmul(
                        l_acc[:], ones_bf[:, 0:1],
                        pT[:, i * QT:(i + 1) * QT],
                        start=(kc == 0), stop=(last and i == 1),
                    )

            prev = None
            for p in range(npair):
                sp = s_ps.tile([P, 2 * QT], F32, name="sp")
                for i in range(2):
                    kc = 2 * p + i
                    nc.tensor.matmul(
                        sp[:, i * QT:(i + 1) * QT],
                        kT[h][:, kc * P:(kc + 1) * P], q_ap,
                        start=True, stop=True,
                    )
                feeder.step(per_pair)
                th = thp.tile([P, 2 * QT], F32, name="th")
                nc.scalar.activation(th[:], sp[:], Tanh, scale=C1)
                pT = pp.tile([P, 2 * QT], BF16, name="pTt")
                nc.scalar.activation(pT[:], th[:], Exp, scale=SOFTCAP)
                # masked pairs are the last two: p==2t (u=0,1), p==2t+1 (u=2,3)
                u0 = 2 * (p - 2 * t)
                if u0 >= 0:
                    nc.vector.tensor_mul(
                        pT[:], pT[:], mask_sb[:, u0 * QT:(u0 + 2) * QT])
                if prev is not None:
                    emit_pv(prev[0], prev[1], last=False)
                prev = (pT, p)
            emit_pv(prev[0], prev[1], last=True)
            recip = np_.tile([1, QT], F32, name="recip")
            nc.vector.reciprocal(recip[:], l_acc[:])
            bcast = np_.tile([P, QT], F32, name="bcast")
            nc.gpsimd.partition_broadcast(bcast[:], recip[:])
            nc.vector.tensor_mul(
                oT[h][:, t * QT:(t + 1) * QT], o_acc[:], bcast[:])

        # ---------- phase A (pre-attention part) ----------
        # head 0's q/k + the first 4 v chunks. Tiles needing only the x
        # first halves come first, k-interleaved within 3-tile windows so
        # the PE tracks the DMA wavefront instead of stalling on one tile.
        def interleave(units):
            out = []
            for step in range(max(len(u) for u in units)):
                for u in units:
                    if step < len(u):
                        out.append(u[step])
            return out

        # The A phase is DMA-bound (~35us of input wavefront), so all v
        # chunks ride along in its PE bubbles, ordered by which x quarter
        # they need.
        with ExitStack() as ctxA:
            qkA = ctxA.enter_context(
                tc.tile_pool(name="qkA", bufs=6, space="PSUM"))
            pre = Feeder()
            pre.add(interleave([qk_chunks(qkA, 0, 0), qk_chunks(qkA, 2, 0)]))
            for sc in range(0, 4):
                pre.add(v_chunks(qkA, sc))
            pre.add(interleave([qk_chunks(qkA, 0, 1), qk_chunks(qkA, 2, 1)]))
            for sc in range(4, 8):
                pre.add(v_chunks(qkA, sc))
            pre.add(interleave([qk_chunks(qkA, 0, 2), qk_chunks(qkA, 2, 2)]))
            for sc in range(8, 12):
                pre.add(v_chunks(qkA, sc))
            pre.add(interleave([qk_chunks(qkA, 0, 3), qk_chunks(qkA, 2, 3)]))
            for sc in range(12, 16):
                pre.add(v_chunks(qkA, sc))
            pre.drain()

        # ---------- phase B0: head-0 attention + A-fill ----------
        # shared fill/output-projection PSUM pool (one tag, 2 banks);
        # created below the B pools so those can close before the drain
        fps = ctx.enter_context(tc.tile_pool(name="fps", bufs=2, space="PSUM"))
        outp = ctx.enter_context(tc.tile_pool(name="out", bufs=4))
        fill = Feeder()
        with ExitStack() as ctxB:
            s_ps = ctxB.enter_context(
                tc.tile_pool(name="s_ps", bufs=2, space="PSUM"))
            o_ps = ctxB.enter_context(
                tc.tile_pool(name="o_ps", bufs=1, space="PSUM"))
            l_ps = ctxB.enter_context(
                tc.tile_pool(name="l_ps", bufs=1, space="PSUM"))
            thp = ctxB.enter_context(tc.tile_pool(name="tanh", bufs=3))
            pp = ctxB.enter_context(tc.tile_pool(name="pT", bufs=3))
            np_ = ctxB.enter_context(tc.tile_pool(name="norm", bufs=4))
            bpools = (s_ps, o_ps, l_ps, thp, pp, np_)

            for st in (0, 1):
                fill.add(qk_chunks(fps, 1, st))
                fill.add(qk_chunks(fps, 3, st))
            for t in range(NQT):
                emit_attn(0, t, bpools, fill, per_pair=3)
            # q1/k1 st2/st3 are first needed by B1 t2/t3: defer them into
            # the otherwise-unfilled B1 t0/t1 windows.
            for st in (2, 3):
                fill.add(qk_chunks(fps, 1, st))
                fill.add(qk_chunks(fps, 3, st))

            # ---------- phase B1 + C: attention + output projection ----
            for t, per in zip(range(NQT), (6, 5, 3, 1)):
                emit_attn(1, t, bpools, fill, per_pair=per)
                if t < NQT - 1:
                    fill.add(c_chunks(t, outp, fps))
        # drain round: B pools are closed, give the last output-projection
        # round a wide PSUM pool so its pieces pipeline
        cD = ctx.enter_context(tc.tile_pool(name="cD", bufs=5, space="PSUM"))
        fill.add(c_chunks(NQT - 1, outp, cD, alt=True))
        fill.drain()


_NC_CACHE = None


def _get_nc():
    global _NC_CACHE
    if _NC_CACHE is None:
        _NC_CACHE = build_nc()
    return _NC_CACHE


def _rope_perm():
    """per-head column permutation de-interleaving rotary pairs"""
    perm = np.zeros(DM, np.int64)
    for h in range(H):
        base = h * HD
        perm[base:base + HD // 2] = base + np.arange(0, HD, 2)
        perm[base + HD // 2:base + HD] = base + np.arange(1, HD, 2)
    return perm


def make_in_maps(x, wq, wk, wv, wo, freqs_cos, freqs_sin):
    x = np.asarray(x, np.float32).reshape(S, DM)
    wq = np.asarray(wq, np.float32)
    wk = np.asarray(wk, np.float32)
    wv = np.asarray(wv, np.float32)
    wo = np.asarray(wo, np.float32)
    xT = np.ascontiguousarray(x.T).astype(NPBF16)
    perm = _rope_perm()
    wq_p = wq[:, perm]
    wk_p = wk[:, perm]
    # transposed rope tables: C = [cosT; cosT], S' = [-sinT; sinT]
    cosT = np.asarray(freqs_cos, np.float32).T  # [64, S]
    sinT = np.asarray(freqs_sin, np.float32).T
    cosT2 = np.concatenate([cosT, cosT], axis=0).astype(NPBF16)
    sinT2 = np.concatenate([-sinT, sinT], axis=0).astype(NPBF16)
    # mask[i, u*QT + j] = 1 if i <= j - 128*u else 0  (keep kj <= qi)
    i_idx = np.arange(P)[:, None]
    j_idx = np.arange(QT)[None, :]
    mask = np.concatenate(
        [(i_idx <= j_idx - P * u) for u in range(4)], axis=1
    ).astype(NPBF16)
    in_maps = []
    for c in range(N_CORES):
        cs = slice(c * CW, (c + 1) * CW)
        h0 = slice(c * CW, c * CW + HD)
        h1 = slice(c * CW + HD, (c + 1) * CW)
        # device column order: [q-h0, k-h0, v, q-h1, k-h1]
        w_all = np.concatenate(
            [wq_p[:, h0], wk_p[:, h0], wv[:, cs],
             wq_p[:, h1], wk_p[:, h1]], axis=1).astype(NPBF16)
        wo_c = np.ascontiguousarray(wo[cs, :]).astype(NPBF16)
        in_maps.append({
            "xT": xT,
            "w_all": np.ascontiguousarray(w_all),
            "wo_c": wo_c,
            "cosT2": cosT2,
            "sinT2": sinT2,
            "mask": mask,
        })
    return in_maps


def assemble_output(results):
    acc = results[0]["outT"].astype(np.float32)
    for r in results[1:]:
        acc += np.asarray(r["outT"]).astype(np.float32)
    return np.ascontiguousarray(acc.T).reshape(1, S, DM).astype(np.float32)


def kernel(x, wq, wk, wv, wo, freqs_cos, freqs_sin):
    nc = _get_nc()
    in_maps = make_in_maps(x, wq, wk, wv, wo, freqs_cos, freqs_sin)
    res = run_bass_kernel_spmd(nc, in_maps, core_ids=list(range(N_CORES)))
    return assemble_output(res.results)


if __name__ == "__main__":
    rng = np.random.default_rng(0)
    ins = {
        "x": rng.standard_normal((1, S, DM), np.float32),
        "wq": rng.standard_normal((DM, DM), np.float32) / np.sqrt(DM),
        "wk": rng.standard_normal((DM, DM), np.float32) / np.sqrt(DM),
        "wv": rng.standard_normal((DM, DM), np.float32) / np.sqrt(DM),
        "wo": rng.standard_normal((DM, DM), np.float32) / np.sqrt(DM),
        "freqs_cos": rng.standard_normal((S, HD // 2), np.float32),
        "freqs_sin": rng.standard_normal((S, HD // 2), np.float32),
    }
    out = kernel(**ins)
    print("out", out.shape, out.dtype, np.abs(out).mean())
